# revision 1
# baseline (speedup 1.0000x reference)
"""Trainium2 Bass kernel for DocumentClassificationGNN (3-layer GCN + BN/ReLU +
global mean pool + MLP head), distributed over 8 NeuronCores.

Strategy (node/graph parallel, per the sharding hint):
  - Nodes are assigned to (core, slot) sorted by in-degree so every core/tile
    carries a balanced edge load.  Edges are partitioned by DESTINATION core so
    the segment-sum scatter is device-local.
  - Per layer: a dense GEMM produces a node-major fp16 feature table that the
    host replicates to all cores ("all-gather" through the host between
    launches); each core gathers its in-edge source rows with dma_gather and
    scatter-adds them into PSUM with one-hot (norm-valued) matmuls.
  - deg^-1/2 edge norms are folded into the one-hot matrices; conv bias + BN +
    ReLU are fused into a single per-feature scalar-engine activation; the
    third conv layer feeds a per-tile onehot(batch) pooling matmul.
  - Device output: per-core pooled partial sums [64, 128].  Host: sum, +n_g*b3,
    divide by counts, tiny classifier MLP.

Programs (3 compiles, 4 launches):
  A : xw1 = x @ W1                                  -> T1 table shard
  BC: Y = scatter(T); h' = relu(BN(Y + b)); T' = h' @ W_next   (used twice)
  D : Y3 = scatter(T3); pooled_partial = onehot(batch)^T @ Y3
"""

import hashlib
import numpy as np
from contextlib import ExitStack

import concourse.bass as bass
import concourse.bacc as bacc
import concourse.tile as tile
from concourse import mybir
from concourse.bass_utils import run_bass_kernel_spmd
from concourse.masks import make_identity

P = 128
NCORES = 8
N = 50000
D_IN = 256
H = 128
NGRAPH = 64
SLOTS = 6272            # 49 tiles of 128 slots per core (6250 real nodes + pad)
TILES = SLOTS // P      # 49
RAW = NCORES * SLOTS    # 50176
LOB = 32767             # table row 32767 is the lo-region zero row
TAB = RAW + 2           # +2 zero rows (lo @32767, hi @TAB-1)
ZLO = LOB               # lo-local zero row index
ZHI_LOCAL = TAB - 1 - 32768   # hi-local zero row index
GS = 7                  # dst tiles per gather group
NGROUPS = TILES // GS   # 7
BN_EPS = 1e-5

F16 = mybir.dt.float16
F32 = mybir.dt.float32
F32R = mybir.dt.float32r
I16 = mybir.dt.int16

# module-level knobs / perf results (test.py pokes these)
TRACE = False
LAST_EXEC_NS = []       # per-launch exec_time_ns (when TRACE)

_PLAN_CACHE = {}
_PROG_CACHE = {}


# ---------------------------------------------------------------- host prep --

def _wrap_idx(flat):
    """dma_gather index layout: idx i -> [i%16, i//16], replicated to 128 parts."""
    n = len(flat)
    assert n % 16 == 0
    arr = np.asarray(flat, dtype=np.int16).reshape(n // 16, 16).T.copy()
    return np.tile(arr, (8, 1))


class _Plan:
    pass


def _make_plan(edge_index, batch, x):
    pl = _Plan()
    src = np.asarray(edge_index[0], dtype=np.int64)
    dst = np.asarray(edge_index[1], dtype=np.int64)
    batch = np.asarray(batch, dtype=np.int64)

    deg = np.bincount(dst, minlength=N).astype(np.int64) + 1
    dinv = (1.0 / np.sqrt(deg)).astype(np.float32)

    order = np.argsort(-deg, kind="stable")
    rank = np.empty(N, dtype=np.int64)
    rank[order] = np.arange(N)
    core_of = rank % NCORES
    slot_of = rank // NCORES
    raw_of = core_of * SLOTS + slot_of
    grow_of = raw_of + (raw_of >= LOB)      # table row per node

    # edges + self loops
    es = np.concatenate([src, np.arange(N)])
    ed = np.concatenate([dst, np.arange(N)])
    enorm = (dinv[es] * dinv[ed]).astype(np.float32)
    ecore = core_of[ed]
    eslot = slot_of[ed]
    etile = eslot // P
    edstloc = eslot % P
    esg = grow_of[es]
    islo = esg < LOB

    # per-core sorted segment arrays
    NSEG = TILES * 2   # segment id: 2*tile + (0 lo / 1 hi)
    per_core = []
    seg_counts = np.zeros((NCORES, NSEG), dtype=np.int64)
    for c in range(NCORES):
        m = ecore == c
        seg = etile[m] * 2 + (~islo[m]).astype(np.int64)
        o2 = np.lexsort((esg[m], seg))
        d = {
            "seg": seg[o2],
            "dstloc": edstloc[m][o2],
            "norm": enorm[m][o2],
            "esg": esg[m][o2],
        }
        seg_counts[c] = np.bincount(d["seg"], minlength=NSEG)
        per_core.append(d)

    # chunk plan: per tile, lo/hi chunk counts = max over cores
    CLO = np.maximum(1, np.ceil(seg_counts[:, 0::2].max(axis=0) / P)).astype(int)
    CHI = np.maximum(1, np.ceil(seg_counts[:, 1::2].max(axis=0) / P)).astype(int)
    # chunk order: group-major; within group: all lo chunks (tile order), then hi
    seg_chunk_start = np.zeros(NSEG, dtype=np.int64)   # global chunk idx per seg
    grp_clo = np.zeros(NGROUPS, dtype=np.int64)
    grp_chi = np.zeros(NGROUPS, dtype=np.int64)
    gcb = np.zeros(NGROUPS + 1, dtype=np.int64)
    for g in range(NGROUPS):
        ts = range(g * GS, (g + 1) * GS)
        grp_clo[g] = sum(CLO[t] for t in ts)
        grp_chi[g] = sum(CHI[t] for t in ts)
        ofs = gcb[g]
        for t in ts:
            seg_chunk_start[2 * t] = ofs
            ofs += CLO[t]
        for t in ts:
            seg_chunk_start[2 * t + 1] = ofs
            ofs += CHI[t]
        gcb[g + 1] = ofs
    CTOT = int(gcb[-1])

    # per-chunk default fill (pads): lo chunks -> ZLO, hi chunks -> absolute hi zero
    chunk_is_hi = np.zeros(CTOT, dtype=bool)
    for t in range(TILES):
        s = seg_chunk_start[2 * t + 1]
        chunk_is_hi[s:s + CHI[t]] = True

    pl.cores = []
    for c in range(NCORES):
        d = per_core[c]
        npad = CTOT * P
        dstloc_pad = np.zeros(npad, dtype=np.float16)
        norm_pad = np.zeros(npad, dtype=np.float16)
        row_pad = np.where(np.repeat(chunk_is_hi, P), TAB - 1, ZLO).astype(np.int64)
        # position of each real edge
        cnt = seg_counts[c]
        seg_first = np.concatenate([[0], np.cumsum(cnt)[:-1]])
        within = np.arange(len(d["seg"])) - seg_first[d["seg"]]
        pos = seg_chunk_start[d["seg"]] * P + within
        dstloc_pad[pos] = d["dstloc"].astype(np.float16)
        norm_pad[pos] = d["norm"].astype(np.float16)
        row_pad[pos] = d["esg"]

        # gather index arrays (lo then hi, group-major)
        lo_parts, hi_parts = [], []
        for g in range(NGROUPS):
            a = gcb[g] * P
            b = a + grp_clo[g] * P
            e = gcb[g + 1] * P
            lo_parts.append(row_pad[a:b])
            hi_parts.append(row_pad[b:e] - 32768)
        lo_flat = np.concatenate(lo_parts)
        hi_flat = np.concatenate(hi_parts)
        assert lo_flat.min() >= 0 and lo_flat.max() <= LOB
        assert hi_flat.min() >= 0 and hi_flat.max() <= ZHI_LOCAL

        core = {
            "idxlo": _wrap_idx(lo_flat),
            "idxhi": _wrap_idx(hi_flat),
            "dstloc": dstloc_pad.reshape(CTOT, P).T.copy(),
            "norm": norm_pad.reshape(CTOT, P).T.copy(),
        }
        pl.cores.append(core)

    # group gather call metadata (columns into wrapped idx tensors)
    pl.lo_cols = int(grp_clo.sum() * P // 16)
    pl.hi_cols = int(grp_chi.sum() * P // 16)
    lo_c0 = np.concatenate([[0], np.cumsum(grp_clo * 8)])
    hi_c0 = np.concatenate([[0], np.cumsum(grp_chi * 8)])
    pl.groups = []
    for g in range(NGROUPS):
        tiles = []
        for t in range(g * GS, (g + 1) * GS):
            lo_local = seg_chunk_start[2 * t] - gcb[g]
            hi_local = seg_chunk_start[2 * t + 1] - gcb[g]
            chunks = [(int(lo_local + j), int(seg_chunk_start[2 * t] + j))
                      for j in range(CLO[t])]
            chunks += [(int(hi_local + j), int(seg_chunk_start[2 * t + 1] + j))
                       for j in range(CHI[t])]
            tiles.append(chunks)
        pl.groups.append({
            "nclo": int(grp_clo[g]), "nchi": int(grp_chi[g]),
            "lo_col0": int(lo_c0[g]), "hi_col0": int(hi_c0[g]),
            "tiles": tiles,
        })
    pl.CTOT = CTOT

    # slot -> node map, batch values, xT shards, table row map
    node_at = np.full((NCORES, SLOTS), -1, dtype=np.int64)
    node_at[core_of, slot_of] = np.arange(N)
    bv = np.full((NCORES, SLOTS), 99.0, dtype=np.float16)
    valid = node_at >= 0
    bv[valid] = batch[node_at[valid]].astype(np.float16)
    for c in range(NCORES):
        pl.cores[c]["batchval"] = bv[c].reshape(TILES, P).T.copy()  # [128, 49]
        xt = np.zeros((D_IN, SLOTS), dtype=np.float32)
        v = valid[c]
        xt[:, v] = np.asarray(x, dtype=np.float32)[node_at[c][v]].T
        pl.cores[c]["xT"] = xt

    rm = np.arange(RAW, dtype=np.int64)
    pl.rowmap = (rm + (rm >= LOB)).reshape(NCORES, SLOTS)
    pl.counts = np.bincount(batch, minlength=NGRAPH).astype(np.float32)
    pl.iota = np.arange(P, dtype=np.float16).reshape(1, P)
    pl.key = (tuple(CLO), tuple(CHI))
    return pl


# ---------------------------------------------------------- program builders --

def _gemm_to_table(nc, ctx, tc, k_tiles, o_T, ident32, tps_pool=None):
    """xw^T[fout, slot] = sum_k lhsT_k^T @ rhs_k ; transpose+cast -> o_T rows.

    k_tiles: list of (lhsT_sbuf_f32 [128, 128], rhs_slab [128, SLOTS] f32).
    """
    gps_pool = ctx.enter_context(tc.tile_pool(name="gemm_ps", bufs=2, space="PSUM"))
    gcp_pool = ctx.enter_context(tc.tile_pool(name="gemm_cp", bufs=2))
    if tps_pool is None:
        tps_pool = ctx.enter_context(
            tc.tile_pool(name="gemm_tp", bufs=2, space="PSUM"))
    to_pool = ctx.enter_context(tc.tile_pool(name="gemm_to", bufs=3))
    CH = 512
    for o in range(0, SLOTS, CH):
        w = min(CH, SLOTS - o)
        gps = gps_pool.tile([P, CH], F32, space="PSUM")
        for ki, (lhsT, rhs) in enumerate(k_tiles):
            nc.tensor.matmul(
                out=gps[:, :w],
                lhsT=lhsT[:],
                rhs=rhs[:, o:o + w],
                start=(ki == 0), stop=(ki == len(k_tiles) - 1),
            )
        gcp = gcp_pool.tile([P, CH], F32)
        nc.vector.tensor_copy(out=gcp[:, :w], in_=gps[:, :w])
        for s in range(0, w, P):
            tp = tps_pool.tile([P, P], F32, space="PSUM", tag="tp")
            nc.tensor.transpose(out=tp[:], in_=gcp[:, s:s + P], identity=ident32[:])
            to = to_pool.tile([P, H], F16)
            nc.vector.tensor_copy(out=to[:], in_=tp[:])
            nc.sync.dma_start(out=o_T[o + s:o + s + P, :], in_=to[:])


def _build_A():
    nc = bacc.Bacc("TRN2", target_bir_lowering=False, debug=False, num_devices=NCORES)
    i_xT = nc.dram_tensor("xT", [D_IN, SLOTS], F32, kind="ExternalInput").ap()
    i_W = nc.dram_tensor("W", [D_IN, H], F32, kind="ExternalInput").ap()
    o_T = nc.dram_tensor("Tout", [SLOTS, H], F16, kind="ExternalOutput").ap()
    with tile.TileContext(nc) as tc:
        with ExitStack() as ctx:
            const = ctx.enter_context(tc.tile_pool(name="const", bufs=1))
            ident32 = const.tile([P, P], F32)
            make_identity(nc, ident32[:])
            w0 = const.tile([P, H], F32)
            nc.sync.dma_start(out=w0[:], in_=i_W[0:P, :])
            w1 = const.tile([P, H], F32)
            nc.sync.dma_start(out=w1[:], in_=i_W[P:2 * P, :])
            x0 = const.tile([P, SLOTS], F32)
            nc.sync.dma_start(out=x0[:], in_=i_xT[0:P, :])
            x1 = const.tile([P, SLOTS], F32)
            nc.sync.dma_start(out=x1[:], in_=i_xT[P:2 * P, :])
            _gemm_to_table(nc, ctx, tc, [(w0, x0), (w1, x1)], o_T, ident32)
    nc.compile()
    return nc


def _scatter_body(nc, ctx, tc, pl, i_T, consume_tile):
    """Shared gather + one-hot matmul scatter loop.

    consume_tile(t, ypsum, iota_sb) handles the per-tile PSUM result.
    """
    const = ctx.enter_context(tc.tile_pool(name="sc_const", bufs=1))
    stage = ctx.enter_context(tc.tile_pool(name="staging", bufs=2))
    st_pool = ctx.enter_context(tc.tile_pool(name="st", bufs=6))
    yp_pool = ctx.enter_context(tc.tile_pool(name="yps", bufs=3, space="PSUM"))

    i_idxlo = nc.dram_tensor("idxlo", [P, pl.lo_cols], I16, kind="ExternalInput").ap()
    i_idxhi = nc.dram_tensor("idxhi", [P, pl.hi_cols], I16, kind="ExternalInput").ap()
    i_dstloc = nc.dram_tensor("dstloc", [P, pl.CTOT], F16, kind="ExternalInput").ap()
    i_norm = nc.dram_tensor("norm", [P, pl.CTOT], F16, kind="ExternalInput").ap()
    i_iota = nc.dram_tensor("iota", [1, P], F16, kind="ExternalInput").ap()

    idxlo_sb = const.tile([P, pl.lo_cols], I16)
    nc.sync.dma_start(out=idxlo_sb[:], in_=i_idxlo[:])
    idxhi_sb = const.tile([P, pl.hi_cols], I16)
    nc.sync.dma_start(out=idxhi_sb[:], in_=i_idxhi[:])
    dstloc_sb = const.tile([P, pl.CTOT], F16)
    nc.sync.dma_start(out=dstloc_sb[:], in_=i_dstloc[:])
    norm_sb = const.tile([P, pl.CTOT], F16)
    nc.sync.dma_start(out=norm_sb[:], in_=i_norm[:])
    iota_sb = const.tile([P, P], F16)
    nc.sync.dma_start(out=iota_sb[:], in_=i_iota.to_broadcast([P, P]))

    MAXCH = 8  # 1024 indices per dma_gather call (HW SWDGE ring limit)
    for g, grp in enumerate(pl.groups):
        nclo, nchi = grp["nclo"], grp["nchi"]
        staging = stage.tile([P, nclo + nchi, H], F16, tag="staging")
        for o in range(0, nclo, MAXCH):
            n = min(MAXCH, nclo - o)
            c0 = grp["lo_col0"] + o * 8
            nc.gpsimd.dma_gather(
                out_ap=staging[:, o:o + n, :], in_ap=i_T[:],
                idxs_ap=idxlo_sb[:, c0:c0 + n * 8],
                num_idxs=n * P, num_idxs_reg=n * P, elem_size=H)
        for o in range(0, nchi, MAXCH):
            n = min(MAXCH, nchi - o)
            c0 = grp["hi_col0"] + o * 8
            nc.gpsimd.dma_gather(
                out_ap=staging[:, nclo + o:nclo + o + n, :], in_ap=i_T[32768:, :],
                idxs_ap=idxhi_sb[:, c0:c0 + n * 8],
                num_idxs=n * P, num_idxs_reg=n * P, elem_size=H)
        for ti, chunks in enumerate(grp["tiles"]):
            t = g * GS + ti
            ypsum = yp_pool.tile([P, H], F32, space="PSUM")
            for j, (sp, gc) in enumerate(chunks):
                s_t = st_pool.tile([P, P], F16, tag="s_t")
                nc.vector.scalar_tensor_tensor(
                    out=s_t[:], in0=iota_sb[:],
                    scalar=dstloc_sb[:, gc:gc + 1],
                    in1=norm_sb[:, gc:gc + 1].to_broadcast([P, P]),
                    op0=mybir.AluOpType.is_equal, op1=mybir.AluOpType.mult)
                nc.tensor.matmul(out=ypsum[:], lhsT=s_t[:], rhs=staging[:, sp, :],
                                 start=(j == 0), stop=(j == len(chunks) - 1))
            consume_tile(t, ypsum, iota_sb)
    return iota_sb


def _vec_input(nc, const, name):
    ap = nc.dram_tensor(name, [H, 1], F32, kind="ExternalInput").ap()
    sb = const.tile([H, 1], F32, tag=f"vec_{name}")
    nc.sync.dma_start(out=sb[:], in_=ap[:])
    return sb


def _build_BC(pl):
    nc = bacc.Bacc("TRN2", target_bir_lowering=False, debug=False, num_devices=NCORES)
    i_T = nc.dram_tensor("T", [TAB, H], F16, kind="ExternalInput").ap()
    i_W = nc.dram_tensor("W", [H, H], F32, kind="ExternalInput").ap()
    o_T = nc.dram_tensor("Tout", [SLOTS, H], F16, kind="ExternalOutput").ap()
    with tile.TileContext(nc) as tc:
        with ExitStack() as ctx:
            const = ctx.enter_context(tc.tile_pool(name="bc_const", bufs=1))
            ycp_pool = ctx.enter_context(tc.tile_pool(name="ycp", bufs=3))
            tps_pool = ctx.enter_context(tc.tile_pool(name="tps", bufs=2, space="PSUM"))

            b_sb = _vec_input(nc, const, "bvec")
            g_sb = _vec_input(nc, const, "bn_g")
            bb_sb = _vec_input(nc, const, "bn_b")
            m_sb = _vec_input(nc, const, "bn_m")
            v_sb = _vec_input(nc, const, "bn_v")
            # scale = g / sqrt(v+eps); bias = (b - m)*scale + beta
            eps = const.tile([H, 1], F32)
            nc.vector.memset(eps[:], BN_EPS)
            sq = const.tile([H, 1], F32)
            nc.scalar.activation(out=sq[:], in_=v_sb[:],
                                 func=mybir.ActivationFunctionType.Sqrt,
                                 bias=eps[:], scale=1.0)
            rs = const.tile([H, 1], F32)
            nc.vector.reciprocal(out=rs[:], in_=sq[:])
            scale = const.tile([H, 1], F32)
            nc.vector.tensor_mul(out=scale[:], in0=rs[:], in1=g_sb[:])
            bias = const.tile([H, 1], F32)
            nc.vector.tensor_sub(out=bias[:], in0=b_sb[:], in1=m_sb[:])
            nc.vector.tensor_mul(out=bias[:], in0=bias[:], in1=scale[:])
            nc.vector.tensor_add(out=bias[:], in0=bias[:], in1=bb_sb[:])

            ident32 = const.tile([P, P], F32)
            make_identity(nc, ident32[:])
            w_sb = const.tile([H, H], F32)
            nc.sync.dma_start(out=w_sb[:], in_=i_W[:])
            hT = const.tile([P, SLOTS], F32)

            def consume(t, ypsum, _iota):
                ycp = ycp_pool.tile([P, H], F32)
                nc.vector.tensor_copy(out=ycp[:], in_=ypsum[:])
                tp = tps_pool.tile([P, P], F32, space="PSUM")
                nc.tensor.transpose(out=tp[:], in_=ycp[:], identity=ident32[:])
                nc.scalar.activation(
                    out=hT[:, t * P:(t + 1) * P], in_=tp[:],
                    func=mybir.ActivationFunctionType.Relu,
                    bias=bias[:], scale=scale[:])

            _scatter_body(nc, ctx, tc, pl, i_T, consume)
            _gemm_to_table(nc, ctx, tc, [(w_sb, hT)], o_T, ident32,
                           tps_pool=tps_pool)
    nc.compile()
    return nc


def _build_D(pl):
    nc = bacc.Bacc("TRN2", target_bir_lowering=False, debug=False, num_devices=NCORES)
    i_T = nc.dram_tensor("T", [TAB, H], F16, kind="ExternalInput").ap()
    i_bv = nc.dram_tensor("batchval", [P, TILES], F16, kind="ExternalInput").ap()
    o_pool = nc.dram_tensor("pool", [NGRAPH, H], F32, kind="ExternalOutput").ap()
    with tile.TileContext(nc) as tc:
        with ExitStack() as ctx:
            const = ctx.enter_context(tc.tile_pool(name="d_const", bufs=1))
            h3_pool = ctx.enter_context(tc.tile_pool(name="h3", bufs=3))
            oh_pool = ctx.enter_context(tc.tile_pool(name="oh", bufs=3))
            pp_pool = ctx.enter_context(tc.tile_pool(name="pp", bufs=2, space="PSUM"))

            bv_sb = const.tile([P, TILES], F16)
            nc.sync.dma_start(out=bv_sb[:], in_=i_bv[:])
            pool_acc = const.tile([NGRAPH, H], F32)
            nc.vector.memset(pool_acc[:], 0.0)

            def consume(t, ypsum, iota_sb):
                h3 = h3_pool.tile([P, H], F16)
                nc.vector.tensor_copy(out=h3[:], in_=ypsum[:])
                oh = oh_pool.tile([P, NGRAPH], F16)
                nc.vector.tensor_tensor(
                    out=oh[:], in0=bv_sb[:, t:t + 1].to_broadcast([P, NGRAPH]),
                    in1=iota_sb[:, :NGRAPH], op=mybir.AluOpType.is_equal)
                pp = pp_pool.tile([NGRAPH, H], F32, space="PSUM")
                nc.tensor.matmul(out=pp[:], lhsT=oh[:], rhs=h3[:],
                                 start=True, stop=True)
                nc.vector.tensor_add(out=pool_acc[:], in0=pool_acc[:], in1=pp[:])

            _scatter_body(nc, ctx, tc, pl, i_T, consume)
            nc.sync.dma_start(out=o_pool[:], in_=pool_acc[:])
    nc.compile()
    return nc


# ------------------------------------------------------------------- driver --

def _run(nc, in_maps):
    res = run_bass_kernel_spmd(nc, in_maps, core_ids=list(range(NCORES)),
                               trace=TRACE)
    if TRACE:
        LAST_EXEC_NS.append(res.exec_time_ns)
    return res.results


def _assemble_table(pl, shards):
    T = np.zeros((TAB, H), dtype=np.float16)
    for c in range(NCORES):
        T[pl.rowmap[c]] = shards[c]
    return T


def kernel(**inputs):
    ins = {k: np.asarray(v) for k, v in inputs.items()}
    key = hashlib.sha1(
        ins["edge_index"].tobytes() + ins["batch"].tobytes()
    ).hexdigest()
    if key not in _PLAN_CACHE:
        _PLAN_CACHE[key] = _make_plan(ins["edge_index"], ins["batch"], ins["x"])
    pl = _PLAN_CACHE[key]

    pk = pl.key
    if pk not in _PROG_CACHE:
        _PROG_CACHE[pk] = {
            "A": _build_A(),
            "BC": _build_BC(pl),
            "D": _build_D(pl),
        }
    progs = _PROG_CACHE[pk]

    LAST_EXEC_NS.clear()
    W1 = ins["W1"].astype(np.float32)
    # Launch A: T1 = x @ W1
    resA = _run(progs["A"], [
        {"xT": pl.cores[c]["xT"], "W": W1} for c in range(NCORES)
    ])
    T1 = _assemble_table(pl, [r["Tout"] for r in resA])

    def meta(c):
        cc = pl.cores[c]
        return {"idxlo": cc["idxlo"], "idxhi": cc["idxhi"],
                "dstloc": cc["dstloc"], "norm": cc["norm"], "iota": pl.iota}

    def vec(name):
        return ins[name].astype(np.float32).reshape(H, 1)

    # Launch B: layer-1 scatter + BN1/ReLU + @W2
    resB = _run(progs["BC"], [
        {**meta(c), "T": T1, "W": ins["W2"].astype(np.float32),
         "bvec": vec("b1"), "bn_g": vec("bn1_g"), "bn_b": vec("bn1_b"),
         "bn_m": vec("bn1_m"), "bn_v": vec("bn1_v")} for c in range(NCORES)
    ])
    T2 = _assemble_table(pl, [r["Tout"] for r in resB])

    # Launch C: layer-2 scatter + BN2/ReLU + @W3
    resC = _run(progs["BC"], [
        {**meta(c), "T": T2, "W": ins["W3"].astype(np.float32),
         "bvec": vec("b2"), "bn_g": vec("bn2_g"), "bn_b": vec("bn2_b"),
         "bn_m": vec("bn2_m"), "bn_v": vec("bn2_v")} for c in range(NCORES)
    ])
    T3 = _assemble_table(pl, [r["Tout"] for r in resC])

    # Launch D: layer-3 scatter + pooling partials
    resD = _run(progs["D"], [
        {**meta(c), "T": T3, "batchval": pl.cores[c]["batchval"]}
        for c in range(NCORES)
    ])
    pooled_sum = np.sum([r["pool"] for r in resD], axis=0).astype(np.float64)

    counts = pl.counts.astype(np.float64)
    pooled_sum += counts[:, None] * ins["b3"].astype(np.float64)[None, :]
    pooled = pooled_sum / np.maximum(counts, 1.0)[:, None]

    z = np.maximum(pooled @ ins["Wc1"].astype(np.float64)
                   + ins["bc1"].astype(np.float64), 0.0)
    out = z @ ins["Wc2"].astype(np.float64) + ins["bc2"].astype(np.float64)
    return out.astype(np.float32)



# revision 8
# speedup vs baseline: 1.2333x; 1.2333x over previous
"""Trainium2 Bass kernel for DocumentClassificationGNN (3-layer GCN + BN/ReLU +
global mean pool + MLP head), distributed over 8 NeuronCores.

Strategy (node/graph parallel, per the sharding hint):
  - Nodes are assigned to (core, slot) sorted by in-degree so every core/tile
    carries a balanced edge load.  Edges are partitioned by DESTINATION core so
    the segment-sum scatter is device-local.
  - Per layer: a dense GEMM produces a node-major fp16 feature table that the
    host replicates to all cores ("all-gather" through the host between
    launches); each core gathers its in-edge source rows with dma_gather and
    scatter-adds them into PSUM with one-hot matmuls.
  - The symmetric norm deg^-1/2[src]*deg^-1/2[dst] is SEPARABLE: table rows
    are pre-scaled by dinv[src] at write time and the scatter output is
    post-scaled by dinv[dst], so the one-hot matrices are pure 0/1 and are
    generated in batched DVE is_equal ops (2-byte fast path) with the chunk
    dim innermost: s_t[p, j, c].
  - Self-loops never enter the edge stream: each tile's own table rows are
    bulk-loaded and added via one identity matmul (contribution dinv_d*T'[d]).
  - conv bias + BN + ReLU fuse into one scalar-engine activation; GEMMs run in
    bf16; launch D does per-tile onehot(batch) pooling accumulated in one PSUM
    bank.
  - Device output: per-core pooled partial sums [64, 128].  Host: sum, +n_g*b3,
    divide by counts, tiny classifier MLP.

Programs (3 compiles, 4 launches):
  A : T1 = dinv * (x @ W1)                          -> T1 table shard
  BC: Y = scatter(T); h' = relu(BN(dinv*Y + b)); T' = dinv * (h' @ W_next)
  D : Y3 = scatter(T3); pooled_partial = onehot(batch)^T @ (dinv*Y3)
"""

import hashlib
import numpy as np
from contextlib import ExitStack

import ml_dtypes

import concourse.bass as bass
import concourse.bacc as bacc
import concourse.tile as tile
from concourse import mybir
from concourse.bass_utils import run_bass_kernel_spmd
from concourse.masks import make_identity

P = 128
NCORES = 8
N = 50000
D_IN = 256
H = 128
NGRAPH = 64
SLOTS = 6272            # 49 tiles of 128 slots per core (6250 real nodes + pad)
TILES = SLOTS // P      # 49
RAW = NCORES * SLOTS    # 50176
LOB = 32767             # table row 32767 is the lo-region zero row
TAB = RAW + 2           # +2 zero rows (lo @32767, hi @TAB-1)
ZLO = LOB               # lo-local zero row index
ZHI_LOCAL = TAB - 1 - 32768   # hi-local zero row index
GS = 7                  # dst tiles per gather group
NGROUPS = TILES // GS   # 7
BN_EPS = 1e-5

SCRATCH = 16384         # SWDGE ring: 16384/16 = 1024 descriptors per queue
MAXCH = 8               # chunks per dma_gather call (8*128 = 1024, HW limit)
NQ = 2                  # SWDGE queues (desc-gen pipelines against transfer)

F16 = mybir.dt.float16
BF16 = mybir.dt.bfloat16
F32 = mybir.dt.float32
I16 = mybir.dt.int16
BF16_NP = ml_dtypes.bfloat16

# module-level knobs / perf results (test.py pokes these)
TRACE = False
LAST_EXEC_NS = []       # per-launch exec_time_ns (when TRACE)

_PLAN_CACHE = {}
_PROG_CACHE = {}


# ---------------------------------------------------------------- host prep --

def _wrap_idx(flat):
    """dma_gather index layout: idx i -> [i%16, i//16], replicated to 128 parts."""
    n = len(flat)
    assert n % 16 == 0
    arr = np.asarray(flat, dtype=np.int16).reshape(n // 16, 16).T.copy()
    return np.tile(arr, (8, 1))


class _Plan:
    pass


def _make_plan(edge_index, batch, x):
    pl = _Plan()
    src = np.asarray(edge_index[0], dtype=np.int64)
    dst = np.asarray(edge_index[1], dtype=np.int64)
    batch = np.asarray(batch, dtype=np.int64)

    deg = np.bincount(dst, minlength=N).astype(np.int64) + 1
    dinv = (1.0 / np.sqrt(deg)).astype(np.float32)

    order = np.argsort(-deg, kind="stable")
    rank = np.empty(N, dtype=np.int64)
    rank[order] = np.arange(N)
    core_of = rank % NCORES
    slot_of = rank // NCORES
    raw_of = core_of * SLOTS + slot_of
    grow_of = raw_of + (raw_of >= LOB)      # table row per node

    # real edges only: self-loops are handled by the per-tile identity matmul
    es, ed = src, dst
    ecore = core_of[ed]
    eslot = slot_of[ed]
    etile = eslot // P
    edstloc = eslot % P
    esg = grow_of[es]
    islo = esg < LOB

    # per-core sorted segment arrays
    NSEG = TILES * 2   # segment id: 2*tile + (0 lo / 1 hi)
    per_core = []
    seg_counts = np.zeros((NCORES, NSEG), dtype=np.int64)
    for c in range(NCORES):
        m = ecore == c
        seg = etile[m] * 2 + (~islo[m]).astype(np.int64)
        o2 = np.lexsort((esg[m], seg))
        d = {
            "seg": seg[o2],
            "dstloc": edstloc[m][o2],
            "esg": esg[m][o2],
        }
        seg_counts[c] = np.bincount(d["seg"], minlength=NSEG)
        per_core.append(d)

    # chunk plan: per tile, lo/hi chunk counts = max over cores
    CLO = np.ceil(seg_counts[:, 0::2].max(axis=0) / P).astype(int)
    CHI = np.ceil(seg_counts[:, 1::2].max(axis=0) / P).astype(int)
    # chunk order: group-major; within group: all lo chunks (tile order), then hi
    seg_chunk_start = np.zeros(NSEG, dtype=np.int64)   # global chunk idx per seg
    grp_clo = np.zeros(NGROUPS, dtype=np.int64)
    grp_chi = np.zeros(NGROUPS, dtype=np.int64)
    gcb = np.zeros(NGROUPS + 1, dtype=np.int64)
    for g in range(NGROUPS):
        ts = range(g * GS, (g + 1) * GS)
        grp_clo[g] = sum(CLO[t] for t in ts)
        grp_chi[g] = sum(CHI[t] for t in ts)
        ofs = gcb[g]
        for t in ts:
            seg_chunk_start[2 * t] = ofs
            ofs += CLO[t]
        for t in ts:
            seg_chunk_start[2 * t + 1] = ofs
            ofs += CHI[t]
        gcb[g + 1] = ofs
    CTOT = int(gcb[-1])

    # per-chunk default fill (pads): lo chunks -> ZLO, hi chunks -> absolute hi zero
    chunk_is_hi = np.zeros(CTOT, dtype=bool)
    for t in range(TILES):
        s = seg_chunk_start[2 * t + 1]
        chunk_is_hi[s:s + CHI[t]] = True

    pl.cores = []
    for c in range(NCORES):
        d = per_core[c]
        npad = CTOT * P
        dstloc_pad = np.zeros(npad, dtype=np.float16)
        row_pad = np.where(np.repeat(chunk_is_hi, P), TAB - 1, ZLO).astype(np.int64)
        # position of each real edge
        cnt = seg_counts[c]
        seg_first = np.concatenate([[0], np.cumsum(cnt)[:-1]])
        within = np.arange(len(d["seg"])) - seg_first[d["seg"]]
        pos = seg_chunk_start[d["seg"]] * P + within
        dstloc_pad[pos] = d["dstloc"].astype(np.float16)
        row_pad[pos] = d["esg"]

        # gather index arrays (lo then hi, group-major)
        lo_parts, hi_parts = [], []
        for g in range(NGROUPS):
            a = gcb[g] * P
            b = a + grp_clo[g] * P
            e = gcb[g + 1] * P
            lo_parts.append(row_pad[a:b])
            hi_parts.append(row_pad[b:e] - 32768)
        lo_flat = np.concatenate(lo_parts)
        hi_flat = np.concatenate(hi_parts)
        assert lo_flat.min() >= 0 and lo_flat.max() <= LOB
        assert hi_flat.min() >= 0 and hi_flat.max() <= ZHI_LOCAL

        core = {
            "idxlo": _wrap_idx(lo_flat),
            "idxhi": _wrap_idx(hi_flat),
            "dstloc": dstloc_pad.reshape(CTOT, P).T.copy(),
        }
        pl.cores.append(core)

    # group gather call metadata (columns into wrapped idx tensors)
    pl.lo_cols = int(grp_clo.sum() * P // 16)
    pl.hi_cols = int(grp_chi.sum() * P // 16)
    lo_c0 = np.concatenate([[0], np.cumsum(grp_clo * 8)])
    hi_c0 = np.concatenate([[0], np.cumsum(grp_chi * 8)])
    pl.groups = []
    for g in range(NGROUPS):
        tiles = []
        for t in range(g * GS, (g + 1) * GS):
            lo_local = int(seg_chunk_start[2 * t] - gcb[g])
            hi_local = int(seg_chunk_start[2 * t + 1] - gcb[g])
            tiles.append({
                "clo": int(CLO[t]), "chi": int(CHI[t]),
                "sp_lo": lo_local, "sp_hi": hi_local,
                "gc_lo": int(seg_chunk_start[2 * t]),
                "gc_hi": int(seg_chunk_start[2 * t + 1]),
            })
        pl.groups.append({
            "nclo": int(grp_clo[g]), "nchi": int(grp_chi[g]),
            "lo_col0": int(lo_c0[g]), "hi_col0": int(hi_c0[g]),
            "tiles": tiles,
        })
    pl.CTOT = CTOT
    pl.NCHMAX = int(max(CLO.max(), CHI.max()))

    # slot -> node map, batch values, dinv per slot, xT shards, table row map
    node_at = np.full((NCORES, SLOTS), -1, dtype=np.int64)
    node_at[core_of, slot_of] = np.arange(N)
    bv = np.full((NCORES, SLOTS), 99.0, dtype=np.float16)
    dv = np.ones((NCORES, SLOTS), dtype=np.float32)
    valid = node_at >= 0
    bv[valid] = batch[node_at[valid]].astype(np.float16)
    dv[valid] = dinv[node_at[valid]]
    for c in range(NCORES):
        pl.cores[c]["batchval"] = bv[c].reshape(TILES, P).T.copy()  # [128, 49]
        pl.cores[c]["dinv"] = dv[c].reshape(TILES, P).T.copy()      # [128, 49]
        xt = np.zeros((D_IN, SLOTS), dtype=np.float32)
        v = valid[c]
        xt[:, v] = np.asarray(x, dtype=np.float32)[node_at[c][v]].T
        pl.cores[c]["xT"] = xt.astype(BF16_NP)

    rm = np.arange(RAW, dtype=np.int64)
    pl.rowmap = (rm + (rm >= LOB)).reshape(NCORES, SLOTS)
    pl.counts = np.bincount(batch, minlength=NGRAPH).astype(np.float32)
    pl.iota_rep = np.repeat(np.arange(P), pl.NCHMAX).astype(np.float16).reshape(1, -1)
    pl.giota = np.repeat(np.arange(NGRAPH), TILES).astype(np.float16).reshape(1, -1)
    pl.key = (tuple(CLO), tuple(CHI))
    return pl


# ---------------------------------------------------------- program builders --

def _gemm_to_table(nc, ctx, tc, k_tiles, o_T, identB, dinv_sb, tps_pool=None):
    """xw^T[fout, slot] = sum_k lhsT_k^T @ rhs_k (bf16);
    transpose + dinv-scale + cast -> o_T rows (fp16)."""
    gps_pool = ctx.enter_context(tc.tile_pool(name="gemm_ps", bufs=2, space="PSUM"))
    gcp_pool = ctx.enter_context(tc.tile_pool(name="gemm_cp", bufs=2))
    if tps_pool is None:
        tps_pool = ctx.enter_context(
            tc.tile_pool(name="gemm_tp", bufs=2, space="PSUM"))
    to_pool = ctx.enter_context(tc.tile_pool(name="gemm_to", bufs=3))
    CH = 512
    for o in range(0, SLOTS, CH):
        w = min(CH, SLOTS - o)
        gps = gps_pool.tile([P, CH], F32, space="PSUM")
        for ki, (lhsT, rhs) in enumerate(k_tiles):
            nc.tensor.matmul(
                out=gps[:, :w],
                lhsT=lhsT[:],
                rhs=rhs[:, o:o + w],
                start=(ki == 0), stop=(ki == len(k_tiles) - 1),
            )
        gcp = gcp_pool.tile([P, CH], BF16)
        nc.scalar.activation(out=gcp[:, :w], in_=gps[:, :w],
                             func=mybir.ActivationFunctionType.Copy)
        for s in range(0, w, P):
            blk = (o + s) // P
            tp = tps_pool.tile([P, P], BF16, space="PSUM", tag="tp")
            nc.tensor.transpose(out=tp[:], in_=gcp[:, s:s + P], identity=identB[:])
            to = to_pool.tile([P, H], F16)
            nc.scalar.activation(out=to[:], in_=tp[:],
                                 func=mybir.ActivationFunctionType.Copy,
                                 scale=dinv_sb[:, blk:blk + 1])
            nc.sync.dma_start(out=o_T[o + s:o + s + P, :], in_=to[:])


def _build_A(pl):
    nc = bacc.Bacc("TRN2", target_bir_lowering=False, debug=False, num_devices=NCORES)
    i_xT = nc.dram_tensor("xT", [D_IN, SLOTS], BF16, kind="ExternalInput").ap()
    i_W = nc.dram_tensor("W", [D_IN, H], BF16, kind="ExternalInput").ap()
    i_dinv = nc.dram_tensor("dinv", [P, TILES], F32, kind="ExternalInput").ap()
    o_T = nc.dram_tensor("Tout", [SLOTS, H], F16, kind="ExternalOutput").ap()
    with tile.TileContext(nc) as tc:
        with ExitStack() as ctx:
            const = ctx.enter_context(tc.tile_pool(name="const", bufs=1))
            identB = const.tile([P, P], BF16)
            make_identity(nc, identB[:])
            dinv_sb = const.tile([P, TILES], F32)
            nc.sync.dma_start(out=dinv_sb[:], in_=i_dinv[:])
            w0 = const.tile([P, H], BF16)
            nc.sync.dma_start(out=w0[:], in_=i_W[0:P, :])
            w1 = const.tile([P, H], BF16)
            nc.sync.dma_start(out=w1[:], in_=i_W[P:2 * P, :])
            x0 = const.tile([P, SLOTS], BF16)
            nc.sync.dma_start(out=x0[:], in_=i_xT[0:P, :])
            x1 = const.tile([P, SLOTS], BF16)
            nc.sync.dma_start(out=x1[:], in_=i_xT[P:2 * P, :])
            _gemm_to_table(nc, ctx, tc, [(w0, x0), (w1, x1)], o_T, identB, dinv_sb)
    nc.compile()
    return nc


def _scatter_body(nc, ctx, tc, pl, i_T, consume_tile):
    """Shared gather + one-hot matmul scatter loop.

    consume_tile(t, ypsum) handles the per-tile PSUM result
    (ypsum = sum over in-edges of dinv[src]-scaled source rows, incl self-loop).
    """
    const = ctx.enter_context(tc.tile_pool(name="sc_const", bufs=1))
    stage = ctx.enter_context(tc.tile_pool(name="staging", bufs=2))
    st_pool = ctx.enter_context(tc.tile_pool(name="st", bufs=4))
    own_pool = ctx.enter_context(tc.tile_pool(name="own", bufs=3))
    yp_pool = ctx.enter_context(tc.tile_pool(name="yps", bufs=3, space="PSUM"))

    i_idxlo = nc.dram_tensor("idxlo", [P, pl.lo_cols], I16, kind="ExternalInput").ap()
    i_idxhi = nc.dram_tensor("idxhi", [P, pl.hi_cols], I16, kind="ExternalInput").ap()
    i_dstloc = nc.dram_tensor("dstloc", [P, pl.CTOT], F16, kind="ExternalInput").ap()
    i_iota = nc.dram_tensor("iota_rep", [1, P * pl.NCHMAX], F16,
                            kind="ExternalInput").ap()
    i_own = nc.dram_tensor("own", [SLOTS, H], F16, kind="ExternalInput").ap()

    idxlo_sb = const.tile([P, pl.lo_cols], I16)
    nc.sync.dma_start(out=idxlo_sb[:], in_=i_idxlo[:])
    idxhi_sb = const.tile([P, pl.hi_cols], I16)
    nc.sync.dma_start(out=idxhi_sb[:], in_=i_idxhi[:])
    dstloc_sb = const.tile([P, pl.CTOT], F16)
    nc.sync.dma_start(out=dstloc_sb[:], in_=i_dstloc[:])
    iota_sb = const.tile([P, P * pl.NCHMAX], F16)
    nc.sync.dma_start(out=iota_sb[:], in_=i_iota.to_broadcast([P, P * pl.NCHMAX]))
    iota3 = iota_sb[:].rearrange("p (j c) -> p j c", j=P, c=pl.NCHMAX)
    identH = const.tile([P, P], F16)
    make_identity(nc, identH[:])

    qn = [0]

    def gather(staging, base, src_ap, idx_sb, col0, nch):
        for o in range(0, nch, MAXCH):
            n = min(MAXCH, nch - o)
            c0 = col0 + o * 8
            nc.gpsimd.dma_gather(
                out_ap=staging[:, base + o:base + o + n, :], in_ap=src_ap,
                idxs_ap=idx_sb[:, c0:c0 + n * 8],
                num_idxs=n * P, num_idxs_reg=n * P, elem_size=H,
                queue_num=qn[0])
            qn[0] = (qn[0] + 1) % NQ

    def onehot(gc0, nch):
        st = st_pool.tile([P, P, nch], F16, tag="st")
        nc.vector.tensor_tensor(
            out=st[:],
            in0=iota3[:, :, 0:nch],
            in1=dstloc_sb[:, gc0:gc0 + nch].unsqueeze(1).to_broadcast([P, P, nch]),
            op=mybir.AluOpType.is_equal)
        return st

    for g, grp in enumerate(pl.groups):
        nclo, nchi = grp["nclo"], grp["nchi"]
        staging = stage.tile([P, nclo + nchi, H], F16, tag="staging")
        gather(staging, 0, i_T[:], idxlo_sb, grp["lo_col0"], nclo)
        gather(staging, nclo, i_T[32768:, :], idxhi_sb, grp["hi_col0"], nchi)
        for ti, td in enumerate(grp["tiles"]):
            t = g * GS + ti
            ow = own_pool.tile([P, H], F16, tag="ow")
            nc.sync.dma_start(out=ow[:], in_=i_own[t * P:(t + 1) * P, :])
            stlo = onehot(td["gc_lo"], td["clo"]) if td["clo"] else None
            sthi = onehot(td["gc_hi"], td["chi"]) if td["chi"] else None
            ypsum = yp_pool.tile([P, H], F32, space="PSUM")
            nc.tensor.matmul(out=ypsum[:], lhsT=identH[:], rhs=ow[:],
                             start=True, stop=(td["clo"] + td["chi"] == 0))
            for i in range(td["clo"]):
                nc.tensor.matmul(
                    out=ypsum[:], lhsT=stlo[:, :, i],
                    rhs=staging[:, td["sp_lo"] + i, :],
                    start=False,
                    stop=(i == td["clo"] - 1 and td["chi"] == 0))
            for i in range(td["chi"]):
                nc.tensor.matmul(
                    out=ypsum[:], lhsT=sthi[:, :, i],
                    rhs=staging[:, td["sp_hi"] + i, :],
                    start=False, stop=(i == td["chi"] - 1))
            consume_tile(t, ypsum)


def _vec_input(nc, const, name):
    ap = nc.dram_tensor(name, [H, 1], F32, kind="ExternalInput").ap()
    sb = const.tile([H, 1], F32, tag=f"vec_{name}")
    nc.sync.dma_start(out=sb[:], in_=ap[:])
    return sb


def _build_BC(pl):
    nc = bacc.Bacc("TRN2", target_bir_lowering=False, debug=False,
                   num_devices=NCORES, dynamic_dma_scratch_size=SCRATCH,
                   num_swdge_queues=NQ)
    i_T = nc.dram_tensor("T", [TAB, H], F16, kind="ExternalInput").ap()
    i_W = nc.dram_tensor("W", [H, H], BF16, kind="ExternalInput").ap()
    i_dinv = nc.dram_tensor("dinv", [P, TILES], F32, kind="ExternalInput").ap()
    o_T = nc.dram_tensor("Tout", [SLOTS, H], F16, kind="ExternalOutput").ap()
    with tile.TileContext(nc) as tc:
        with ExitStack() as ctx:
            const = ctx.enter_context(tc.tile_pool(name="bc_const", bufs=1))
            ycp_pool = ctx.enter_context(tc.tile_pool(name="ycp", bufs=3))
            tps_pool = ctx.enter_context(tc.tile_pool(name="tps", bufs=2, space="PSUM"))

            b_sb = _vec_input(nc, const, "bvec")
            g_sb = _vec_input(nc, const, "bn_g")
            bb_sb = _vec_input(nc, const, "bn_b")
            m_sb = _vec_input(nc, const, "bn_m")
            v_sb = _vec_input(nc, const, "bn_v")
            # scale = g / sqrt(v+eps); bias = (b - m)*scale + beta
            eps = const.tile([H, 1], F32)
            nc.vector.memset(eps[:], BN_EPS)
            sq = const.tile([H, 1], F32)
            nc.scalar.activation(out=sq[:], in_=v_sb[:],
                                 func=mybir.ActivationFunctionType.Sqrt,
                                 bias=eps[:], scale=1.0)
            rs = const.tile([H, 1], F32)
            nc.vector.reciprocal(out=rs[:], in_=sq[:])
            scale = const.tile([H, 1], F32)
            nc.vector.tensor_mul(out=scale[:], in0=rs[:], in1=g_sb[:])
            bias = const.tile([H, 1], F32)
            nc.vector.tensor_sub(out=bias[:], in0=b_sb[:], in1=m_sb[:])
            nc.vector.tensor_mul(out=bias[:], in0=bias[:], in1=scale[:])
            nc.vector.tensor_add(out=bias[:], in0=bias[:], in1=bb_sb[:])

            identB = const.tile([P, P], BF16)
            make_identity(nc, identB[:])
            dinv_sb = const.tile([P, TILES], F32)
            nc.sync.dma_start(out=dinv_sb[:], in_=i_dinv[:])
            w_sb = const.tile([H, H], BF16)
            nc.sync.dma_start(out=w_sb[:], in_=i_W[:])
            hT = const.tile([P, SLOTS], BF16)

            def consume(t, ypsum):
                ycp = ycp_pool.tile([P, H], BF16)
                nc.scalar.activation(out=ycp[:], in_=ypsum[:],
                                     func=mybir.ActivationFunctionType.Copy,
                                     scale=dinv_sb[:, t:t + 1])
                tp = tps_pool.tile([P, P], BF16, space="PSUM")
                nc.tensor.transpose(out=tp[:], in_=ycp[:], identity=identB[:])
                nc.scalar.activation(
                    out=hT[:, t * P:(t + 1) * P], in_=tp[:],
                    func=mybir.ActivationFunctionType.Relu,
                    bias=bias[:], scale=scale[:])

            _scatter_body(nc, ctx, tc, pl, i_T, consume)
            _gemm_to_table(nc, ctx, tc, [(w_sb, hT)], o_T, identB, dinv_sb,
                           tps_pool=tps_pool)
    nc.compile()
    return nc


def _build_D(pl):
    nc = bacc.Bacc("TRN2", target_bir_lowering=False, debug=False,
                   num_devices=NCORES, dynamic_dma_scratch_size=SCRATCH,
                   num_swdge_queues=NQ)
    i_T = nc.dram_tensor("T", [TAB, H], F16, kind="ExternalInput").ap()
    i_bv = nc.dram_tensor("batchval", [P, TILES], F16, kind="ExternalInput").ap()
    i_gi = nc.dram_tensor("giota", [1, NGRAPH * TILES], F16,
                          kind="ExternalInput").ap()
    i_dinv = nc.dram_tensor("dinv", [P, TILES], F32, kind="ExternalInput").ap()
    o_pool = nc.dram_tensor("pool", [NGRAPH, H], F32, kind="ExternalOutput").ap()
    with tile.TileContext(nc) as tc:
        with ExitStack() as ctx:
            const = ctx.enter_context(tc.tile_pool(name="d_const", bufs=1))
            h3_pool = ctx.enter_context(tc.tile_pool(name="h3", bufs=3))
            pp_pool = ctx.enter_context(tc.tile_pool(name="pp", bufs=1, space="PSUM"))

            bv_sb = const.tile([P, TILES], F16)
            nc.sync.dma_start(out=bv_sb[:], in_=i_bv[:])
            gi_sb = const.tile([P, NGRAPH * TILES], F16)
            nc.sync.dma_start(out=gi_sb[:], in_=i_gi.to_broadcast([P, NGRAPH * TILES]))
            dinv_sb = const.tile([P, TILES], F32)
            nc.sync.dma_start(out=dinv_sb[:], in_=i_dinv[:])
            # oh_all[p, g, t] = (batchval[p, t] == g)
            oh_all = const.tile([P, NGRAPH, TILES], F16)
            nc.vector.tensor_tensor(
                out=oh_all[:],
                in0=gi_sb[:].rearrange("p (g t) -> p g t", g=NGRAPH, t=TILES),
                in1=bv_sb[:].unsqueeze(1).to_broadcast([P, NGRAPH, TILES]),
                op=mybir.AluOpType.is_equal)
            pp = pp_pool.tile([NGRAPH, H], F32, space="PSUM")

            def consume(t, ypsum):
                h3 = h3_pool.tile([P, H], F16)
                nc.scalar.activation(out=h3[:], in_=ypsum[:],
                                     func=mybir.ActivationFunctionType.Copy,
                                     scale=dinv_sb[:, t:t + 1])
                nc.tensor.matmul(out=pp[:], lhsT=oh_all[:, :, t], rhs=h3[:],
                                 start=(t == 0), stop=(t == TILES - 1))

            _scatter_body(nc, ctx, tc, pl, i_T, consume)
            pcp = const.tile([NGRAPH, H], F32)
            nc.vector.tensor_copy(out=pcp[:], in_=pp[:])
            nc.sync.dma_start(out=o_pool[:], in_=pcp[:])
    nc.compile()
    return nc


# ------------------------------------------------------------------- driver --

def _run(nc, in_maps):
    res = run_bass_kernel_spmd(nc, in_maps, core_ids=list(range(NCORES)),
                               trace=TRACE)
    if TRACE:
        LAST_EXEC_NS.append(res.exec_time_ns)
    return res.results


def _assemble_table(pl, shards):
    T = np.zeros((TAB, H), dtype=np.float16)
    for c in range(NCORES):
        T[pl.rowmap[c]] = shards[c]
    return T


def kernel(**inputs):
    ins = {k: np.asarray(v) for k, v in inputs.items()}
    key = hashlib.sha1(
        ins["edge_index"].tobytes() + ins["batch"].tobytes()
    ).hexdigest()
    if key not in _PLAN_CACHE:
        _PLAN_CACHE[key] = _make_plan(ins["edge_index"], ins["batch"], ins["x"])
    pl = _PLAN_CACHE[key]

    pk = pl.key
    if pk not in _PROG_CACHE:
        _PROG_CACHE[pk] = {
            "A": _build_A(pl),
            "BC": _build_BC(pl),
            "D": _build_D(pl),
        }
    progs = _PROG_CACHE[pk]

    LAST_EXEC_NS.clear()
    W1 = ins["W1"].astype(BF16_NP)
    # Launch A: T1 = dinv * (x @ W1)
    resA = _run(progs["A"], [
        {"xT": pl.cores[c]["xT"], "W": W1, "dinv": pl.cores[c]["dinv"]}
        for c in range(NCORES)
    ])
    shardsA = [r["Tout"] for r in resA]
    T1 = _assemble_table(pl, shardsA)

    def meta(c):
        cc = pl.cores[c]
        return {"idxlo": cc["idxlo"], "idxhi": cc["idxhi"],
                "dstloc": cc["dstloc"], "iota_rep": pl.iota_rep,
                "dinv": cc["dinv"]}

    def vec(name):
        return ins[name].astype(np.float32).reshape(H, 1)

    # Launch B: layer-1 scatter + BN1/ReLU + @W2
    resB = _run(progs["BC"], [
        {**meta(c), "T": T1, "own": shardsA[c], "W": ins["W2"].astype(BF16_NP),
         "bvec": vec("b1"), "bn_g": vec("bn1_g"), "bn_b": vec("bn1_b"),
         "bn_m": vec("bn1_m"), "bn_v": vec("bn1_v")} for c in range(NCORES)
    ])
    shardsB = [r["Tout"] for r in resB]
    T2 = _assemble_table(pl, shardsB)

    # Launch C: layer-2 scatter + BN2/ReLU + @W3
    resC = _run(progs["BC"], [
        {**meta(c), "T": T2, "own": shardsB[c], "W": ins["W3"].astype(BF16_NP),
         "bvec": vec("b2"), "bn_g": vec("bn2_g"), "bn_b": vec("bn2_b"),
         "bn_m": vec("bn2_m"), "bn_v": vec("bn2_v")} for c in range(NCORES)
    ])
    shardsC = [r["Tout"] for r in resC]
    T3 = _assemble_table(pl, shardsC)

    # Launch D: layer-3 scatter + pooling partials
    resD = _run(progs["D"], [
        {**meta(c), "T": T3, "own": shardsC[c],
         "batchval": pl.cores[c]["batchval"], "giota": pl.giota}
        for c in range(NCORES)
    ])
    pooled_sum = np.sum([r["pool"] for r in resD], axis=0).astype(np.float64)

    counts = pl.counts.astype(np.float64)
    pooled_sum += counts[:, None] * ins["b3"].astype(np.float64)[None, :]
    pooled = pooled_sum / np.maximum(counts, 1.0)[:, None]

    z = np.maximum(pooled @ ins["Wc1"].astype(np.float64)
                   + ins["bc1"].astype(np.float64), 0.0)
    out = z @ ins["Wc2"].astype(np.float64) + ins["bc2"].astype(np.float64)
    return out.astype(np.float32)


# revision 15
# speedup vs baseline: 1.3781x; 1.1174x over previous
"""Trainium2 Bass kernel for DocumentClassificationGNN (3-layer GCN + BN/ReLU +
global mean pool + MLP head), distributed over 8 NeuronCores.

Strategy (node/graph parallel, per the sharding hint):
  - Nodes are assigned to (core, slot) sorted by in-degree so every core/tile
    carries a balanced edge load.  Edges are partitioned by DESTINATION core so
    the segment-sum scatter is device-local.
  - Per layer: a dense GEMM produces a node-major fp16 feature table that the
    host replicates to all cores ("all-gather" through the host between
    launches); each core gathers its in-edge source rows with dma_gather and
    scatter-adds them into PSUM with one-hot matmuls.
  - The symmetric norm deg^-1/2[src]*deg^-1/2[dst] is SEPARABLE: table rows
    are pre-scaled by dinv[src] at write time and the scatter output is
    post-scaled by dinv[dst], so the one-hot matrices are pure 0/1 and are
    generated in batched DVE is_equal ops (2-byte fast path) with the chunk
    dim innermost: s_t[p, j, c].
  - Self-loops never enter the edge stream: each tile's own table rows are
    bulk-loaded and added via one identity matmul (contribution dinv_d*T'[d]).
  - conv bias + BN + ReLU fuse into one scalar-engine activation; GEMMs run in
    bf16; launch D does per-tile onehot(batch) pooling accumulated in one PSUM
    bank.
  - Device output: per-core pooled partial sums [64, 128].  Host: sum, +n_g*b3,
    divide by counts, tiny classifier MLP.

Programs (3 compiles, 4 launches):
  A : T1 = dinv * (x @ W1)                          -> T1 table shard
  BC: Y = scatter(T); h' = relu(BN(dinv*Y + b)); T' = dinv * (h' @ W_next)
  D : Y3 = scatter(T3); pooled_partial = onehot(batch)^T @ (dinv*Y3)
"""

import hashlib
import numpy as np
from contextlib import ExitStack

import ml_dtypes

import concourse.bass as bass
import concourse.bacc as bacc
import concourse.tile as tile
from concourse import mybir
from concourse.bass_utils import run_bass_kernel_spmd
from concourse.masks import make_identity

P = 128
NCORES = 8
N = 50000
D_IN = 256
H = 128
NGRAPH = 64
SLOTS = 6272            # 49 tiles of 128 slots per core (6250 real nodes + pad)
TILES = SLOTS // P      # 49
RAW = NCORES * SLOTS    # 50176
LOB = 32767             # table row 32767 is the lo-region zero row
TAB = RAW + 2           # +2 zero rows (lo @32767, hi @TAB-1)
ZLO = LOB               # lo-local zero row index
ZHI_LOCAL = TAB - 1 - 32768   # hi-local zero row index
GS = 7                  # dst tiles per gather group
NGROUPS = TILES // GS   # 7
BN_EPS = 1e-5

SCRATCH = 16384         # SWDGE ring: 16384/16 = 1024 descriptors per queue
MAXCH = 8               # chunks per dma_gather call (8*128 = 1024, HW limit)
NQ = 2                  # SWDGE queues (desc-gen pipelines against transfer)

F16 = mybir.dt.float16
BF16 = mybir.dt.bfloat16
F32 = mybir.dt.float32
I16 = mybir.dt.int16
BF16_NP = ml_dtypes.bfloat16

# module-level knobs / perf results (test.py pokes these)
TRACE = False
LAST_EXEC_NS = []       # per-launch exec_time_ns (when TRACE)

_PLAN_CACHE = {}
_PROG_CACHE = {}


# ---------------------------------------------------------------- host prep --

def _wrap_idx(flat):
    """dma_gather index layout: idx i -> [i%16, i//16], replicated to 128 parts."""
    n = len(flat)
    assert n % 16 == 0
    arr = np.asarray(flat, dtype=np.int16).reshape(n // 16, 16).T.copy()
    return np.tile(arr, (8, 1))


class _Plan:
    pass


def _make_plan(edge_index, batch, x):
    pl = _Plan()
    src = np.asarray(edge_index[0], dtype=np.int64)
    dst = np.asarray(edge_index[1], dtype=np.int64)
    batch = np.asarray(batch, dtype=np.int64)

    deg = np.bincount(dst, minlength=N).astype(np.int64) + 1
    dinv = (1.0 / np.sqrt(deg)).astype(np.float32)

    order = np.argsort(-deg, kind="stable")
    rank = np.empty(N, dtype=np.int64)
    rank[order] = np.arange(N)
    core_of = rank % NCORES
    slot_of = rank // NCORES
    raw_of = core_of * SLOTS + slot_of
    grow_of = raw_of + (raw_of >= LOB)      # table row per node

    # real edges only: self-loops are handled by the per-tile identity matmul
    es, ed = src, dst
    ecore = core_of[ed]
    eslot = slot_of[ed]
    etile = eslot // P
    edstloc = eslot % P
    esg = grow_of[es]
    islo = esg < LOB

    # per-core sorted segment arrays
    NSEG = TILES * 2   # segment id: 2*tile + (0 lo / 1 hi)
    per_core = []
    seg_counts = np.zeros((NCORES, NSEG), dtype=np.int64)
    for c in range(NCORES):
        m = ecore == c
        seg = etile[m] * 2 + (~islo[m]).astype(np.int64)
        o2 = np.lexsort((esg[m], seg))
        d = {
            "seg": seg[o2],
            "dstloc": edstloc[m][o2],
            "esg": esg[m][o2],
        }
        seg_counts[c] = np.bincount(d["seg"], minlength=NSEG)
        per_core.append(d)

    # chunk plan: per tile, lo/hi chunk counts = max over cores
    CLO = np.ceil(seg_counts[:, 0::2].max(axis=0) / P).astype(int)
    CHI = np.ceil(seg_counts[:, 1::2].max(axis=0) / P).astype(int)
    # chunk order: group-major; within group: all lo chunks (tile order), then hi
    seg_chunk_start = np.zeros(NSEG, dtype=np.int64)   # global chunk idx per seg
    grp_clo = np.zeros(NGROUPS, dtype=np.int64)
    grp_chi = np.zeros(NGROUPS, dtype=np.int64)
    gcb = np.zeros(NGROUPS + 1, dtype=np.int64)
    for g in range(NGROUPS):
        ts = range(g * GS, (g + 1) * GS)
        grp_clo[g] = sum(CLO[t] for t in ts)
        grp_chi[g] = sum(CHI[t] for t in ts)
        ofs = gcb[g]
        for t in ts:
            seg_chunk_start[2 * t] = ofs
            ofs += CLO[t]
        for t in ts:
            seg_chunk_start[2 * t + 1] = ofs
            ofs += CHI[t]
        gcb[g + 1] = ofs
    CTOT = int(gcb[-1])

    # per-chunk default fill (pads): lo chunks -> ZLO, hi chunks -> absolute hi zero
    chunk_is_hi = np.zeros(CTOT, dtype=bool)
    for t in range(TILES):
        s = seg_chunk_start[2 * t + 1]
        chunk_is_hi[s:s + CHI[t]] = True

    pl.cores = []
    for c in range(NCORES):
        d = per_core[c]
        npad = CTOT * P
        dstloc_pad = np.zeros(npad, dtype=np.float16)
        row_pad = np.where(np.repeat(chunk_is_hi, P), TAB - 1, ZLO).astype(np.int64)
        # position of each real edge
        cnt = seg_counts[c]
        seg_first = np.concatenate([[0], np.cumsum(cnt)[:-1]])
        within = np.arange(len(d["seg"])) - seg_first[d["seg"]]
        pos = seg_chunk_start[d["seg"]] * P + within
        dstloc_pad[pos] = d["dstloc"].astype(np.float16)
        row_pad[pos] = d["esg"]

        # gather index arrays (lo then hi, group-major)
        lo_parts, hi_parts = [], []
        for g in range(NGROUPS):
            a = gcb[g] * P
            b = a + grp_clo[g] * P
            e = gcb[g + 1] * P
            lo_parts.append(row_pad[a:b])
            hi_parts.append(row_pad[b:e] - 32768)
        lo_flat = np.concatenate(lo_parts)
        hi_flat = np.concatenate(hi_parts)
        assert lo_flat.min() >= 0 and lo_flat.max() <= LOB
        assert hi_flat.min() >= 0 and hi_flat.max() <= ZHI_LOCAL

        core = {
            "idxlo": _wrap_idx(lo_flat),
            "idxhi": _wrap_idx(hi_flat),
            "dstloc": dstloc_pad.reshape(CTOT, P).T.copy(),
        }
        pl.cores.append(core)

    # group gather call metadata (columns into wrapped idx tensors)
    pl.lo_cols = int(grp_clo.sum() * P // 16)
    pl.hi_cols = int(grp_chi.sum() * P // 16)
    lo_c0 = np.concatenate([[0], np.cumsum(grp_clo * 8)])
    hi_c0 = np.concatenate([[0], np.cumsum(grp_chi * 8)])
    pl.groups = []
    for g in range(NGROUPS):
        tiles = []
        for t in range(g * GS, (g + 1) * GS):
            lo_local = int(seg_chunk_start[2 * t] - gcb[g])
            hi_local = int(seg_chunk_start[2 * t + 1] - gcb[g])
            tiles.append({
                "clo": int(CLO[t]), "chi": int(CHI[t]),
                "sp_lo": lo_local, "sp_hi": hi_local,
                "gc_lo": int(seg_chunk_start[2 * t]),
                "gc_hi": int(seg_chunk_start[2 * t + 1]),
            })
        pl.groups.append({
            "nclo": int(grp_clo[g]), "nchi": int(grp_chi[g]),
            "lo_col0": int(lo_c0[g]), "hi_col0": int(hi_c0[g]),
            "tiles": tiles,
        })
    pl.CTOT = CTOT
    pl.NCHMAX = int(max(CLO.max(), CHI.max()))

    # slot -> node map, batch values, dinv per slot, xT shards, table row map
    node_at = np.full((NCORES, SLOTS), -1, dtype=np.int64)
    node_at[core_of, slot_of] = np.arange(N)
    bv = np.full((NCORES, SLOTS), 99.0, dtype=np.float16)
    dv = np.ones((NCORES, SLOTS), dtype=np.float32)
    valid = node_at >= 0
    bv[valid] = batch[node_at[valid]].astype(np.float16)
    dv[valid] = dinv[node_at[valid]]
    for c in range(NCORES):
        pl.cores[c]["batchval"] = bv[c].reshape(TILES, P).T.copy()  # [128, 49]
        pl.cores[c]["dinv"] = dv[c].reshape(TILES, P).T.copy()      # [128, 49]
        xt = np.zeros((D_IN, SLOTS), dtype=np.float32)
        v = valid[c]
        xt[:, v] = np.asarray(x, dtype=np.float32)[node_at[c][v]].T
        pl.cores[c]["xT"] = xt.astype(BF16_NP)

    rm = np.arange(RAW, dtype=np.int64)
    pl.rowmap = (rm + (rm >= LOB)).reshape(NCORES, SLOTS)
    pl.counts = np.bincount(batch, minlength=NGRAPH).astype(np.float32)
    pl.iota_rep = np.repeat(np.arange(P), pl.NCHMAX).astype(np.float16).reshape(1, -1)
    pl.giota = np.repeat(np.arange(NGRAPH), TILES).astype(np.float16).reshape(1, -1)
    pl.key = (tuple(CLO), tuple(CHI))
    return pl


# ---------------------------------------------------------- program builders --

GEMM_CH = 512
GEMM_NCHUNK = (SLOTS + GEMM_CH - 1) // GEMM_CH


def _make_gemm_emitter(nc, ctx, tc, k_tiles, o_T, identB, dinv_sb,
                       tps_pool=None, bufs=2):
    """Returns emit(k): xw^T[fout, slot-chunk k] = sum_k lhsT^T @ rhs (bf16);
    transpose + dinv-scale + cast -> o_T rows (fp16)."""
    gps_pool = ctx.enter_context(
        tc.tile_pool(name="gemm_ps", bufs=bufs, space="PSUM"))
    gcp_pool = ctx.enter_context(tc.tile_pool(name="gemm_cp", bufs=bufs + 1))
    if tps_pool is None:
        tps_pool = ctx.enter_context(
            tc.tile_pool(name="gemm_tp", bufs=bufs + 2, space="PSUM"))
    to_pool = ctx.enter_context(tc.tile_pool(name="gemm_to", bufs=bufs + 2))

    def emit(k):
        o = k * GEMM_CH
        w = min(GEMM_CH, SLOTS - o)
        gps = gps_pool.tile([P, GEMM_CH], F32, space="PSUM")
        for ki, (lhsT, rhs) in enumerate(k_tiles):
            nc.tensor.matmul(
                out=gps[:, :w],
                lhsT=lhsT[:],
                rhs=rhs[:, o:o + w],
                start=(ki == 0), stop=(ki == len(k_tiles) - 1),
            )
        gcp = gcp_pool.tile([P, GEMM_CH], BF16)
        nc.scalar.activation(out=gcp[:, :w], in_=gps[:, :w],
                             func=mybir.ActivationFunctionType.Copy)
        for s in range(0, w, P):
            blk = (o + s) // P
            tp = tps_pool.tile([P, P], BF16, space="PSUM", tag="tp")
            nc.tensor.transpose(out=tp[:], in_=gcp[:, s:s + P], identity=identB[:])
            to = to_pool.tile([P, H], F16)
            nc.scalar.activation(out=to[:], in_=tp[:],
                                 func=mybir.ActivationFunctionType.Copy,
                                 scale=dinv_sb[:, blk:blk + 1])
            nc.sync.dma_start(out=o_T[o + s:o + s + P, :], in_=to[:])

    return emit


def _build_A(pl):
    nc = bacc.Bacc("TRN2", target_bir_lowering=False, debug=False, num_devices=NCORES)
    i_xT = nc.dram_tensor("xT", [D_IN, SLOTS], BF16, kind="ExternalInput").ap()
    i_W = nc.dram_tensor("W", [D_IN, H], BF16, kind="ExternalInput").ap()
    i_dinv = nc.dram_tensor("dinv", [P, TILES], F32, kind="ExternalInput").ap()
    o_T = nc.dram_tensor("Tout", [SLOTS, H], F16, kind="ExternalOutput").ap()
    with tile.TileContext(nc) as tc:
        with ExitStack() as ctx:
            const = ctx.enter_context(tc.tile_pool(name="const", bufs=1))
            identB = const.tile([P, P], BF16)
            make_identity(nc, identB[:])
            dinv_sb = const.tile([P, TILES], F32)
            nc.sync.dma_start(out=dinv_sb[:], in_=i_dinv[:])
            w0 = const.tile([P, H], BF16)
            nc.sync.dma_start(out=w0[:], in_=i_W[0:P, :])
            w1 = const.tile([P, H], BF16)
            nc.sync.dma_start(out=w1[:], in_=i_W[P:2 * P, :])
            x0 = const.tile([P, SLOTS], BF16)
            x1 = const.tile([P, SLOTS], BF16)
            for o in range(0, SLOTS, GEMM_CH):
                w = min(GEMM_CH, SLOTS - o)
                nc.sync.dma_start(out=x0[:, o:o + w], in_=i_xT[0:P, o:o + w])
                nc.sync.dma_start(out=x1[:, o:o + w], in_=i_xT[P:2 * P, o:o + w])
            emit = _make_gemm_emitter(nc, ctx, tc, [(w0, x0), (w1, x1)], o_T,
                                      identB, dinv_sb, bufs=3)
            for k in range(GEMM_NCHUNK):
                emit(k)
    nc.compile()
    return nc


def _scatter_body(nc, ctx, tc, pl, i_T, consume_tile, after_tile=None):
    """Shared gather + one-hot matmul scatter loop.

    consume_tile(t, ypsum) handles the per-tile PSUM result
    (ypsum = sum over in-edges of dinv[src]-scaled source rows, incl self-loop).
    """
    const = ctx.enter_context(tc.tile_pool(name="sc_const", bufs=1))
    stage = ctx.enter_context(tc.tile_pool(name="staging", bufs=2))
    st_pool = ctx.enter_context(tc.tile_pool(name="st", bufs=4))
    own_pool = ctx.enter_context(tc.tile_pool(name="own", bufs=3))
    yp_pool = ctx.enter_context(tc.tile_pool(name="yps", bufs=3, space="PSUM"))

    i_idxlo = nc.dram_tensor("idxlo", [P, pl.lo_cols], I16, kind="ExternalInput").ap()
    i_idxhi = nc.dram_tensor("idxhi", [P, pl.hi_cols], I16, kind="ExternalInput").ap()
    i_dstloc = nc.dram_tensor("dstloc", [P, pl.CTOT], F16, kind="ExternalInput").ap()
    i_iota = nc.dram_tensor("iota_rep", [1, P * pl.NCHMAX], F16,
                            kind="ExternalInput").ap()
    i_own = nc.dram_tensor("own", [SLOTS, H], F16, kind="ExternalInput").ap()

    idxlo_sb = const.tile([P, pl.lo_cols], I16)
    nc.sync.dma_start(out=idxlo_sb[:], in_=i_idxlo[:])
    idxhi_sb = const.tile([P, pl.hi_cols], I16)
    nc.sync.dma_start(out=idxhi_sb[:], in_=i_idxhi[:])
    dstloc_sb = const.tile([P, pl.CTOT], F16)
    nc.sync.dma_start(out=dstloc_sb[:], in_=i_dstloc[:])
    iota_sb = const.tile([P, P * pl.NCHMAX], F16)
    nc.sync.dma_start(out=iota_sb[:], in_=i_iota.to_broadcast([P, P * pl.NCHMAX]))
    iota3 = iota_sb[:].rearrange("p (j c) -> p j c", j=P, c=pl.NCHMAX)
    identH = const.tile([P, P], F16)
    make_identity(nc, identH[:])

    qn = [0]

    def gather(staging, base, src_ap, idx_sb, col0, nch):
        for o in range(0, nch, MAXCH):
            n = min(MAXCH, nch - o)
            c0 = col0 + o * 8
            nc.gpsimd.dma_gather(
                out_ap=staging[:, base + o:base + o + n, :], in_ap=src_ap,
                idxs_ap=idx_sb[:, c0:c0 + n * 8],
                num_idxs=n * P, num_idxs_reg=n * P, elem_size=H,
                queue_num=qn[0])
            qn[0] = (qn[0] + 1) % NQ

    def onehot(gc0, nch):
        st = st_pool.tile([P, P, nch], F16, tag="st")
        nc.vector.tensor_tensor(
            out=st[:],
            in0=iota3[:, :, 0:nch],
            in1=dstloc_sb[:, gc0:gc0 + nch].unsqueeze(1).to_broadcast([P, P, nch]),
            op=mybir.AluOpType.is_equal)
        return st

    for g, grp in enumerate(pl.groups):
        nclo, nchi = grp["nclo"], grp["nchi"]
        staging = stage.tile([P, nclo + nchi, H], F16, tag="staging")
        gather(staging, 0, i_T[:], idxlo_sb, grp["lo_col0"], nclo)
        gather(staging, nclo, i_T[32768:, :], idxhi_sb, grp["hi_col0"], nchi)
        for ti, td in enumerate(grp["tiles"]):
            t = g * GS + ti
            ow = own_pool.tile([P, H], F16, tag="ow")
            nc.sync.dma_start(out=ow[:], in_=i_own[t * P:(t + 1) * P, :])
            stlo = onehot(td["gc_lo"], td["clo"]) if td["clo"] else None
            sthi = onehot(td["gc_hi"], td["chi"]) if td["chi"] else None
            ypsum = yp_pool.tile([P, H], F32, space="PSUM")
            nc.tensor.matmul(out=ypsum[:], lhsT=identH[:], rhs=ow[:],
                             start=True, stop=(td["clo"] + td["chi"] == 0))
            for i in range(td["clo"]):
                nc.tensor.matmul(
                    out=ypsum[:], lhsT=stlo[:, :, i],
                    rhs=staging[:, td["sp_lo"] + i, :],
                    start=False,
                    stop=(i == td["clo"] - 1 and td["chi"] == 0))
            for i in range(td["chi"]):
                nc.tensor.matmul(
                    out=ypsum[:], lhsT=sthi[:, :, i],
                    rhs=staging[:, td["sp_hi"] + i, :],
                    start=False, stop=(i == td["chi"] - 1))
            consume_tile(t, ypsum)
            if after_tile is not None:
                after_tile(t)


def _vec_input(nc, const, name):
    ap = nc.dram_tensor(name, [H, 1], F32, kind="ExternalInput").ap()
    sb = const.tile([H, 1], F32, tag=f"vec_{name}")
    nc.sync.dma_start(out=sb[:], in_=ap[:])
    return sb


def _build_BC(pl):
    nc = bacc.Bacc("TRN2", target_bir_lowering=False, debug=False,
                   num_devices=NCORES, dynamic_dma_scratch_size=SCRATCH,
                   num_swdge_queues=NQ)
    i_T = nc.dram_tensor("T", [TAB, H], F16, kind="ExternalInput").ap()
    i_W = nc.dram_tensor("W", [H, H], BF16, kind="ExternalInput").ap()
    i_dinv = nc.dram_tensor("dinv", [P, TILES], F32, kind="ExternalInput").ap()
    o_T = nc.dram_tensor("Tout", [SLOTS, H], F16, kind="ExternalOutput").ap()
    with tile.TileContext(nc) as tc:
        with ExitStack() as ctx:
            const = ctx.enter_context(tc.tile_pool(name="bc_const", bufs=1))
            ycp_pool = ctx.enter_context(tc.tile_pool(name="ycp", bufs=3))
            tps_pool = ctx.enter_context(tc.tile_pool(name="tps", bufs=2, space="PSUM"))

            b_sb = _vec_input(nc, const, "bvec")
            g_sb = _vec_input(nc, const, "bn_g")
            bb_sb = _vec_input(nc, const, "bn_b")
            m_sb = _vec_input(nc, const, "bn_m")
            v_sb = _vec_input(nc, const, "bn_v")
            # scale = g / sqrt(v+eps); bias = (b - m)*scale + beta
            eps = const.tile([H, 1], F32)
            nc.vector.memset(eps[:], BN_EPS)
            sq = const.tile([H, 1], F32)
            nc.scalar.activation(out=sq[:], in_=v_sb[:],
                                 func=mybir.ActivationFunctionType.Sqrt,
                                 bias=eps[:], scale=1.0)
            rs = const.tile([H, 1], F32)
            nc.vector.reciprocal(out=rs[:], in_=sq[:])
            scale = const.tile([H, 1], F32)
            nc.vector.tensor_mul(out=scale[:], in0=rs[:], in1=g_sb[:])
            bias = const.tile([H, 1], F32)
            nc.vector.tensor_sub(out=bias[:], in0=b_sb[:], in1=m_sb[:])
            nc.vector.tensor_mul(out=bias[:], in0=bias[:], in1=scale[:])
            nc.vector.tensor_add(out=bias[:], in0=bias[:], in1=bb_sb[:])

            identB = const.tile([P, P], BF16)
            make_identity(nc, identB[:])
            dinv_sb = const.tile([P, TILES], F32)
            nc.sync.dma_start(out=dinv_sb[:], in_=i_dinv[:])
            w_sb = const.tile([H, H], BF16)
            nc.sync.dma_start(out=w_sb[:], in_=i_W[:])
            hT = const.tile([P, SLOTS], BF16)

            def consume(t, ypsum):
                ycp = ycp_pool.tile([P, H], BF16)
                nc.scalar.activation(out=ycp[:], in_=ypsum[:],
                                     func=mybir.ActivationFunctionType.Copy,
                                     scale=dinv_sb[:, t:t + 1])
                tp = tps_pool.tile([P, P], BF16, space="PSUM")
                nc.tensor.transpose(out=tp[:], in_=ycp[:], identity=identB[:])
                nc.scalar.activation(
                    out=hT[:, t * P:(t + 1) * P], in_=tp[:],
                    func=mybir.ActivationFunctionType.Relu,
                    bias=bias[:], scale=scale[:])

            emit = _make_gemm_emitter(nc, ctx, tc, [(w_sb, hT)], o_T,
                                      identB, dinv_sb, tps_pool=tps_pool)

            def after_tile(t):
                # gemm chunk k covers slots [512k, 512k+512) = tiles 4k..4k+3;
                # emit as soon as its hT columns are complete so the GEMM
                # overlaps the remaining scatter instead of tailing it.
                if t % 4 == 3:
                    emit(t // 4)
                elif t == TILES - 1:
                    emit(GEMM_NCHUNK - 1)

            _scatter_body(nc, ctx, tc, pl, i_T, consume, after_tile)
    nc.compile()
    return nc


def _build_D(pl):
    nc = bacc.Bacc("TRN2", target_bir_lowering=False, debug=False,
                   num_devices=NCORES, dynamic_dma_scratch_size=SCRATCH,
                   num_swdge_queues=NQ)
    i_T = nc.dram_tensor("T", [TAB, H], F16, kind="ExternalInput").ap()
    i_bv = nc.dram_tensor("batchval", [P, TILES], F16, kind="ExternalInput").ap()
    i_gi = nc.dram_tensor("giota", [1, NGRAPH * TILES], F16,
                          kind="ExternalInput").ap()
    i_dinv = nc.dram_tensor("dinv", [P, TILES], F32, kind="ExternalInput").ap()
    o_pool = nc.dram_tensor("pool", [NGRAPH, H], F32, kind="ExternalOutput").ap()
    with tile.TileContext(nc) as tc:
        with ExitStack() as ctx:
            const = ctx.enter_context(tc.tile_pool(name="d_const", bufs=1))
            h3_pool = ctx.enter_context(tc.tile_pool(name="h3", bufs=3))
            pp_pool = ctx.enter_context(tc.tile_pool(name="pp", bufs=1, space="PSUM"))

            bv_sb = const.tile([P, TILES], F16)
            nc.sync.dma_start(out=bv_sb[:], in_=i_bv[:])
            gi_sb = const.tile([P, NGRAPH * TILES], F16)
            nc.sync.dma_start(out=gi_sb[:], in_=i_gi.to_broadcast([P, NGRAPH * TILES]))
            dinv_sb = const.tile([P, TILES], F32)
            nc.sync.dma_start(out=dinv_sb[:], in_=i_dinv[:])
            # oh_all[p, g, t] = (batchval[p, t] == g)
            oh_all = const.tile([P, NGRAPH, TILES], F16)
            nc.vector.tensor_tensor(
                out=oh_all[:],
                in0=gi_sb[:].rearrange("p (g t) -> p g t", g=NGRAPH, t=TILES),
                in1=bv_sb[:].unsqueeze(1).to_broadcast([P, NGRAPH, TILES]),
                op=mybir.AluOpType.is_equal)
            pp = pp_pool.tile([NGRAPH, H], F32, space="PSUM")

            def consume(t, ypsum):
                h3 = h3_pool.tile([P, H], F16)
                nc.scalar.activation(out=h3[:], in_=ypsum[:],
                                     func=mybir.ActivationFunctionType.Copy,
                                     scale=dinv_sb[:, t:t + 1])
                nc.tensor.matmul(out=pp[:], lhsT=oh_all[:, :, t], rhs=h3[:],
                                 start=(t == 0), stop=(t == TILES - 1))

            _scatter_body(nc, ctx, tc, pl, i_T, consume)
            pcp = const.tile([NGRAPH, H], F32)
            nc.vector.tensor_copy(out=pcp[:], in_=pp[:])
            nc.sync.dma_start(out=o_pool[:], in_=pcp[:])
    nc.compile()
    return nc


# ------------------------------------------------------------------- driver --

def _run(nc, in_maps):
    res = run_bass_kernel_spmd(nc, in_maps, core_ids=list(range(NCORES)),
                               trace=TRACE)
    if TRACE:
        LAST_EXEC_NS.append(res.exec_time_ns)
    return res.results


def _assemble_table(pl, shards):
    T = np.zeros((TAB, H), dtype=np.float16)
    for c in range(NCORES):
        T[pl.rowmap[c]] = shards[c]
    return T


def kernel(**inputs):
    ins = {k: np.asarray(v) for k, v in inputs.items()}
    key = hashlib.sha1(
        ins["edge_index"].tobytes() + ins["batch"].tobytes()
    ).hexdigest()
    if key not in _PLAN_CACHE:
        _PLAN_CACHE[key] = _make_plan(ins["edge_index"], ins["batch"], ins["x"])
    pl = _PLAN_CACHE[key]

    pk = pl.key
    if pk not in _PROG_CACHE:
        _PROG_CACHE[pk] = {
            "A": _build_A(pl),
            "BC": _build_BC(pl),
            "D": _build_D(pl),
        }
    progs = _PROG_CACHE[pk]

    LAST_EXEC_NS.clear()
    W1 = ins["W1"].astype(BF16_NP)
    # Launch A: T1 = dinv * (x @ W1)
    resA = _run(progs["A"], [
        {"xT": pl.cores[c]["xT"], "W": W1, "dinv": pl.cores[c]["dinv"]}
        for c in range(NCORES)
    ])
    shardsA = [r["Tout"] for r in resA]
    T1 = _assemble_table(pl, shardsA)

    def meta(c):
        cc = pl.cores[c]
        return {"idxlo": cc["idxlo"], "idxhi": cc["idxhi"],
                "dstloc": cc["dstloc"], "iota_rep": pl.iota_rep,
                "dinv": cc["dinv"]}

    def vec(name):
        return ins[name].astype(np.float32).reshape(H, 1)

    # Launch B: layer-1 scatter + BN1/ReLU + @W2
    resB = _run(progs["BC"], [
        {**meta(c), "T": T1, "own": shardsA[c], "W": ins["W2"].astype(BF16_NP),
         "bvec": vec("b1"), "bn_g": vec("bn1_g"), "bn_b": vec("bn1_b"),
         "bn_m": vec("bn1_m"), "bn_v": vec("bn1_v")} for c in range(NCORES)
    ])
    shardsB = [r["Tout"] for r in resB]
    T2 = _assemble_table(pl, shardsB)

    # Launch C: layer-2 scatter + BN2/ReLU + @W3
    resC = _run(progs["BC"], [
        {**meta(c), "T": T2, "own": shardsB[c], "W": ins["W3"].astype(BF16_NP),
         "bvec": vec("b2"), "bn_g": vec("bn2_g"), "bn_b": vec("bn2_b"),
         "bn_m": vec("bn2_m"), "bn_v": vec("bn2_v")} for c in range(NCORES)
    ])
    shardsC = [r["Tout"] for r in resC]
    T3 = _assemble_table(pl, shardsC)

    # Launch D: layer-3 scatter + pooling partials
    resD = _run(progs["D"], [
        {**meta(c), "T": T3, "own": shardsC[c],
         "batchval": pl.cores[c]["batchval"], "giota": pl.giota}
        for c in range(NCORES)
    ])
    pooled_sum = np.sum([r["pool"] for r in resD], axis=0).astype(np.float64)

    counts = pl.counts.astype(np.float64)
    pooled_sum += counts[:, None] * ins["b3"].astype(np.float64)[None, :]
    pooled = pooled_sum / np.maximum(counts, 1.0)[:, None]

    z = np.maximum(pooled @ ins["Wc1"].astype(np.float64)
                   + ins["bc1"].astype(np.float64), 0.0)
    out = z @ ins["Wc2"].astype(np.float64) + ins["bc2"].astype(np.float64)
    return out.astype(np.float32)


# revision 21
# speedup vs baseline: 1.4114x; 1.0241x over previous
"""Trainium2 Bass kernel for DocumentClassificationGNN (3-layer GCN + BN/ReLU +
global mean pool + MLP head), distributed over 8 NeuronCores.

Strategy (node/graph parallel, per the sharding hint):
  - Nodes are assigned to (core, slot) sorted by in-degree so every core/tile
    carries a balanced edge load.  Edges are partitioned by DESTINATION core so
    the segment-sum scatter is device-local.
  - Per layer: a dense GEMM produces a node-major fp16 feature table that the
    host replicates to all cores ("all-gather" through the host between
    launches); each core gathers its in-edge source rows with dma_gather and
    scatter-adds them into PSUM with one-hot matmuls.
  - The symmetric norm deg^-1/2[src]*deg^-1/2[dst] is SEPARABLE: table rows
    are pre-scaled by dinv[src] at write time and the scatter output is
    post-scaled by dinv[dst], so the one-hot matrices are pure 0/1 and are
    generated in batched DVE is_equal ops (2-byte fast path) with the chunk
    dim innermost: s_t[p, j, c].
  - Self-loops never enter the edge stream: each tile's own table rows are
    bulk-loaded and added via one identity matmul (contribution dinv_d*T'[d]).
  - conv bias + BN + ReLU fuse into one scalar-engine activation; GEMMs run in
    bf16; launch D does per-tile onehot(batch) pooling accumulated in one PSUM
    bank.
  - Device output: per-core pooled partial sums [64, 128].  Host: sum, +n_g*b3,
    divide by counts, tiny classifier MLP.

Programs (3 compiles, 4 launches):
  A : T1 = dinv * (x @ W1)                          -> T1 table shard
  BC: Y = scatter(T); h' = relu(BN(dinv*Y + b)); T' = dinv * (h' @ W_next)
  D : Y3 = scatter(T3); pooled_partial = onehot(batch)^T @ (dinv*Y3)
"""

import hashlib
import numpy as np
from contextlib import ExitStack

import ml_dtypes

import concourse.bass as bass
import concourse.bacc as bacc
import concourse.tile as tile
from concourse import mybir
from concourse.bass_utils import run_bass_kernel_spmd
from concourse.masks import make_identity

P = 128
NCORES = 8
N = 50000
D_IN = 256
H = 128
NGRAPH = 64
SLOTS = 6272            # 49 tiles of 128 slots per core (6250 real nodes + pad)
TILES = SLOTS // P      # 49
RAW = NCORES * SLOTS    # 50176
LOB = 32767             # table row 32767 is the lo-region zero row
TAB = RAW + 2           # +2 zero rows (lo @32767, hi @TAB-1)
ZLO = LOB               # lo-local zero row index
ZHI_LOCAL = TAB - 1 - 32768   # hi-local zero row index
GS = 7                  # dst tiles per gather group
NGROUPS = TILES // GS   # 7
BN_EPS = 1e-5

SCRATCH = 16384         # SWDGE ring: 16384/16 = 1024 descriptors per queue
MAXCH = 8               # chunks per dma_gather call (8*128 = 1024, HW limit)
NQ = 2                  # SWDGE queues (desc-gen pipelines against transfer)

F16 = mybir.dt.float16
BF16 = mybir.dt.bfloat16
F32 = mybir.dt.float32
I16 = mybir.dt.int16
BF16_NP = ml_dtypes.bfloat16

# module-level knobs / perf results (test.py pokes these)
TRACE = False
LAST_EXEC_NS = []       # per-launch exec_time_ns (when TRACE)

_PLAN_CACHE = {}
_PROG_CACHE = {}


# ---------------------------------------------------------------- host prep --

def _wrap_idx(flat):
    """dma_gather index layout: idx i -> [i%16, i//16], replicated to 128 parts."""
    n = len(flat)
    assert n % 16 == 0
    arr = np.asarray(flat, dtype=np.int16).reshape(n // 16, 16).T.copy()
    return np.tile(arr, (8, 1))


class _Plan:
    pass


def _make_plan(edge_index, batch, x):
    pl = _Plan()
    src = np.asarray(edge_index[0], dtype=np.int64)
    dst = np.asarray(edge_index[1], dtype=np.int64)
    batch = np.asarray(batch, dtype=np.int64)

    deg = np.bincount(dst, minlength=N).astype(np.int64) + 1
    dinv = (1.0 / np.sqrt(deg)).astype(np.float32)

    order = np.argsort(-deg, kind="stable")
    rank = np.empty(N, dtype=np.int64)
    rank[order] = np.arange(N)
    core_of = rank % NCORES
    slot_of = rank // NCORES
    raw_of = core_of * SLOTS + slot_of
    grow_of = raw_of + (raw_of >= LOB)      # table row per node

    # real edges only: self-loops are handled by the per-tile identity matmul
    es, ed = src, dst
    ecore = core_of[ed]
    eslot = slot_of[ed]
    etile = eslot // P
    edstloc = eslot % P
    esg = grow_of[es]
    islo = esg < LOB

    # per-core sorted segment arrays
    NSEG = TILES * 2   # segment id: 2*tile + (0 lo / 1 hi)
    per_core = []
    seg_counts = np.zeros((NCORES, NSEG), dtype=np.int64)
    for c in range(NCORES):
        m = ecore == c
        seg = etile[m] * 2 + (~islo[m]).astype(np.int64)
        o2 = np.lexsort((esg[m], seg))
        d = {
            "seg": seg[o2],
            "dstloc": edstloc[m][o2],
            "esg": esg[m][o2],
        }
        seg_counts[c] = np.bincount(d["seg"], minlength=NSEG)
        per_core.append(d)

    # chunk plan: per tile, lo/hi chunk counts = max over cores
    CLO = np.ceil(seg_counts[:, 0::2].max(axis=0) / P).astype(int)
    CHI = np.ceil(seg_counts[:, 1::2].max(axis=0) / P).astype(int)
    # chunk order: group-major; within group: all lo chunks (tile order), then hi
    seg_chunk_start = np.zeros(NSEG, dtype=np.int64)   # global chunk idx per seg
    grp_clo = np.zeros(NGROUPS, dtype=np.int64)
    grp_chi = np.zeros(NGROUPS, dtype=np.int64)
    gcb = np.zeros(NGROUPS + 1, dtype=np.int64)
    for g in range(NGROUPS):
        ts = range(g * GS, (g + 1) * GS)
        grp_clo[g] = sum(CLO[t] for t in ts)
        grp_chi[g] = sum(CHI[t] for t in ts)
        ofs = gcb[g]
        for t in ts:
            seg_chunk_start[2 * t] = ofs
            ofs += CLO[t]
        for t in ts:
            seg_chunk_start[2 * t + 1] = ofs
            ofs += CHI[t]
        gcb[g + 1] = ofs
    CTOT = int(gcb[-1])

    # per-chunk default fill (pads): lo chunks -> ZLO, hi chunks -> absolute hi zero
    chunk_is_hi = np.zeros(CTOT, dtype=bool)
    for t in range(TILES):
        s = seg_chunk_start[2 * t + 1]
        chunk_is_hi[s:s + CHI[t]] = True

    pl.cores = []
    for c in range(NCORES):
        d = per_core[c]
        npad = CTOT * P
        dstloc_pad = np.zeros(npad, dtype=np.float16)
        row_pad = np.where(np.repeat(chunk_is_hi, P), TAB - 1, ZLO).astype(np.int64)
        # position of each real edge
        cnt = seg_counts[c]
        seg_first = np.concatenate([[0], np.cumsum(cnt)[:-1]])
        within = np.arange(len(d["seg"])) - seg_first[d["seg"]]
        pos = seg_chunk_start[d["seg"]] * P + within
        dstloc_pad[pos] = d["dstloc"].astype(np.float16)
        row_pad[pos] = d["esg"]

        # gather index arrays (lo then hi, group-major)
        lo_parts, hi_parts = [], []
        for g in range(NGROUPS):
            a = gcb[g] * P
            b = a + grp_clo[g] * P
            e = gcb[g + 1] * P
            lo_parts.append(row_pad[a:b])
            hi_parts.append(row_pad[b:e] - 32768)
        lo_flat = np.concatenate(lo_parts)
        hi_flat = np.concatenate(hi_parts)
        assert lo_flat.min() >= 0 and lo_flat.max() <= LOB
        assert hi_flat.min() >= 0 and hi_flat.max() <= ZHI_LOCAL

        core = {
            "idxlo": _wrap_idx(lo_flat),
            "idxhi": _wrap_idx(hi_flat),
            "dstloc": dstloc_pad.reshape(CTOT, P).T.copy(),
        }
        pl.cores.append(core)

    # group gather call metadata (columns into wrapped idx tensors)
    pl.lo_cols = int(grp_clo.sum() * P // 16)
    pl.hi_cols = int(grp_chi.sum() * P // 16)
    lo_c0 = np.concatenate([[0], np.cumsum(grp_clo * 8)])
    hi_c0 = np.concatenate([[0], np.cumsum(grp_chi * 8)])
    pl.groups = []
    for g in range(NGROUPS):
        tiles = []
        for t in range(g * GS, (g + 1) * GS):
            lo_local = int(seg_chunk_start[2 * t] - gcb[g])
            hi_local = int(seg_chunk_start[2 * t + 1] - gcb[g])
            tiles.append({
                "clo": int(CLO[t]), "chi": int(CHI[t]),
                "sp_lo": lo_local, "sp_hi": hi_local,
                "gc_lo": int(seg_chunk_start[2 * t]),
                "gc_hi": int(seg_chunk_start[2 * t + 1]),
            })
        pl.groups.append({
            "nclo": int(grp_clo[g]), "nchi": int(grp_chi[g]),
            "lo_col0": int(lo_c0[g]), "hi_col0": int(hi_c0[g]),
            "tiles": tiles,
        })
    pl.CTOT = CTOT
    pl.NCHMAX = int(max(CLO.max(), CHI.max()))

    # slot -> node map, batch values, dinv per slot, xT shards, table row map
    node_at = np.full((NCORES, SLOTS), -1, dtype=np.int64)
    node_at[core_of, slot_of] = np.arange(N)
    bv = np.full((NCORES, SLOTS), 99.0, dtype=np.float16)
    dv = np.ones((NCORES, SLOTS), dtype=np.float32)
    valid = node_at >= 0
    bv[valid] = batch[node_at[valid]].astype(np.float16)
    dv[valid] = dinv[node_at[valid]]
    for c in range(NCORES):
        pl.cores[c]["batchval"] = bv[c].reshape(TILES, P).T.copy()  # [128, 49]
        pl.cores[c]["dinv"] = dv[c].reshape(TILES, P).T.copy()      # [128, 49]
        xt = np.zeros((D_IN, SLOTS), dtype=np.float32)
        v = valid[c]
        xt[:, v] = np.asarray(x, dtype=np.float32)[node_at[c][v]].T
        pl.cores[c]["xT"] = xt.astype(BF16_NP)

    rm = np.arange(RAW, dtype=np.int64)
    pl.rowmap = (rm + (rm >= LOB)).reshape(NCORES, SLOTS)
    pl.counts = np.bincount(batch, minlength=NGRAPH).astype(np.float32)
    pl.iota_rep = np.repeat(np.arange(P), pl.NCHMAX).astype(np.float16).reshape(1, -1)
    pl.giota = np.repeat(np.arange(NGRAPH), TILES).astype(np.float16).reshape(1, -1)
    pl.key = (tuple(CLO), tuple(CHI))
    return pl


# ---------------------------------------------------------- program builders --

GEMM_CH = 512
GEMM_NCHUNK = (SLOTS + GEMM_CH - 1) // GEMM_CH


def _make_gemm_emitter(nc, ctx, tc, k_tiles, o_T, identB, dinv_sb,
                       tps_pool=None, bufs=2):
    """Returns emit(k): xw^T[fout, slot-chunk k] = sum_k lhsT^T @ rhs (bf16);
    transpose + dinv-scale + cast -> o_T rows (fp16)."""
    gps_pool = ctx.enter_context(
        tc.tile_pool(name="gemm_ps", bufs=bufs, space="PSUM"))
    gcp_pool = ctx.enter_context(tc.tile_pool(name="gemm_cp", bufs=bufs + 1))
    if tps_pool is None:
        tps_pool = ctx.enter_context(
            tc.tile_pool(name="gemm_tp", bufs=bufs + 2, space="PSUM"))
    to_pool = ctx.enter_context(tc.tile_pool(name="gemm_to", bufs=bufs + 2))

    def emit(k):
        o = k * GEMM_CH
        w = min(GEMM_CH, SLOTS - o)
        gps = gps_pool.tile([P, GEMM_CH], F32, space="PSUM")
        for ki, (lhsT, rhs) in enumerate(k_tiles):
            nc.tensor.matmul(
                out=gps[:, :w],
                lhsT=lhsT[:],
                rhs=rhs[:, o:o + w],
                start=(ki == 0), stop=(ki == len(k_tiles) - 1),
            )
        gcp = gcp_pool.tile([P, GEMM_CH], BF16)
        nc.scalar.activation(out=gcp[:, :w], in_=gps[:, :w],
                             func=mybir.ActivationFunctionType.Copy)
        for s in range(0, w, P):
            blk = (o + s) // P
            tp = tps_pool.tile([P, P], BF16, space="PSUM", tag="tp")
            nc.tensor.transpose(out=tp[:], in_=gcp[:, s:s + P], identity=identB[:])
            to = to_pool.tile([P, H], F16)
            nc.scalar.activation(out=to[:], in_=tp[:],
                                 func=mybir.ActivationFunctionType.Copy,
                                 scale=dinv_sb[:, blk:blk + 1])
            nc.sync.dma_start(out=o_T[o + s:o + s + P, :], in_=to[:])

    return emit


def _build_A(pl):
    nc = bacc.Bacc("TRN2", target_bir_lowering=False, debug=False, num_devices=NCORES)
    i_xT = nc.dram_tensor("xT", [D_IN, SLOTS], BF16, kind="ExternalInput").ap()
    i_W = nc.dram_tensor("W", [D_IN, H], BF16, kind="ExternalInput").ap()
    i_dinv = nc.dram_tensor("dinv", [P, TILES], F32, kind="ExternalInput").ap()
    o_T = nc.dram_tensor("Tout", [SLOTS, H], F16, kind="ExternalOutput").ap()
    with tile.TileContext(nc) as tc:
        with ExitStack() as ctx:
            const = ctx.enter_context(tc.tile_pool(name="const", bufs=1))
            identB = const.tile([P, P], BF16)
            make_identity(nc, identB[:])
            dinv_sb = const.tile([P, TILES], F32)
            nc.sync.dma_start(out=dinv_sb[:], in_=i_dinv[:])
            w0 = const.tile([P, H], BF16)
            nc.sync.dma_start(out=w0[:], in_=i_W[0:P, :])
            w1 = const.tile([P, H], BF16)
            nc.sync.dma_start(out=w1[:], in_=i_W[P:2 * P, :])
            x0 = const.tile([P, SLOTS], BF16)
            x1 = const.tile([P, SLOTS], BF16)
            for o in range(0, SLOTS, GEMM_CH):
                w = min(GEMM_CH, SLOTS - o)
                nc.sync.dma_start(out=x0[:, o:o + w], in_=i_xT[0:P, o:o + w])
                nc.sync.dma_start(out=x1[:, o:o + w], in_=i_xT[P:2 * P, o:o + w])
            emit = _make_gemm_emitter(nc, ctx, tc, [(w0, x0), (w1, x1)], o_T,
                                      identB, dinv_sb, bufs=3)
            for k in range(GEMM_NCHUNK):
                emit(k)
    nc.compile()
    return nc


def _scatter_body(nc, ctx, tc, pl, i_T, consume_tile, after_tile=None):
    """Shared gather + one-hot matmul scatter loop.

    consume_tile(t, ypsum) handles the per-tile PSUM result
    (ypsum = sum over in-edges of dinv[src]-scaled source rows, incl self-loop).
    """
    const = ctx.enter_context(tc.tile_pool(name="sc_const", bufs=1))
    stage = ctx.enter_context(tc.tile_pool(name="staging", bufs=2))
    st_pool = ctx.enter_context(tc.tile_pool(name="st", bufs=4))
    yp_pool = ctx.enter_context(tc.tile_pool(name="yps", bufs=3, space="PSUM"))

    i_idxlo = nc.dram_tensor("idxlo", [P, pl.lo_cols], I16, kind="ExternalInput").ap()
    i_idxhi = nc.dram_tensor("idxhi", [P, pl.hi_cols], I16, kind="ExternalInput").ap()
    i_dstloc = nc.dram_tensor("dstloc", [P, pl.CTOT], F16, kind="ExternalInput").ap()
    i_iota = nc.dram_tensor("iota_rep", [1, P * pl.NCHMAX], F16,
                            kind="ExternalInput").ap()
    i_ownT = nc.dram_tensor("ownT", [H, SLOTS], F16, kind="ExternalInput").ap()

    idxlo_sb = const.tile([P, pl.lo_cols], I16)
    nc.sync.dma_start(out=idxlo_sb[:], in_=i_idxlo[:])
    idxhi_sb = const.tile([P, pl.hi_cols], I16)
    nc.sync.dma_start(out=idxhi_sb[:], in_=i_idxhi[:])
    dstloc_sb = const.tile([P, pl.CTOT], F16)
    nc.sync.dma_start(out=dstloc_sb[:], in_=i_dstloc[:])
    iota_sb = const.tile([P, P * pl.NCHMAX], F16)
    nc.sync.dma_start(out=iota_sb[:], in_=i_iota.to_broadcast([P, P * pl.NCHMAX]))
    iota3 = iota_sb[:].rearrange("p (j c) -> p j c", j=P, c=pl.NCHMAX)
    identH = const.tile([P, P], F16)
    make_identity(nc, identH[:])
    ownT_sb = const.tile([P, SLOTS], F16)
    nc.sync.dma_start(out=ownT_sb[:], in_=i_ownT[:])

    qn = [0]

    def gather(staging, base, src_ap, idx_sb, col0, nch):
        for o in range(0, nch, MAXCH):
            n = min(MAXCH, nch - o)
            c0 = col0 + o * 8
            nc.gpsimd.dma_gather(
                out_ap=staging[:, base + o:base + o + n, :], in_ap=src_ap,
                idxs_ap=idx_sb[:, c0:c0 + n * 8],
                num_idxs=n * P, num_idxs_reg=n * P, elem_size=H,
                queue_num=qn[0])
            qn[0] = (qn[0] + 1) % NQ

    def onehot(gc0, nch):
        st = st_pool.tile([P, P, nch], F16, tag="st")
        nc.vector.tensor_tensor(
            out=st[:],
            in0=iota3[:, :, 0:nch],
            in1=dstloc_sb[:, gc0:gc0 + nch].unsqueeze(1).to_broadcast([P, P, nch]),
            op=mybir.AluOpType.is_equal)
        return st

    for g, grp in enumerate(pl.groups):
        nclo, nchi = grp["nclo"], grp["nchi"]
        staging = stage.tile([P, nclo + nchi, H], F16, tag="staging")
        gather(staging, 0, i_T[:], idxlo_sb, grp["lo_col0"], nclo)
        gather(staging, nclo, i_T[32768:, :], idxhi_sb, grp["hi_col0"], nchi)
        for ti, td in enumerate(grp["tiles"]):
            t = g * GS + ti
            stlo = onehot(td["gc_lo"], td["clo"]) if td["clo"] else None
            sthi = onehot(td["gc_hi"], td["chi"]) if td["chi"] else None
            ypsum = yp_pool.tile([P, H], F32, space="PSUM")
            # self-loop rows: ypsum = ownT_tile^T @ I  (= own rows, [slot, feat])
            nc.tensor.matmul(out=ypsum[:], lhsT=ownT_sb[:, t * P:(t + 1) * P],
                             rhs=identH[:],
                             start=True, stop=(td["clo"] + td["chi"] == 0))
            for i in range(td["clo"]):
                nc.tensor.matmul(
                    out=ypsum[:], lhsT=stlo[:, :, i],
                    rhs=staging[:, td["sp_lo"] + i, :],
                    start=False,
                    stop=(i == td["clo"] - 1 and td["chi"] == 0))
            for i in range(td["chi"]):
                nc.tensor.matmul(
                    out=ypsum[:], lhsT=sthi[:, :, i],
                    rhs=staging[:, td["sp_hi"] + i, :],
                    start=False, stop=(i == td["chi"] - 1))
            consume_tile(t, ypsum)
            if after_tile is not None:
                after_tile(t)


def _vec_input(nc, const, name):
    ap = nc.dram_tensor(name, [H, 1], F32, kind="ExternalInput").ap()
    sb = const.tile([H, 1], F32, tag=f"vec_{name}")
    nc.sync.dma_start(out=sb[:], in_=ap[:])
    return sb


def _build_BC(pl):
    nc = bacc.Bacc("TRN2", target_bir_lowering=False, debug=False,
                   num_devices=NCORES, dynamic_dma_scratch_size=SCRATCH,
                   num_swdge_queues=NQ)
    i_T = nc.dram_tensor("T", [TAB, H], F16, kind="ExternalInput").ap()
    i_W = nc.dram_tensor("W", [H, H], BF16, kind="ExternalInput").ap()
    i_dinv = nc.dram_tensor("dinv", [P, TILES], F32, kind="ExternalInput").ap()
    o_T = nc.dram_tensor("Tout", [SLOTS, H], F16, kind="ExternalOutput").ap()
    with tile.TileContext(nc) as tc:
        with ExitStack() as ctx:
            const = ctx.enter_context(tc.tile_pool(name="bc_const", bufs=1))
            ycp_pool = ctx.enter_context(tc.tile_pool(name="ycp", bufs=3))
            tps_pool = ctx.enter_context(tc.tile_pool(name="tps", bufs=2, space="PSUM"))

            b_sb = _vec_input(nc, const, "bvec")
            g_sb = _vec_input(nc, const, "bn_g")
            bb_sb = _vec_input(nc, const, "bn_b")
            m_sb = _vec_input(nc, const, "bn_m")
            v_sb = _vec_input(nc, const, "bn_v")
            # scale = g / sqrt(v+eps); bias = (b - m)*scale + beta
            eps = const.tile([H, 1], F32)
            nc.vector.memset(eps[:], BN_EPS)
            sq = const.tile([H, 1], F32)
            nc.scalar.activation(out=sq[:], in_=v_sb[:],
                                 func=mybir.ActivationFunctionType.Sqrt,
                                 bias=eps[:], scale=1.0)
            rs = const.tile([H, 1], F32)
            nc.vector.reciprocal(out=rs[:], in_=sq[:])
            scale = const.tile([H, 1], F32)
            nc.vector.tensor_mul(out=scale[:], in0=rs[:], in1=g_sb[:])
            bias = const.tile([H, 1], F32)
            nc.vector.tensor_sub(out=bias[:], in0=b_sb[:], in1=m_sb[:])
            nc.vector.tensor_mul(out=bias[:], in0=bias[:], in1=scale[:])
            nc.vector.tensor_add(out=bias[:], in0=bias[:], in1=bb_sb[:])

            identB = const.tile([P, P], BF16)
            make_identity(nc, identB[:])
            dinv_sb = const.tile([P, TILES], F32)
            nc.sync.dma_start(out=dinv_sb[:], in_=i_dinv[:])
            w_sb = const.tile([H, H], BF16)
            nc.sync.dma_start(out=w_sb[:], in_=i_W[:])
            hT = const.tile([P, SLOTS], BF16)

            def consume(t, ypsum):
                ycp = ycp_pool.tile([P, H], BF16)
                nc.scalar.activation(out=ycp[:], in_=ypsum[:],
                                     func=mybir.ActivationFunctionType.Copy,
                                     scale=dinv_sb[:, t:t + 1])
                tp = tps_pool.tile([P, P], BF16, space="PSUM")
                nc.tensor.transpose(out=tp[:], in_=ycp[:], identity=identB[:])
                nc.scalar.activation(
                    out=hT[:, t * P:(t + 1) * P], in_=tp[:],
                    func=mybir.ActivationFunctionType.Relu,
                    bias=bias[:], scale=scale[:])

            emit = _make_gemm_emitter(nc, ctx, tc, [(w_sb, hT)], o_T,
                                      identB, dinv_sb, tps_pool=tps_pool)

            def after_tile(t):
                # gemm chunk k covers slots [512k, 512k+512) = tiles 4k..4k+3;
                # emit as soon as its hT columns are complete so the GEMM
                # overlaps the remaining scatter instead of tailing it.
                if t % 4 == 3:
                    emit(t // 4)
                elif t == TILES - 1:
                    emit(GEMM_NCHUNK - 1)

            _scatter_body(nc, ctx, tc, pl, i_T, consume, after_tile)
    nc.compile()
    return nc


def _build_D(pl):
    nc = bacc.Bacc("TRN2", target_bir_lowering=False, debug=False,
                   num_devices=NCORES, dynamic_dma_scratch_size=SCRATCH,
                   num_swdge_queues=NQ)
    i_T = nc.dram_tensor("T", [TAB, H], F16, kind="ExternalInput").ap()
    i_bv = nc.dram_tensor("batchval", [P, TILES], F16, kind="ExternalInput").ap()
    i_gi = nc.dram_tensor("giota", [1, NGRAPH * TILES], F16,
                          kind="ExternalInput").ap()
    i_dinv = nc.dram_tensor("dinv", [P, TILES], F32, kind="ExternalInput").ap()
    o_pool = nc.dram_tensor("pool", [NGRAPH, H], F32, kind="ExternalOutput").ap()
    with tile.TileContext(nc) as tc:
        with ExitStack() as ctx:
            const = ctx.enter_context(tc.tile_pool(name="d_const", bufs=1))
            h3_pool = ctx.enter_context(tc.tile_pool(name="h3", bufs=3))
            pp_pool = ctx.enter_context(tc.tile_pool(name="pp", bufs=1, space="PSUM"))

            bv_sb = const.tile([P, TILES], F16)
            nc.sync.dma_start(out=bv_sb[:], in_=i_bv[:])
            gi_sb = const.tile([P, NGRAPH * TILES], F16)
            nc.sync.dma_start(out=gi_sb[:], in_=i_gi.to_broadcast([P, NGRAPH * TILES]))
            dinv_sb = const.tile([P, TILES], F32)
            nc.sync.dma_start(out=dinv_sb[:], in_=i_dinv[:])
            # oh_all[p, g, t] = (batchval[p, t] == g)
            oh_all = const.tile([P, NGRAPH, TILES], F16)
            nc.vector.tensor_tensor(
                out=oh_all[:],
                in0=gi_sb[:].rearrange("p (g t) -> p g t", g=NGRAPH, t=TILES),
                in1=bv_sb[:].unsqueeze(1).to_broadcast([P, NGRAPH, TILES]),
                op=mybir.AluOpType.is_equal)
            pp = pp_pool.tile([NGRAPH, H], F32, space="PSUM")

            def consume(t, ypsum):
                h3 = h3_pool.tile([P, H], F16)
                nc.scalar.activation(out=h3[:], in_=ypsum[:],
                                     func=mybir.ActivationFunctionType.Copy,
                                     scale=dinv_sb[:, t:t + 1])
                nc.tensor.matmul(out=pp[:], lhsT=oh_all[:, :, t], rhs=h3[:],
                                 start=(t == 0), stop=(t == TILES - 1))

            _scatter_body(nc, ctx, tc, pl, i_T, consume)
            pcp = const.tile([NGRAPH, H], F32)
            nc.vector.tensor_copy(out=pcp[:], in_=pp[:])
            nc.sync.dma_start(out=o_pool[:], in_=pcp[:])
    nc.compile()
    return nc


# ------------------------------------------------------------------- driver --

def _run(nc, in_maps):
    res = run_bass_kernel_spmd(nc, in_maps, core_ids=list(range(NCORES)),
                               trace=TRACE)
    if TRACE:
        LAST_EXEC_NS.append(res.exec_time_ns)
    return res.results


def _assemble_table(pl, shards):
    T = np.zeros((TAB, H), dtype=np.float16)
    for c in range(NCORES):
        T[pl.rowmap[c]] = shards[c]
    return T


def kernel(**inputs):
    ins = {k: np.asarray(v) for k, v in inputs.items()}
    key = hashlib.sha1(
        ins["edge_index"].tobytes() + ins["batch"].tobytes()
    ).hexdigest()
    if key not in _PLAN_CACHE:
        _PLAN_CACHE[key] = _make_plan(ins["edge_index"], ins["batch"], ins["x"])
    pl = _PLAN_CACHE[key]

    pk = pl.key
    if pk not in _PROG_CACHE:
        _PROG_CACHE[pk] = {
            "A": _build_A(pl),
            "BC": _build_BC(pl),
            "D": _build_D(pl),
        }
    progs = _PROG_CACHE[pk]

    LAST_EXEC_NS.clear()
    W1 = ins["W1"].astype(BF16_NP)
    # Launch A: T1 = dinv * (x @ W1)
    resA = _run(progs["A"], [
        {"xT": pl.cores[c]["xT"], "W": W1, "dinv": pl.cores[c]["dinv"]}
        for c in range(NCORES)
    ])
    shardsA = [r["Tout"] for r in resA]
    T1 = _assemble_table(pl, shardsA)

    def meta(c):
        cc = pl.cores[c]
        return {"idxlo": cc["idxlo"], "idxhi": cc["idxhi"],
                "dstloc": cc["dstloc"], "iota_rep": pl.iota_rep,
                "dinv": cc["dinv"]}

    def ownT(shard):
        return np.ascontiguousarray(shard.T)

    def vec(name):
        return ins[name].astype(np.float32).reshape(H, 1)

    # Launch B: layer-1 scatter + BN1/ReLU + @W2
    resB = _run(progs["BC"], [
        {**meta(c), "T": T1, "ownT": ownT(shardsA[c]), "W": ins["W2"].astype(BF16_NP),
         "bvec": vec("b1"), "bn_g": vec("bn1_g"), "bn_b": vec("bn1_b"),
         "bn_m": vec("bn1_m"), "bn_v": vec("bn1_v")} for c in range(NCORES)
    ])
    shardsB = [r["Tout"] for r in resB]
    T2 = _assemble_table(pl, shardsB)

    # Launch C: layer-2 scatter + BN2/ReLU + @W3
    resC = _run(progs["BC"], [
        {**meta(c), "T": T2, "ownT": ownT(shardsB[c]), "W": ins["W3"].astype(BF16_NP),
         "bvec": vec("b2"), "bn_g": vec("bn2_g"), "bn_b": vec("bn2_b"),
         "bn_m": vec("bn2_m"), "bn_v": vec("bn2_v")} for c in range(NCORES)
    ])
    shardsC = [r["Tout"] for r in resC]
    T3 = _assemble_table(pl, shardsC)

    # Launch D: layer-3 scatter + pooling partials
    resD = _run(progs["D"], [
        {**meta(c), "T": T3, "ownT": ownT(shardsC[c]),
         "batchval": pl.cores[c]["batchval"], "giota": pl.giota}
        for c in range(NCORES)
    ])
    pooled_sum = np.sum([r["pool"] for r in resD], axis=0).astype(np.float64)

    counts = pl.counts.astype(np.float64)
    pooled_sum += counts[:, None] * ins["b3"].astype(np.float64)[None, :]
    pooled = pooled_sum / np.maximum(counts, 1.0)[:, None]

    z = np.maximum(pooled @ ins["Wc1"].astype(np.float64)
                   + ins["bc1"].astype(np.float64), 0.0)
    out = z @ ins["Wc2"].astype(np.float64) + ins["bc2"].astype(np.float64)
    return out.astype(np.float32)


# revision 25
# speedup vs baseline: 1.4347x; 1.0166x over previous
"""Trainium2 Bass kernel for DocumentClassificationGNN (3-layer GCN + BN/ReLU +
global mean pool + MLP head), distributed over 8 NeuronCores.

Strategy (node/graph parallel, per the sharding hint):
  - Nodes are assigned to (core, slot) sorted by in-degree so every core/tile
    carries a balanced edge load.  Edges are partitioned by DESTINATION core so
    the segment-sum scatter is device-local.
  - Per layer: a dense GEMM produces a node-major fp16 feature table that the
    host replicates to all cores ("all-gather" through the host between
    launches); each core gathers its in-edge source rows with dma_gather and
    scatter-adds them into PSUM with one-hot matmuls.
  - The symmetric norm deg^-1/2[src]*deg^-1/2[dst] is SEPARABLE: table rows
    are pre-scaled by dinv[src] at write time and the scatter output is
    post-scaled by dinv[dst], so the one-hot matrices are pure 0/1 and are
    generated in batched DVE is_equal ops (2-byte fast path) with the chunk
    dim innermost: s_t[p, j, c].
  - Self-loops never enter the edge stream: each tile's own table rows are
    bulk-loaded and added via one identity matmul (contribution dinv_d*T'[d]).
  - conv bias + BN + ReLU fuse into one scalar-engine activation; GEMMs run in
    bf16; launch D does per-tile onehot(batch) pooling accumulated in one PSUM
    bank.
  - Device output: per-core pooled partial sums [64, 128].  Host: sum, +n_g*b3,
    divide by counts, tiny classifier MLP.

Programs (3 compiles, 4 launches):
  A : T1 = dinv * (x @ W1)                          -> T1 table shard
  BC: Y = scatter(T); h' = relu(BN(dinv*Y + b)); T' = dinv * (h' @ W_next)
  D : Y3 = scatter(T3); pooled_partial = onehot(batch)^T @ (dinv*Y3)
"""

import hashlib
import numpy as np
from contextlib import ExitStack

import ml_dtypes

import concourse.bass as bass
import concourse.bacc as bacc
import concourse.tile as tile
from concourse import mybir
from concourse.bass_utils import run_bass_kernel_spmd
from concourse.masks import make_identity

P = 128
NCORES = 8
N = 50000
D_IN = 256
H = 128
NGRAPH = 64
SLOTS = 6272            # 49 tiles of 128 slots per core (6250 real nodes + pad)
TILES = SLOTS // P      # 49
RAW = NCORES * SLOTS    # 50176
LOB = 32767             # table row 32767 is the lo-region zero row
TAB = RAW + 2           # +2 zero rows (lo @32767, hi @TAB-1)
ZLO = LOB               # lo-local zero row index
ZHI_LOCAL = TAB - 1 - 32768   # hi-local zero row index
GS = 7                  # dst tiles per gather group
NGROUPS = TILES // GS   # 7
BN_EPS = 1e-5

SCRATCH = 16384         # SWDGE ring: 16384/16 = 1024 descriptors per queue
MAXCH = 8               # chunks per dma_gather call (8*128 = 1024, HW limit)
NQ = 2                  # SWDGE queues (desc-gen pipelines against transfer)

F16 = mybir.dt.float16
BF16 = mybir.dt.bfloat16
F32 = mybir.dt.float32
I16 = mybir.dt.int16
BF16_NP = ml_dtypes.bfloat16

# module-level knobs / perf results (test.py pokes these)
TRACE = False
LAST_EXEC_NS = []       # per-launch exec_time_ns (when TRACE)

_PLAN_CACHE = {}
_PROG_CACHE = {}


# ---------------------------------------------------------------- host prep --

def _wrap_idx(flat):
    """dma_gather index layout: idx i -> [i%16, i//16], replicated to 128 parts."""
    n = len(flat)
    assert n % 16 == 0
    arr = np.asarray(flat, dtype=np.int16).reshape(n // 16, 16).T.copy()
    return np.tile(arr, (8, 1))


class _Plan:
    pass


def _make_plan(edge_index, batch, x):
    pl = _Plan()
    src = np.asarray(edge_index[0], dtype=np.int64)
    dst = np.asarray(edge_index[1], dtype=np.int64)
    batch = np.asarray(batch, dtype=np.int64)

    deg = np.bincount(dst, minlength=N).astype(np.int64) + 1
    dinv = (1.0 / np.sqrt(deg)).astype(np.float32)

    order = np.argsort(-deg, kind="stable")
    rank = np.empty(N, dtype=np.int64)
    rank[order] = np.arange(N)
    core_of = rank % NCORES
    slot_of = rank // NCORES
    raw_of = core_of * SLOTS + slot_of
    grow_of = raw_of + (raw_of >= LOB)      # table row per node

    # real edges only: self-loops are handled by the per-tile identity matmul
    es, ed = src, dst
    ecore = core_of[ed]
    eslot = slot_of[ed]
    etile = eslot // P
    edstloc = eslot % P
    esg = grow_of[es]
    islo = esg < LOB

    # per-core sorted segment arrays
    NSEG = TILES * 2   # segment id: 2*tile + (0 lo / 1 hi)
    per_core = []
    seg_counts = np.zeros((NCORES, NSEG), dtype=np.int64)
    for c in range(NCORES):
        m = ecore == c
        seg = etile[m] * 2 + (~islo[m]).astype(np.int64)
        o2 = np.lexsort((esg[m], seg))
        d = {
            "seg": seg[o2],
            "dstloc": edstloc[m][o2],
            "esg": esg[m][o2],
        }
        seg_counts[c] = np.bincount(d["seg"], minlength=NSEG)
        per_core.append(d)

    # chunk plan: per tile, lo/hi chunk counts = max over cores
    CLO = np.ceil(seg_counts[:, 0::2].max(axis=0) / P).astype(int)
    CHI = np.ceil(seg_counts[:, 1::2].max(axis=0) / P).astype(int)
    # chunk order: group-major; within group: all lo chunks (tile order), then hi
    seg_chunk_start = np.zeros(NSEG, dtype=np.int64)   # global chunk idx per seg
    grp_clo = np.zeros(NGROUPS, dtype=np.int64)
    grp_chi = np.zeros(NGROUPS, dtype=np.int64)
    gcb = np.zeros(NGROUPS + 1, dtype=np.int64)
    for g in range(NGROUPS):
        ts = range(g * GS, (g + 1) * GS)
        grp_clo[g] = sum(CLO[t] for t in ts)
        grp_chi[g] = sum(CHI[t] for t in ts)
        ofs = gcb[g]
        for t in ts:
            seg_chunk_start[2 * t] = ofs
            ofs += CLO[t]
        for t in ts:
            seg_chunk_start[2 * t + 1] = ofs
            ofs += CHI[t]
        gcb[g + 1] = ofs
    CTOT = int(gcb[-1])

    # per-chunk default fill (pads): lo chunks -> ZLO, hi chunks -> absolute hi zero
    chunk_is_hi = np.zeros(CTOT, dtype=bool)
    for t in range(TILES):
        s = seg_chunk_start[2 * t + 1]
        chunk_is_hi[s:s + CHI[t]] = True

    pl.cores = []
    for c in range(NCORES):
        d = per_core[c]
        npad = CTOT * P
        dstloc_pad = np.zeros(npad, dtype=np.float16)
        row_pad = np.where(np.repeat(chunk_is_hi, P), TAB - 1, ZLO).astype(np.int64)
        # position of each real edge
        cnt = seg_counts[c]
        seg_first = np.concatenate([[0], np.cumsum(cnt)[:-1]])
        within = np.arange(len(d["seg"])) - seg_first[d["seg"]]
        pos = seg_chunk_start[d["seg"]] * P + within
        dstloc_pad[pos] = d["dstloc"].astype(np.float16)
        row_pad[pos] = d["esg"]

        # gather index arrays (lo then hi, group-major)
        lo_parts, hi_parts = [], []
        for g in range(NGROUPS):
            a = gcb[g] * P
            b = a + grp_clo[g] * P
            e = gcb[g + 1] * P
            lo_parts.append(row_pad[a:b])
            hi_parts.append(row_pad[b:e] - 32768)
        lo_flat = np.concatenate(lo_parts)
        hi_flat = np.concatenate(hi_parts)
        assert lo_flat.min() >= 0 and lo_flat.max() <= LOB
        assert hi_flat.min() >= 0 and hi_flat.max() <= ZHI_LOCAL

        core = {
            "idxlo": _wrap_idx(lo_flat),
            "idxhi": _wrap_idx(hi_flat),
            "dstloc": dstloc_pad.reshape(CTOT, P).T.copy(),
        }
        pl.cores.append(core)

    # group gather call metadata (columns into wrapped idx tensors)
    pl.lo_cols = int(grp_clo.sum() * P // 16)
    pl.hi_cols = int(grp_chi.sum() * P // 16)
    lo_c0 = np.concatenate([[0], np.cumsum(grp_clo * 8)])
    hi_c0 = np.concatenate([[0], np.cumsum(grp_chi * 8)])
    pl.groups = []
    for g in range(NGROUPS):
        tiles = []
        for t in range(g * GS, (g + 1) * GS):
            lo_local = int(seg_chunk_start[2 * t] - gcb[g])
            hi_local = int(seg_chunk_start[2 * t + 1] - gcb[g])
            tiles.append({
                "clo": int(CLO[t]), "chi": int(CHI[t]),
                "sp_lo": lo_local, "sp_hi": hi_local,
                "gc_lo": int(seg_chunk_start[2 * t]),
                "gc_hi": int(seg_chunk_start[2 * t + 1]),
            })
        pl.groups.append({
            "nclo": int(grp_clo[g]), "nchi": int(grp_chi[g]),
            "lo_col0": int(lo_c0[g]), "hi_col0": int(hi_c0[g]),
            "tiles": tiles,
        })
    pl.CTOT = CTOT
    pl.NCHMAX = int(max(CLO.max(), CHI.max()))

    # slot -> node map, batch values, dinv per slot, xT shards, table row map
    node_at = np.full((NCORES, SLOTS), -1, dtype=np.int64)
    node_at[core_of, slot_of] = np.arange(N)
    bv = np.full((NCORES, SLOTS), 99.0, dtype=np.float16)
    dv = np.ones((NCORES, SLOTS), dtype=np.float32)
    valid = node_at >= 0
    bv[valid] = batch[node_at[valid]].astype(np.float16)
    dv[valid] = dinv[node_at[valid]]
    for c in range(NCORES):
        pl.cores[c]["batchval"] = bv[c].reshape(TILES, P).T.copy()  # [128, 49]
        pl.cores[c]["dinv"] = dv[c].reshape(TILES, P).T.copy()      # [128, 49]
        xt = np.zeros((D_IN, SLOTS), dtype=np.float32)
        v = valid[c]
        xt[:, v] = np.asarray(x, dtype=np.float32)[node_at[c][v]].T
        pl.cores[c]["xT"] = xt.astype(BF16_NP)

    rm = np.arange(RAW, dtype=np.int64)
    pl.rowmap = (rm + (rm >= LOB)).reshape(NCORES, SLOTS)
    pl.counts = np.bincount(batch, minlength=NGRAPH).astype(np.float32)
    pl.iota_rep = np.repeat(np.arange(P), pl.NCHMAX).astype(np.float16).reshape(1, -1)
    pl.giota = np.repeat(np.arange(NGRAPH), TILES).astype(np.float16).reshape(1, -1)
    pl.key = (tuple(CLO), tuple(CHI))
    return pl


# ---------------------------------------------------------- program builders --

def _make_gemm_emitter(nc, ctx, tc, k_tiles_fn, o_T, dinv_sb, bufs=2):
    """Returns emit(t): table rows for slot tile t.

    out[slot, fout] = sum_k lhsT_k^T @ rhs_k with lhsT = feat-major input
    block (no output transpose needed); dinv-scale + fp16 cast -> o_T rows.
    """
    gps_pool = ctx.enter_context(
        tc.tile_pool(name="gemm_ps", bufs=bufs, space="PSUM"))
    to_pool = ctx.enter_context(tc.tile_pool(name="gemm_to", bufs=bufs + 2))

    def emit(t):
        kt = k_tiles_fn(t)
        gps = gps_pool.tile([P, H], F32, space="PSUM")
        for ki, (lhsT, rhs) in enumerate(kt):
            nc.tensor.matmul(out=gps[:], lhsT=lhsT, rhs=rhs,
                             start=(ki == 0), stop=(ki == len(kt) - 1))
        to = to_pool.tile([P, H], F16)
        nc.scalar.activation(out=to[:], in_=gps[:],
                             func=mybir.ActivationFunctionType.Copy,
                             scale=dinv_sb[:, t:t + 1])
        nc.sync.dma_start(out=o_T[t * P:(t + 1) * P, :], in_=to[:])

    return emit


def _build_A(pl):
    nc = bacc.Bacc("TRN2", target_bir_lowering=False, debug=False, num_devices=NCORES)
    i_xT = nc.dram_tensor("xT", [D_IN, SLOTS], BF16, kind="ExternalInput").ap()
    i_W = nc.dram_tensor("W", [D_IN, H], BF16, kind="ExternalInput").ap()
    i_dinv = nc.dram_tensor("dinv", [P, TILES], F32, kind="ExternalInput").ap()
    o_T = nc.dram_tensor("Tout", [SLOTS, H], F16, kind="ExternalOutput").ap()
    with tile.TileContext(nc) as tc:
        with ExitStack() as ctx:
            const = ctx.enter_context(tc.tile_pool(name="const", bufs=1))
            dinv_sb = const.tile([P, TILES], F32)
            nc.sync.dma_start(out=dinv_sb[:], in_=i_dinv[:])
            w0 = const.tile([P, H], BF16)
            nc.sync.dma_start(out=w0[:], in_=i_W[0:P, :])
            w1 = const.tile([P, H], BF16)
            nc.sync.dma_start(out=w1[:], in_=i_W[P:2 * P, :])
            x0 = const.tile([P, SLOTS], BF16)
            x1 = const.tile([P, SLOTS], BF16)
            for o in range(0, SLOTS, 512):
                w = min(512, SLOTS - o)
                nc.sync.dma_start(out=x0[:, o:o + w], in_=i_xT[0:P, o:o + w])
                nc.sync.dma_start(out=x1[:, o:o + w], in_=i_xT[P:2 * P, o:o + w])

            def k_tiles(t):
                sl = slice(t * P, (t + 1) * P)
                return [(x0[:, sl], w0[:]), (x1[:, sl], w1[:])]

            emit = _make_gemm_emitter(nc, ctx, tc, k_tiles, o_T, dinv_sb, bufs=3)
            for t in range(TILES):
                emit(t)
    nc.compile()
    return nc


def _scatter_body(nc, ctx, tc, pl, i_T, consume_tile, after_tile=None):
    """Shared gather + one-hot matmul scatter loop.

    consume_tile(t, ypsum) handles the per-tile PSUM result
    (ypsum = sum over in-edges of dinv[src]-scaled source rows, incl self-loop).
    """
    const = ctx.enter_context(tc.tile_pool(name="sc_const", bufs=1))
    stage = ctx.enter_context(tc.tile_pool(name="staging", bufs=2))
    st_pool = ctx.enter_context(tc.tile_pool(name="st", bufs=4))
    yp_pool = ctx.enter_context(tc.tile_pool(name="yps", bufs=3, space="PSUM"))

    i_idxlo = nc.dram_tensor("idxlo", [P, pl.lo_cols], I16, kind="ExternalInput").ap()
    i_idxhi = nc.dram_tensor("idxhi", [P, pl.hi_cols], I16, kind="ExternalInput").ap()
    i_dstloc = nc.dram_tensor("dstloc", [P, pl.CTOT], F16, kind="ExternalInput").ap()
    i_iota = nc.dram_tensor("iota_rep", [1, P * pl.NCHMAX], F16,
                            kind="ExternalInput").ap()
    i_ownT = nc.dram_tensor("ownT", [H, SLOTS], F16, kind="ExternalInput").ap()

    idxlo_sb = const.tile([P, pl.lo_cols], I16)
    nc.sync.dma_start(out=idxlo_sb[:], in_=i_idxlo[:])
    idxhi_sb = const.tile([P, pl.hi_cols], I16)
    nc.sync.dma_start(out=idxhi_sb[:], in_=i_idxhi[:])
    dstloc_sb = const.tile([P, pl.CTOT], F16)
    nc.sync.dma_start(out=dstloc_sb[:], in_=i_dstloc[:])
    iota_sb = const.tile([P, P * pl.NCHMAX], F16)
    nc.sync.dma_start(out=iota_sb[:], in_=i_iota.to_broadcast([P, P * pl.NCHMAX]))
    iota3 = iota_sb[:].rearrange("p (j c) -> p j c", j=P, c=pl.NCHMAX)
    identH = const.tile([P, P], F16)
    make_identity(nc, identH[:])
    ownT_sb = const.tile([P, SLOTS], F16)
    nc.sync.dma_start(out=ownT_sb[:], in_=i_ownT[:])

    qn = [0]

    def gather(staging, base, src_ap, idx_sb, col0, nch):
        for o in range(0, nch, MAXCH):
            n = min(MAXCH, nch - o)
            c0 = col0 + o * 8
            nc.gpsimd.dma_gather(
                out_ap=staging[:, base + o:base + o + n, :], in_ap=src_ap,
                idxs_ap=idx_sb[:, c0:c0 + n * 8],
                num_idxs=n * P, num_idxs_reg=n * P, elem_size=H,
                queue_num=qn[0])
            qn[0] = (qn[0] + 1) % NQ

    def onehot(gc0, nch):
        st = st_pool.tile([P, P, nch], F16, tag="st")
        nc.vector.tensor_tensor(
            out=st[:],
            in0=iota3[:, :, 0:nch],
            in1=dstloc_sb[:, gc0:gc0 + nch].unsqueeze(1).to_broadcast([P, P, nch]),
            op=mybir.AluOpType.is_equal)
        return st

    for g, grp in enumerate(pl.groups):
        nclo, nchi = grp["nclo"], grp["nchi"]
        staging = stage.tile([P, nclo + nchi, H], F16, tag="staging")
        gather(staging, 0, i_T[:], idxlo_sb, grp["lo_col0"], nclo)
        gather(staging, nclo, i_T[32768:, :], idxhi_sb, grp["hi_col0"], nchi)
        for ti, td in enumerate(grp["tiles"]):
            t = g * GS + ti
            stlo = onehot(td["gc_lo"], td["clo"]) if td["clo"] else None
            sthi = onehot(td["gc_hi"], td["chi"]) if td["chi"] else None
            ypsum = yp_pool.tile([P, H], F32, space="PSUM")
            # self-loop rows: ypsum = ownT_tile^T @ I  (= own rows, [slot, feat])
            nc.tensor.matmul(out=ypsum[:], lhsT=ownT_sb[:, t * P:(t + 1) * P],
                             rhs=identH[:],
                             start=True, stop=(td["clo"] + td["chi"] == 0))
            for i in range(td["clo"]):
                nc.tensor.matmul(
                    out=ypsum[:], lhsT=stlo[:, :, i],
                    rhs=staging[:, td["sp_lo"] + i, :],
                    start=False,
                    stop=(i == td["clo"] - 1 and td["chi"] == 0))
            for i in range(td["chi"]):
                nc.tensor.matmul(
                    out=ypsum[:], lhsT=sthi[:, :, i],
                    rhs=staging[:, td["sp_hi"] + i, :],
                    start=False, stop=(i == td["chi"] - 1))
            consume_tile(t, ypsum)
            if after_tile is not None:
                after_tile(t)


def _vec_input(nc, const, name):
    ap = nc.dram_tensor(name, [H, 1], F32, kind="ExternalInput").ap()
    sb = const.tile([H, 1], F32, tag=f"vec_{name}")
    nc.sync.dma_start(out=sb[:], in_=ap[:])
    return sb


def _build_BC(pl):
    nc = bacc.Bacc("TRN2", target_bir_lowering=False, debug=False,
                   num_devices=NCORES, dynamic_dma_scratch_size=SCRATCH,
                   num_swdge_queues=NQ)
    i_T = nc.dram_tensor("T", [TAB, H], F16, kind="ExternalInput").ap()
    i_W = nc.dram_tensor("W", [H, H], BF16, kind="ExternalInput").ap()
    i_dinv = nc.dram_tensor("dinv", [P, TILES], F32, kind="ExternalInput").ap()
    o_T = nc.dram_tensor("Tout", [SLOTS, H], F16, kind="ExternalOutput").ap()
    with tile.TileContext(nc) as tc:
        with ExitStack() as ctx:
            const = ctx.enter_context(tc.tile_pool(name="bc_const", bufs=1))
            ycp_pool = ctx.enter_context(tc.tile_pool(name="ycp", bufs=3))
            h_pool = ctx.enter_context(tc.tile_pool(name="ht", bufs=3))
            tps_pool = ctx.enter_context(tc.tile_pool(name="tps", bufs=2, space="PSUM"))

            b_sb = _vec_input(nc, const, "bvec")
            g_sb = _vec_input(nc, const, "bn_g")
            bb_sb = _vec_input(nc, const, "bn_b")
            m_sb = _vec_input(nc, const, "bn_m")
            v_sb = _vec_input(nc, const, "bn_v")
            # scale = g / sqrt(v+eps); bias = (b - m)*scale + beta
            eps = const.tile([H, 1], F32)
            nc.vector.memset(eps[:], BN_EPS)
            sq = const.tile([H, 1], F32)
            nc.scalar.activation(out=sq[:], in_=v_sb[:],
                                 func=mybir.ActivationFunctionType.Sqrt,
                                 bias=eps[:], scale=1.0)
            rs = const.tile([H, 1], F32)
            nc.vector.reciprocal(out=rs[:], in_=sq[:])
            scale = const.tile([H, 1], F32)
            nc.vector.tensor_mul(out=scale[:], in0=rs[:], in1=g_sb[:])
            bias = const.tile([H, 1], F32)
            nc.vector.tensor_sub(out=bias[:], in0=b_sb[:], in1=m_sb[:])
            nc.vector.tensor_mul(out=bias[:], in0=bias[:], in1=scale[:])
            nc.vector.tensor_add(out=bias[:], in0=bias[:], in1=bb_sb[:])

            identB = const.tile([P, P], BF16)
            make_identity(nc, identB[:])
            dinv_sb = const.tile([P, TILES], F32)
            nc.sync.dma_start(out=dinv_sb[:], in_=i_dinv[:])
            w_sb = const.tile([H, H], BF16)
            nc.sync.dma_start(out=w_sb[:], in_=i_W[:])

            h_tiles = {}

            def consume(t, ypsum):
                ycp = ycp_pool.tile([P, H], BF16)
                nc.scalar.activation(out=ycp[:], in_=ypsum[:],
                                     func=mybir.ActivationFunctionType.Copy,
                                     scale=dinv_sb[:, t:t + 1])
                tp = tps_pool.tile([P, P], BF16, space="PSUM")
                nc.tensor.transpose(out=tp[:], in_=ycp[:], identity=identB[:])
                h_t = h_pool.tile([P, H], BF16)
                nc.scalar.activation(
                    out=h_t[:], in_=tp[:],
                    func=mybir.ActivationFunctionType.Relu,
                    bias=bias[:], scale=scale[:])
                h_tiles[t] = h_t

            emit = _make_gemm_emitter(nc, ctx, tc,
                                      lambda t: [(h_tiles.pop(t)[:], w_sb[:])],
                                      o_T, dinv_sb)

            # emit each tile's GEMM right after its scatter completes so the
            # table write overlaps the remaining scatter instead of tailing it
            _scatter_body(nc, ctx, tc, pl, i_T, consume, emit)
    nc.compile()
    return nc


def _build_D(pl):
    nc = bacc.Bacc("TRN2", target_bir_lowering=False, debug=False,
                   num_devices=NCORES, dynamic_dma_scratch_size=SCRATCH,
                   num_swdge_queues=NQ)
    i_T = nc.dram_tensor("T", [TAB, H], F16, kind="ExternalInput").ap()
    i_bv = nc.dram_tensor("batchval", [P, TILES], F16, kind="ExternalInput").ap()
    i_gi = nc.dram_tensor("giota", [1, NGRAPH * TILES], F16,
                          kind="ExternalInput").ap()
    i_dinv = nc.dram_tensor("dinv", [P, TILES], F32, kind="ExternalInput").ap()
    o_pool = nc.dram_tensor("pool", [NGRAPH, H], F32, kind="ExternalOutput").ap()
    with tile.TileContext(nc) as tc:
        with ExitStack() as ctx:
            const = ctx.enter_context(tc.tile_pool(name="d_const", bufs=1))
            h3_pool = ctx.enter_context(tc.tile_pool(name="h3", bufs=3))
            pp_pool = ctx.enter_context(tc.tile_pool(name="pp", bufs=1, space="PSUM"))

            bv_sb = const.tile([P, TILES], F16)
            nc.sync.dma_start(out=bv_sb[:], in_=i_bv[:])
            gi_sb = const.tile([P, NGRAPH * TILES], F16)
            nc.sync.dma_start(out=gi_sb[:], in_=i_gi.to_broadcast([P, NGRAPH * TILES]))
            dinv_sb = const.tile([P, TILES], F32)
            nc.sync.dma_start(out=dinv_sb[:], in_=i_dinv[:])
            # oh_all[p, g, t] = (batchval[p, t] == g)
            oh_all = const.tile([P, NGRAPH, TILES], F16)
            nc.vector.tensor_tensor(
                out=oh_all[:],
                in0=gi_sb[:].rearrange("p (g t) -> p g t", g=NGRAPH, t=TILES),
                in1=bv_sb[:].unsqueeze(1).to_broadcast([P, NGRAPH, TILES]),
                op=mybir.AluOpType.is_equal)
            pp = pp_pool.tile([NGRAPH, H], F32, space="PSUM")

            def consume(t, ypsum):
                h3 = h3_pool.tile([P, H], F16)
                nc.scalar.activation(out=h3[:], in_=ypsum[:],
                                     func=mybir.ActivationFunctionType.Copy,
                                     scale=dinv_sb[:, t:t + 1])
                nc.tensor.matmul(out=pp[:], lhsT=oh_all[:, :, t], rhs=h3[:],
                                 start=(t == 0), stop=(t == TILES - 1))

            _scatter_body(nc, ctx, tc, pl, i_T, consume)
            pcp = const.tile([NGRAPH, H], F32)
            nc.vector.tensor_copy(out=pcp[:], in_=pp[:])
            nc.sync.dma_start(out=o_pool[:], in_=pcp[:])
    nc.compile()
    return nc


# ------------------------------------------------------------------- driver --

def _run(nc, in_maps):
    res = run_bass_kernel_spmd(nc, in_maps, core_ids=list(range(NCORES)),
                               trace=TRACE)
    if TRACE:
        LAST_EXEC_NS.append(res.exec_time_ns)
    return res.results


def _assemble_table(pl, shards):
    T = np.zeros((TAB, H), dtype=np.float16)
    for c in range(NCORES):
        T[pl.rowmap[c]] = shards[c]
    return T


def kernel(**inputs):
    ins = {k: np.asarray(v) for k, v in inputs.items()}
    key = hashlib.sha1(
        ins["edge_index"].tobytes() + ins["batch"].tobytes()
    ).hexdigest()
    if key not in _PLAN_CACHE:
        _PLAN_CACHE[key] = _make_plan(ins["edge_index"], ins["batch"], ins["x"])
    pl = _PLAN_CACHE[key]

    pk = pl.key
    if pk not in _PROG_CACHE:
        _PROG_CACHE[pk] = {
            "A": _build_A(pl),
            "BC": _build_BC(pl),
            "D": _build_D(pl),
        }
    progs = _PROG_CACHE[pk]

    LAST_EXEC_NS.clear()
    W1 = ins["W1"].astype(BF16_NP)
    # Launch A: T1 = dinv * (x @ W1)
    resA = _run(progs["A"], [
        {"xT": pl.cores[c]["xT"], "W": W1, "dinv": pl.cores[c]["dinv"]}
        for c in range(NCORES)
    ])
    shardsA = [r["Tout"] for r in resA]
    T1 = _assemble_table(pl, shardsA)

    def meta(c):
        cc = pl.cores[c]
        return {"idxlo": cc["idxlo"], "idxhi": cc["idxhi"],
                "dstloc": cc["dstloc"], "iota_rep": pl.iota_rep,
                "dinv": cc["dinv"]}

    def ownT(shard):
        return np.ascontiguousarray(shard.T)

    def vec(name):
        return ins[name].astype(np.float32).reshape(H, 1)

    # Launch B: layer-1 scatter + BN1/ReLU + @W2
    resB = _run(progs["BC"], [
        {**meta(c), "T": T1, "ownT": ownT(shardsA[c]), "W": ins["W2"].astype(BF16_NP),
         "bvec": vec("b1"), "bn_g": vec("bn1_g"), "bn_b": vec("bn1_b"),
         "bn_m": vec("bn1_m"), "bn_v": vec("bn1_v")} for c in range(NCORES)
    ])
    shardsB = [r["Tout"] for r in resB]
    T2 = _assemble_table(pl, shardsB)

    # Launch C: layer-2 scatter + BN2/ReLU + @W3
    resC = _run(progs["BC"], [
        {**meta(c), "T": T2, "ownT": ownT(shardsB[c]), "W": ins["W3"].astype(BF16_NP),
         "bvec": vec("b2"), "bn_g": vec("bn2_g"), "bn_b": vec("bn2_b"),
         "bn_m": vec("bn2_m"), "bn_v": vec("bn2_v")} for c in range(NCORES)
    ])
    shardsC = [r["Tout"] for r in resC]
    T3 = _assemble_table(pl, shardsC)

    # Launch D: layer-3 scatter + pooling partials
    resD = _run(progs["D"], [
        {**meta(c), "T": T3, "ownT": ownT(shardsC[c]),
         "batchval": pl.cores[c]["batchval"], "giota": pl.giota}
        for c in range(NCORES)
    ])
    pooled_sum = np.sum([r["pool"] for r in resD], axis=0).astype(np.float64)

    counts = pl.counts.astype(np.float64)
    pooled_sum += counts[:, None] * ins["b3"].astype(np.float64)[None, :]
    pooled = pooled_sum / np.maximum(counts, 1.0)[:, None]

    z = np.maximum(pooled @ ins["Wc1"].astype(np.float64)
                   + ins["bc1"].astype(np.float64), 0.0)
    out = z @ ins["Wc2"].astype(np.float64) + ins["bc2"].astype(np.float64)
    return out.astype(np.float32)


# revision 39
# speedup vs baseline: 1.5636x; 1.0898x over previous
"""Trainium2 Bass kernel for DocumentClassificationGNN (3-layer GCN + BN/ReLU +
global mean pool + MLP head), distributed over 8 NeuronCores.

Strategy (node/graph parallel, per the sharding hint):
  - Nodes are assigned to (core, slot) sorted by in-degree so every core/tile
    carries a balanced edge load.  Edges are partitioned by DESTINATION core so
    the segment-sum scatter is device-local.
  - Per layer: a dense GEMM produces a node-major fp16 feature table that the
    host replicates to all cores ("all-gather" through the host between
    launches); each core gathers its in-edge source rows with dma_gather and
    scatter-adds them into PSUM with one-hot matmuls.
  - The symmetric norm deg^-1/2[src]*deg^-1/2[dst] is SEPARABLE: table rows
    are pre-scaled by dinv[src] at write time and the scatter output is
    post-scaled by dinv[dst], so the one-hot matrices are pure 0/1 and are
    generated in batched DVE is_equal ops (2-byte fast path) with the chunk
    dim innermost: s_t[p, j, c].
  - Self-loops never enter the edge stream: each tile's own table rows are
    bulk-loaded and added via one identity matmul (contribution dinv_d*T'[d]).
  - conv bias + BN + ReLU fuse into one scalar-engine activation; GEMMs run in
    bf16; launch D does per-tile onehot(batch) pooling accumulated in one PSUM
    bank.
  - Device output: per-core pooled partial sums [64, 128].  Host: sum, +n_g*b3,
    divide by counts, tiny classifier MLP.

Programs (3 compiles, 4 launches):
  A : T1 = dinv * (x @ W1)                          -> T1 table shard
  BC: Y = scatter(T); h' = relu(BN(dinv*Y + b)); T' = dinv * (h' @ W_next)
  D : Y3 = scatter(T3); pooled_partial = onehot(batch)^T @ (dinv*Y3)
"""

import hashlib
import numpy as np
from contextlib import ExitStack

import ml_dtypes

import concourse.bass as bass
import concourse.bacc as bacc
import concourse.tile as tile
from concourse import mybir
from concourse.bass_utils import run_bass_kernel_spmd
from concourse.masks import make_identity

P = 128
NCORES = 8
N = 50000
D_IN = 256
H = 128
NGRAPH = 64
SLOTS = 6272            # 49 tiles of 128 slots per core (6250 real nodes + pad)
TILES = SLOTS // P      # 49
RAW = NCORES * SLOTS    # 50176
TAB = RAW               # table = concatenated shards, no extra rows
HIB = 4 * SLOTS         # hi-region gather base: cores 0-3 lo, cores 4-7 hi
                        # (both index ranges fit the int16 dma_gather indices)
ZLO_ROW = SLOTS - 1     # core-0 pad slot: always-zero row used by lo pads
GS = 7                  # dst tiles per gather group
NGROUPS = TILES // GS   # 7
BN_EPS = 1e-5

SCRATCH = 16384         # SWDGE ring: 16384/16 = 1024 descriptors per queue
MAXCH = 8               # chunks per dma_gather call (8*128 = 1024, HW limit)
NQ = 2                  # SWDGE queues (desc-gen pipelines against transfer)

F16 = mybir.dt.float16
BF16 = mybir.dt.bfloat16
F32 = mybir.dt.float32
I16 = mybir.dt.int16
BF16_NP = ml_dtypes.bfloat16

# module-level knobs / perf results (test.py pokes these)
TRACE = False
LAST_EXEC_NS = []       # per-launch exec_time_ns (when TRACE)

_PLAN_CACHE = {}
_PROG_CACHE = {}


# ---------------------------------------------------------------- host prep --

def _wrap_idx(flat):
    """dma_gather index layout: idx i -> [i%16, i//16], replicated to 128 parts."""
    n = len(flat)
    assert n % 16 == 0
    arr = np.asarray(flat, dtype=np.int16).reshape(n // 16, 16).T.copy()
    return np.tile(arr, (8, 1))


class _Plan:
    pass


def _distribute(total, bins):
    base, extra = divmod(int(total), bins)
    out = np.full(bins, base, dtype=np.int64)
    out[:extra] += 1
    return out


def _pack_core(lo, hi, kL, kH):
    """Pack one core's nodes into TILES tiles of <=128 slots, steering the
    per-tile lo/hi in-edge sums toward the shared chunk budgets kL/kH*128.

    Worst-fit decreasing on min remaining (lo, hi) headroom.
    """
    n = len(lo)
    loR = (kL * P).astype(np.float64)
    hiR = (kH * P).astype(np.float64)
    cap = np.full(TILES, P, dtype=np.int64)
    # all pad (empty) slots must be the LAST slots of the last tile: they are
    # the always-zero rows targeted by gather padding and the table-write memset
    cap[TILES - 1] = P - (TILES * P - n)
    filled = np.zeros(TILES, dtype=np.int64)
    slot = np.empty(n, dtype=np.int64)
    order = np.argsort(-(lo + hi), kind="stable")
    for i in order:
        score = np.minimum(loR - lo[i], hiR - hi[i])
        score[filled >= cap] = -np.inf
        t = int(np.argmax(score))
        loR[t] -= lo[i]
        hiR[t] -= hi[i]
        slot[i] = t * P + filled[t]
        filled[t] += 1
    return slot


def _make_plan(edge_index, batch, x):
    pl = _Plan()
    src = np.asarray(edge_index[0], dtype=np.int64)
    dst = np.asarray(edge_index[1], dtype=np.int64)
    batch = np.asarray(batch, dtype=np.int64)

    deg = np.bincount(dst, minlength=N).astype(np.int64) + 1
    dinv = (1.0 / np.sqrt(deg)).astype(np.float32)

    order = np.argsort(-deg, kind="stable")
    rank = np.empty(N, dtype=np.int64)
    rank[order] = np.arange(N)
    core_of = rank % NCORES

    # lo/hi membership of an edge depends only on its source CORE (the hi
    # gather base sits on the core-3/4 boundary), so per-node lo/hi in-edge
    # counts are fixed before slots are chosen -> bin-pack nodes into tiles
    # so per-(tile, half) counts land just under multiples of 128.
    islo_e = core_of[src] < NCORES // 2
    lo_n = np.bincount(dst[islo_e], minlength=N)
    hi_n = np.bincount(dst[~islo_e], minlength=N)
    totlo = np.zeros(NCORES, dtype=np.int64)
    tothi = np.zeros(NCORES, dtype=np.int64)
    for c in range(NCORES):
        m = core_of == c
        totlo[c] = lo_n[m].sum()
        tothi[c] = hi_n[m].sum()
    SLACK = 2
    kL = _distribute(-(-totlo.max() // P) + SLACK, TILES)
    kH = _distribute(-(-tothi.max() // P) + SLACK, TILES)
    slot_of = np.empty(N, dtype=np.int64)
    for c in range(NCORES):
        nodes = np.where(core_of == c)[0]
        slot_of[nodes] = _pack_core(lo_n[nodes], hi_n[nodes], kL, kH)
    raw_of = core_of * SLOTS + slot_of
    grow_of = raw_of                        # table row per node (pure concat)

    # real edges only: self-loops are handled by the per-tile identity matmul
    es, ed = src, dst
    ecore = core_of[ed]
    eslot = slot_of[ed]
    etile = eslot // P
    edstloc = eslot % P
    esg = grow_of[es]
    islo = islo_e

    # per-core sorted segment arrays
    NSEG = TILES * 2   # segment id: 2*tile + (0 lo / 1 hi)
    per_core = []
    seg_counts = np.zeros((NCORES, NSEG), dtype=np.int64)
    for c in range(NCORES):
        m = ecore == c
        seg = etile[m] * 2 + (~islo[m]).astype(np.int64)
        o2 = np.lexsort((esg[m], seg))
        d = {
            "seg": seg[o2],
            "dstloc": edstloc[m][o2],
            "esg": esg[m][o2],
        }
        seg_counts[c] = np.bincount(d["seg"], minlength=NSEG)
        per_core.append(d)

    # chunk plan: per tile, lo/hi chunk counts = max over cores
    CLO = np.ceil(seg_counts[:, 0::2].max(axis=0) / P).astype(int)
    CHI = np.ceil(seg_counts[:, 1::2].max(axis=0) / P).astype(int)
    # chunk order: group-major; within group: all lo chunks (tile order), then hi
    seg_chunk_start = np.zeros(NSEG, dtype=np.int64)   # global chunk idx per seg
    grp_clo = np.zeros(NGROUPS, dtype=np.int64)
    grp_chi = np.zeros(NGROUPS, dtype=np.int64)
    gcb = np.zeros(NGROUPS + 1, dtype=np.int64)
    for g in range(NGROUPS):
        ts = range(g * GS, (g + 1) * GS)
        grp_clo[g] = sum(CLO[t] for t in ts)
        grp_chi[g] = sum(CHI[t] for t in ts)
        ofs = gcb[g]
        for t in ts:
            seg_chunk_start[2 * t] = ofs
            ofs += CLO[t]
        for t in ts:
            seg_chunk_start[2 * t + 1] = ofs
            ofs += CHI[t]
        gcb[g + 1] = ofs
    CTOT = int(gcb[-1])

    # per-chunk default fill (pads): lo chunks -> ZLO, hi chunks -> absolute hi zero
    chunk_is_hi = np.zeros(CTOT, dtype=bool)
    for t in range(TILES):
        s = seg_chunk_start[2 * t + 1]
        chunk_is_hi[s:s + CHI[t]] = True

    pl.cores = []
    for c in range(NCORES):
        d = per_core[c]
        npad = CTOT * P
        dstloc_pad = np.zeros(npad, dtype=np.float16)
        row_pad = np.where(np.repeat(chunk_is_hi, P), RAW - 1,
                           ZLO_ROW).astype(np.int64)
        # position of each real edge
        cnt = seg_counts[c]
        seg_first = np.concatenate([[0], np.cumsum(cnt)[:-1]])
        within = np.arange(len(d["seg"])) - seg_first[d["seg"]]
        pos = seg_chunk_start[d["seg"]] * P + within
        dstloc_pad[pos] = d["dstloc"].astype(np.float16)
        row_pad[pos] = d["esg"]

        # gather index arrays (lo then hi, group-major)
        lo_parts, hi_parts = [], []
        for g in range(NGROUPS):
            a = gcb[g] * P
            b = a + grp_clo[g] * P
            e = gcb[g + 1] * P
            lo_parts.append(row_pad[a:b])
            hi_parts.append(row_pad[b:e] - HIB)
        lo_flat = np.concatenate(lo_parts)
        hi_flat = np.concatenate(hi_parts)
        assert lo_flat.min() >= 0 and lo_flat.max() < HIB <= 32768
        assert hi_flat.min() >= 0 and hi_flat.max() <= RAW - 1 - HIB <= 32767

        core = {
            "idxlo": _wrap_idx(lo_flat),
            "idxhi": _wrap_idx(hi_flat),
            "dstloc": dstloc_pad.reshape(CTOT, P).T.copy(),
        }
        pl.cores.append(core)

    # group gather call metadata (columns into wrapped idx tensors)
    pl.lo_cols = int(grp_clo.sum() * P // 16)
    pl.hi_cols = int(grp_chi.sum() * P // 16)
    lo_c0 = np.concatenate([[0], np.cumsum(grp_clo * 8)])
    hi_c0 = np.concatenate([[0], np.cumsum(grp_chi * 8)])
    pl.groups = []
    for g in range(NGROUPS):
        tiles = []
        for t in range(g * GS, (g + 1) * GS):
            lo_local = int(seg_chunk_start[2 * t] - gcb[g])
            hi_local = int(seg_chunk_start[2 * t + 1] - gcb[g])
            tiles.append({
                "clo": int(CLO[t]), "chi": int(CHI[t]),
                "sp_lo": lo_local, "sp_hi": hi_local,
                "gc_lo": int(seg_chunk_start[2 * t]),
                "gc_hi": int(seg_chunk_start[2 * t + 1]),
            })
        pl.groups.append({
            "nclo": int(grp_clo[g]), "nchi": int(grp_chi[g]),
            "lo_col0": int(lo_c0[g]), "hi_col0": int(hi_c0[g]),
            "tiles": tiles,
        })
    pl.CTOT = CTOT
    pl.NCHMAX = int(max(CLO.max(), CHI.max()))

    # slot -> node map, batch values, dinv per slot, xT shards, table row map
    node_at = np.full((NCORES, SLOTS), -1, dtype=np.int64)
    node_at[core_of, slot_of] = np.arange(N)
    bv = np.full((NCORES, SLOTS), 99.0, dtype=np.float16)
    dv = np.zeros((NCORES, SLOTS), dtype=np.float32)   # pad slots: dinv = 0
    valid = node_at >= 0
    bv[valid] = batch[node_at[valid]].astype(np.float16)
    dv[valid] = dinv[node_at[valid]]
    for c in range(NCORES):
        pl.cores[c]["batchval"] = bv[c].reshape(TILES, P).T.copy()  # [128, 49]
        pl.cores[c]["dinv"] = dv[c].reshape(TILES, P).T.copy()      # [128, 49]
        xt = np.zeros((D_IN, SLOTS), dtype=np.float32)
        v = valid[c]
        xt[:, v] = np.asarray(x, dtype=np.float32)[node_at[c][v]].T
        pl.cores[c]["xT"] = xt.astype(BF16_NP)

    pl.rowmap = np.arange(RAW, dtype=np.int64).reshape(NCORES, SLOTS)
    pl.counts = np.bincount(batch, minlength=NGRAPH).astype(np.float32)
    pl.iota_rep = np.repeat(np.arange(P), pl.NCHMAX).astype(np.float16).reshape(1, -1)
    pl.giota = np.repeat(np.arange(NGRAPH), TILES).astype(np.float16).reshape(1, -1)
    pl.key = (tuple(CLO), tuple(CHI))
    return pl


# ---------------------------------------------------------- program builders --

def _make_gemm_emitter(nc, ctx, tc, k_tiles_fn, o_T, dinv_sb, bufs=2):
    """Returns emit(t): table rows for slot tile t.

    out[slot, fout] = sum_k lhsT_k^T @ rhs_k with lhsT = feat-major input
    block (no output transpose needed); dinv-scale + fp16 cast -> o_T rows.
    """
    gps_pool = ctx.enter_context(
        tc.tile_pool(name="gemm_ps", bufs=bufs, space="PSUM"))
    to_pool = ctx.enter_context(tc.tile_pool(name="gemm_to", bufs=2))
    WB = 4  # tiles per table-write DMA (amortizes the 625ns HWDGE slot)
    state = {}

    def emit(t):
        kt = k_tiles_fn(t)
        gps = gps_pool.tile([P, H], F32, space="PSUM")
        for ki, (lhsT, rhs) in enumerate(kt):
            nc.tensor.matmul(out=gps[:], lhsT=lhsT, rhs=rhs,
                             start=(ki == 0), stop=(ki == len(kt) - 1))
        j = t % WB
        if j == 0:
            to_new = to_pool.tile([P, WB, H], F16, tag="to")
            state["to"] = to_new
        to = state["to"]
        # pad slots have dinv == 0, so this scale also keeps their table rows
        # ZERO (they serve as the gather targets for chunk padding positions)
        nc.scalar.activation(out=to[:, j, :], in_=gps[:],
                             func=mybir.ActivationFunctionType.Copy,
                             scale=dinv_sb[:, t:t + 1])
        if j == WB - 1 or t == TILES - 1:
            t0, n = t - j, j + 1
            dst = o_T[t0 * P:(t0 + n) * P, :].rearrange(
                "(j p) h -> p j h", j=n, p=P)
            nc.sync.dma_start(out=dst, in_=to[:, 0:n, :])

    return emit


def _build_A(pl):
    nc = bacc.Bacc("TRN2", target_bir_lowering=False, debug=False, num_devices=NCORES)
    i_xT = nc.dram_tensor("xT", [D_IN, SLOTS], BF16, kind="ExternalInput").ap()
    i_W = nc.dram_tensor("W", [D_IN, H], BF16, kind="ExternalInput").ap()
    i_dinv = nc.dram_tensor("dinv", [P, TILES], F32, kind="ExternalInput").ap()
    o_T = nc.dram_tensor("Tout", [SLOTS, H], F16, kind="ExternalOutput").ap()
    with tile.TileContext(nc) as tc:
        with ExitStack() as ctx:
            const = ctx.enter_context(tc.tile_pool(name="const", bufs=1))
            dinv_sb = const.tile([P, TILES], F32)
            nc.sync.dma_start(out=dinv_sb[:], in_=i_dinv[:])
            w0 = const.tile([P, H], BF16)
            nc.sync.dma_start(out=w0[:], in_=i_W[0:P, :])
            w1 = const.tile([P, H], BF16)
            nc.sync.dma_start(out=w1[:], in_=i_W[P:2 * P, :])
            x0 = const.tile([P, SLOTS], BF16)
            x1 = const.tile([P, SLOTS], BF16)
            HLF = SLOTS // 2
            nc.sync.dma_start(out=x0[:, 0:HLF], in_=i_xT[0:P, 0:HLF])
            nc.sync.dma_start(out=x1[:, 0:HLF], in_=i_xT[P:2 * P, 0:HLF])
            nc.sync.dma_start(out=x0[:, HLF:], in_=i_xT[0:P, HLF:])
            nc.sync.dma_start(out=x1[:, HLF:], in_=i_xT[P:2 * P, HLF:])

            def k_tiles(t):
                sl = slice(t * P, (t + 1) * P)
                return [(x0[:, sl], w0[:]), (x1[:, sl], w1[:])]

            emit = _make_gemm_emitter(nc, ctx, tc, k_tiles, o_T, dinv_sb, bufs=3)
            for t in range(TILES):
                emit(t)
    nc.compile()
    return nc


def _scatter_body(nc, ctx, tc, pl, i_T, consume_tile, after_tile=None):
    """Shared gather + one-hot matmul scatter loop.

    consume_tile(t, ypsum) handles the per-tile PSUM result
    (ypsum = sum over in-edges of dinv[src]-scaled source rows, incl self-loop).
    """
    const = ctx.enter_context(tc.tile_pool(name="sc_const", bufs=1))
    stage = ctx.enter_context(tc.tile_pool(name="staging", bufs=2))
    st_pool = ctx.enter_context(tc.tile_pool(name="st", bufs=4))
    yp_pool = ctx.enter_context(tc.tile_pool(name="yps", bufs=3, space="PSUM"))

    i_idxlo = nc.dram_tensor("idxlo", [P, pl.lo_cols], I16, kind="ExternalInput").ap()
    i_idxhi = nc.dram_tensor("idxhi", [P, pl.hi_cols], I16, kind="ExternalInput").ap()
    i_dstloc = nc.dram_tensor("dstloc", [P, pl.CTOT], F16, kind="ExternalInput").ap()
    i_iota = nc.dram_tensor("iota_rep", [1, P * pl.NCHMAX], F16,
                            kind="ExternalInput").ap()
    i_ownT = nc.dram_tensor("ownT", [H, SLOTS], F16, kind="ExternalInput").ap()

    idxlo_sb = const.tile([P, pl.lo_cols], I16)
    nc.sync.dma_start(out=idxlo_sb[:], in_=i_idxlo[:])
    idxhi_sb = const.tile([P, pl.hi_cols], I16)
    nc.sync.dma_start(out=idxhi_sb[:], in_=i_idxhi[:])
    dstloc_sb = const.tile([P, pl.CTOT], F16)
    nc.sync.dma_start(out=dstloc_sb[:], in_=i_dstloc[:])
    iota_sb = const.tile([P, P * pl.NCHMAX], F16)
    nc.sync.dma_start(out=iota_sb[:], in_=i_iota.to_broadcast([P, P * pl.NCHMAX]))
    iota3 = iota_sb[:].rearrange("p (j c) -> p j c", j=P, c=pl.NCHMAX)
    identH = const.tile([P, P], F16)
    make_identity(nc, identH[:])
    ownT_sb = const.tile([P, SLOTS], F16)
    nc.sync.dma_start(out=ownT_sb[:], in_=i_ownT[:])

    qn = [0]

    def gather(staging, base, src_ap, idx_sb, col0, nch):
        for o in range(0, nch, MAXCH):
            n = min(MAXCH, nch - o)
            c0 = col0 + o * 8
            nc.gpsimd.dma_gather(
                out_ap=staging[:, base + o:base + o + n, :], in_ap=src_ap,
                idxs_ap=idx_sb[:, c0:c0 + n * 8],
                num_idxs=n * P, num_idxs_reg=n * P, elem_size=H,
                queue_num=qn[0])
            qn[0] = (qn[0] + 1) % NQ

    def onehot(gc0, nch):
        st = st_pool.tile([P, P, nch], F16, tag="st")
        nc.vector.tensor_tensor(
            out=st[:],
            in0=iota3[:, :, 0:nch],
            in1=dstloc_sb[:, gc0:gc0 + nch].unsqueeze(1).to_broadcast([P, P, nch]),
            op=mybir.AluOpType.is_equal)
        return st

    for g, grp in enumerate(pl.groups):
        nclo, nchi = grp["nclo"], grp["nchi"]
        staging = stage.tile([P, nclo + nchi, H], F16, tag="staging")
        gather(staging, 0, i_T[:], idxlo_sb, grp["lo_col0"], nclo)
        gather(staging, nclo, i_T[HIB:, :], idxhi_sb, grp["hi_col0"], nchi)
        for ti, td in enumerate(grp["tiles"]):
            t = g * GS + ti
            stlo = onehot(td["gc_lo"], td["clo"]) if td["clo"] else None
            sthi = onehot(td["gc_hi"], td["chi"]) if td["chi"] else None
            ypsum = yp_pool.tile([P, H], F32, space="PSUM")
            # self-loop rows: ypsum = ownT_tile^T @ I  (= own rows, [slot, feat])
            nc.tensor.matmul(out=ypsum[:], lhsT=ownT_sb[:, t * P:(t + 1) * P],
                             rhs=identH[:],
                             start=True, stop=(td["clo"] + td["chi"] == 0))
            for i in range(td["clo"]):
                nc.tensor.matmul(
                    out=ypsum[:], lhsT=stlo[:, :, i],
                    rhs=staging[:, td["sp_lo"] + i, :],
                    start=False,
                    stop=(i == td["clo"] - 1 and td["chi"] == 0))
            for i in range(td["chi"]):
                nc.tensor.matmul(
                    out=ypsum[:], lhsT=sthi[:, :, i],
                    rhs=staging[:, td["sp_hi"] + i, :],
                    start=False, stop=(i == td["chi"] - 1))
            consume_tile(t, ypsum)
            if after_tile is not None:
                after_tile(t)


def _vec_input(nc, const, name):
    ap = nc.dram_tensor(name, [H, 1], F32, kind="ExternalInput").ap()
    sb = const.tile([H, 1], F32, tag=f"vec_{name}")
    nc.sync.dma_start(out=sb[:], in_=ap[:])
    return sb


def _build_BC(pl):
    nc = bacc.Bacc("TRN2", target_bir_lowering=False, debug=False,
                   num_devices=NCORES, dynamic_dma_scratch_size=SCRATCH,
                   num_swdge_queues=NQ)
    i_T = nc.dram_tensor("T", [TAB, H], F16, kind="ExternalInput").ap()
    i_W = nc.dram_tensor("W", [H, H], BF16, kind="ExternalInput").ap()
    i_dinv = nc.dram_tensor("dinv", [P, TILES], F32, kind="ExternalInput").ap()
    o_T = nc.dram_tensor("Tout", [SLOTS, H], F16, kind="ExternalOutput").ap()
    with tile.TileContext(nc) as tc:
        with ExitStack() as ctx:
            const = ctx.enter_context(tc.tile_pool(name="bc_const", bufs=1))
            ycp_pool = ctx.enter_context(tc.tile_pool(name="ycp", bufs=3))
            h_pool = ctx.enter_context(tc.tile_pool(name="ht", bufs=3))
            tps_pool = ctx.enter_context(tc.tile_pool(name="tps", bufs=2, space="PSUM"))

            b_sb = _vec_input(nc, const, "bvec")
            g_sb = _vec_input(nc, const, "bn_g")
            bb_sb = _vec_input(nc, const, "bn_b")
            m_sb = _vec_input(nc, const, "bn_m")
            v_sb = _vec_input(nc, const, "bn_v")
            # scale = g / sqrt(v+eps); bias = (b - m)*scale + beta
            eps = const.tile([H, 1], F32)
            nc.vector.memset(eps[:], BN_EPS)
            sq = const.tile([H, 1], F32)
            nc.scalar.activation(out=sq[:], in_=v_sb[:],
                                 func=mybir.ActivationFunctionType.Sqrt,
                                 bias=eps[:], scale=1.0)
            rs = const.tile([H, 1], F32)
            nc.vector.reciprocal(out=rs[:], in_=sq[:])
            scale = const.tile([H, 1], F32)
            nc.vector.tensor_mul(out=scale[:], in0=rs[:], in1=g_sb[:])
            bias = const.tile([H, 1], F32)
            nc.vector.tensor_sub(out=bias[:], in0=b_sb[:], in1=m_sb[:])
            nc.vector.tensor_mul(out=bias[:], in0=bias[:], in1=scale[:])
            nc.vector.tensor_add(out=bias[:], in0=bias[:], in1=bb_sb[:])

            identB = const.tile([P, P], BF16)
            make_identity(nc, identB[:])
            dinv_sb = const.tile([P, TILES], F32)
            nc.sync.dma_start(out=dinv_sb[:], in_=i_dinv[:])
            w_sb = const.tile([H, H], BF16)
            nc.sync.dma_start(out=w_sb[:], in_=i_W[:])

            h_tiles = {}

            def consume(t, ypsum):
                ycp = ycp_pool.tile([P, H], BF16)
                nc.scalar.activation(out=ycp[:], in_=ypsum[:],
                                     func=mybir.ActivationFunctionType.Copy,
                                     scale=dinv_sb[:, t:t + 1])
                tp = tps_pool.tile([P, P], BF16, space="PSUM")
                nc.tensor.transpose(out=tp[:], in_=ycp[:], identity=identB[:])
                h_t = h_pool.tile([P, H], BF16)
                nc.scalar.activation(
                    out=h_t[:], in_=tp[:],
                    func=mybir.ActivationFunctionType.Relu,
                    bias=bias[:], scale=scale[:])
                h_tiles[t] = h_t

            emit = _make_gemm_emitter(nc, ctx, tc,
                                      lambda t: [(h_tiles.pop(t)[:], w_sb[:])],
                                      o_T, dinv_sb)

            # emit each tile's GEMM right after its scatter completes so the
            # table write overlaps the remaining scatter instead of tailing it
            _scatter_body(nc, ctx, tc, pl, i_T, consume, emit)
    nc.compile()
    return nc


def _build_D(pl):
    nc = bacc.Bacc("TRN2", target_bir_lowering=False, debug=False,
                   num_devices=NCORES, dynamic_dma_scratch_size=SCRATCH,
                   num_swdge_queues=NQ)
    i_T = nc.dram_tensor("T", [TAB, H], F16, kind="ExternalInput").ap()
    i_bv = nc.dram_tensor("batchval", [P, TILES], F16, kind="ExternalInput").ap()
    i_gi = nc.dram_tensor("giota", [1, NGRAPH * TILES], F16,
                          kind="ExternalInput").ap()
    i_dinv = nc.dram_tensor("dinv", [P, TILES], F32, kind="ExternalInput").ap()
    o_pool = nc.dram_tensor("pool", [NGRAPH, H], F32, kind="ExternalOutput").ap()
    with tile.TileContext(nc) as tc:
        with ExitStack() as ctx:
            const = ctx.enter_context(tc.tile_pool(name="d_const", bufs=1))
            h3_pool = ctx.enter_context(tc.tile_pool(name="h3", bufs=3))
            pp_pool = ctx.enter_context(tc.tile_pool(name="pp", bufs=1, space="PSUM"))

            bv_sb = const.tile([P, TILES], F16)
            nc.sync.dma_start(out=bv_sb[:], in_=i_bv[:])
            gi_sb = const.tile([P, NGRAPH * TILES], F16)
            nc.sync.dma_start(out=gi_sb[:], in_=i_gi.to_broadcast([P, NGRAPH * TILES]))
            dinv_sb = const.tile([P, TILES], F32)
            nc.sync.dma_start(out=dinv_sb[:], in_=i_dinv[:])
            # oh_all[p, g, t] = (batchval[p, t] == g)
            oh_all = const.tile([P, NGRAPH, TILES], F16)
            nc.vector.tensor_tensor(
                out=oh_all[:],
                in0=gi_sb[:].rearrange("p (g t) -> p g t", g=NGRAPH, t=TILES),
                in1=bv_sb[:].unsqueeze(1).to_broadcast([P, NGRAPH, TILES]),
                op=mybir.AluOpType.is_equal)
            pp = pp_pool.tile([NGRAPH, H], F32, space="PSUM")

            def consume(t, ypsum):
                h3 = h3_pool.tile([P, H], F16)
                nc.scalar.activation(out=h3[:], in_=ypsum[:],
                                     func=mybir.ActivationFunctionType.Copy,
                                     scale=dinv_sb[:, t:t + 1])
                nc.tensor.matmul(out=pp[:], lhsT=oh_all[:, :, t], rhs=h3[:],
                                 start=(t == 0), stop=(t == TILES - 1))

            _scatter_body(nc, ctx, tc, pl, i_T, consume)
            pcp = const.tile([NGRAPH, H], F32)
            nc.vector.tensor_copy(out=pcp[:], in_=pp[:])
            nc.sync.dma_start(out=o_pool[:], in_=pcp[:])
    nc.compile()
    return nc


# ------------------------------------------------------------------- driver --

def _run(nc, in_maps):
    res = run_bass_kernel_spmd(nc, in_maps, core_ids=list(range(NCORES)),
                               trace=TRACE)
    if TRACE:
        LAST_EXEC_NS.append(res.exec_time_ns)
    return res.results


def _assemble_table(pl, shards):
    T = np.zeros((TAB, H), dtype=np.float16)
    for c in range(NCORES):
        T[pl.rowmap[c]] = shards[c]
    return T


def kernel(**inputs):
    ins = {k: np.asarray(v) for k, v in inputs.items()}
    key = hashlib.sha1(
        ins["edge_index"].tobytes() + ins["batch"].tobytes()
    ).hexdigest()
    if key not in _PLAN_CACHE:
        _PLAN_CACHE[key] = _make_plan(ins["edge_index"], ins["batch"], ins["x"])
    pl = _PLAN_CACHE[key]

    pk = pl.key
    if pk not in _PROG_CACHE:
        _PROG_CACHE[pk] = {
            "A": _build_A(pl),
            "BC": _build_BC(pl),
            "D": _build_D(pl),
        }
    progs = _PROG_CACHE[pk]

    LAST_EXEC_NS.clear()
    W1 = ins["W1"].astype(BF16_NP)
    # Launch A: T1 = dinv * (x @ W1)
    resA = _run(progs["A"], [
        {"xT": pl.cores[c]["xT"], "W": W1, "dinv": pl.cores[c]["dinv"]}
        for c in range(NCORES)
    ])
    shardsA = [r["Tout"] for r in resA]
    T1 = _assemble_table(pl, shardsA)

    def meta(c):
        cc = pl.cores[c]
        return {"idxlo": cc["idxlo"], "idxhi": cc["idxhi"],
                "dstloc": cc["dstloc"], "iota_rep": pl.iota_rep,
                "dinv": cc["dinv"]}

    def ownT(shard):
        return np.ascontiguousarray(shard.T)

    def vec(name):
        return ins[name].astype(np.float32).reshape(H, 1)

    # Launch B: layer-1 scatter + BN1/ReLU + @W2
    resB = _run(progs["BC"], [
        {**meta(c), "T": T1, "ownT": ownT(shardsA[c]), "W": ins["W2"].astype(BF16_NP),
         "bvec": vec("b1"), "bn_g": vec("bn1_g"), "bn_b": vec("bn1_b"),
         "bn_m": vec("bn1_m"), "bn_v": vec("bn1_v")} for c in range(NCORES)
    ])
    shardsB = [r["Tout"] for r in resB]
    T2 = _assemble_table(pl, shardsB)

    # Launch C: layer-2 scatter + BN2/ReLU + @W3
    resC = _run(progs["BC"], [
        {**meta(c), "T": T2, "ownT": ownT(shardsB[c]), "W": ins["W3"].astype(BF16_NP),
         "bvec": vec("b2"), "bn_g": vec("bn2_g"), "bn_b": vec("bn2_b"),
         "bn_m": vec("bn2_m"), "bn_v": vec("bn2_v")} for c in range(NCORES)
    ])
    shardsC = [r["Tout"] for r in resC]
    T3 = _assemble_table(pl, shardsC)

    # Launch D: layer-3 scatter + pooling partials
    resD = _run(progs["D"], [
        {**meta(c), "T": T3, "ownT": ownT(shardsC[c]),
         "batchval": pl.cores[c]["batchval"], "giota": pl.giota}
        for c in range(NCORES)
    ])
    pooled_sum = np.sum([r["pool"] for r in resD], axis=0).astype(np.float64)

    counts = pl.counts.astype(np.float64)
    pooled_sum += counts[:, None] * ins["b3"].astype(np.float64)[None, :]
    pooled = pooled_sum / np.maximum(counts, 1.0)[:, None]

    z = np.maximum(pooled @ ins["Wc1"].astype(np.float64)
                   + ins["bc1"].astype(np.float64), 0.0)
    out = z @ ins["Wc2"].astype(np.float64) + ins["bc2"].astype(np.float64)
    return out.astype(np.float32)


# revision 45
# speedup vs baseline: 1.6218x; 1.0372x over previous
"""Trainium2 Bass kernel for DocumentClassificationGNN (3-layer GCN + BN/ReLU +
global mean pool + MLP head), distributed over 8 NeuronCores.

Strategy (node/graph parallel, per the sharding hint):
  - Nodes are assigned to (core, slot) sorted by in-degree so every core/tile
    carries a balanced edge load.  Edges are partitioned by DESTINATION core so
    the segment-sum scatter is device-local.
  - Per layer: a dense GEMM produces a node-major fp16 feature table that the
    host replicates to all cores ("all-gather" through the host between
    launches); each core gathers its in-edge source rows with dma_gather and
    scatter-adds them into PSUM with one-hot matmuls.
  - The symmetric norm deg^-1/2[src]*deg^-1/2[dst] is SEPARABLE: table rows
    are pre-scaled by dinv[src] at write time and the scatter output is
    post-scaled by dinv[dst], so the one-hot matrices are pure 0/1 and are
    generated in batched DVE is_equal ops (2-byte fast path) with the chunk
    dim innermost: s_t[p, j, c].
  - Self-loops never enter the edge stream: each tile's own table rows are
    bulk-loaded and added via one identity matmul (contribution dinv_d*T'[d]).
  - conv bias + BN + ReLU fuse into one scalar-engine activation; GEMMs run in
    bf16; launch D does per-tile onehot(batch) pooling accumulated in one PSUM
    bank.
  - Device output: per-core pooled partial sums [64, 128].  Host: sum, +n_g*b3,
    divide by counts, tiny classifier MLP.

Programs (3 compiles, 4 launches):
  A : T1 = dinv * (x @ W1)                          -> T1 table shard
  BC: Y = scatter(T); h' = relu(BN(dinv*Y + b)); T' = dinv * (h' @ W_next)
  D : Y3 = scatter(T3); pooled_partial = onehot(batch)^T @ (dinv*Y3)
"""

import hashlib
import numpy as np
from contextlib import ExitStack

import ml_dtypes

import concourse.bass as bass
import concourse.bacc as bacc
import concourse.tile as tile
from concourse import mybir
from concourse.bass_utils import run_bass_kernel_spmd
from concourse.masks import make_identity

P = 128
NCORES = 8
N = 50000
D_IN = 256
H = 128
NGRAPH = 64
SLOTS = 6272            # 49 tiles of 128 slots per core (6250 real nodes + pad)
TILES = SLOTS // P      # 49
RAW = NCORES * SLOTS    # 50176
TAB = RAW               # table = concatenated shards, no extra rows
HIB = 4 * SLOTS         # hi-region gather base: cores 0-3 lo, cores 4-7 hi
                        # (both index ranges fit the int16 dma_gather indices)
ZLO_ROW = SLOTS - 1     # core-0 pad slot: always-zero row used by lo pads
# dst tiles per gather group: small first groups so the first tiles'
# staging lands early and the PE/consume pipeline starts ~15us sooner
GROUP_SIZES = [2, 5, 7, 7, 7, 7, 7, 7]
assert sum(GROUP_SIZES) == TILES
NGROUPS = len(GROUP_SIZES)
GROUP_T0 = [sum(GROUP_SIZES[:g]) for g in range(NGROUPS)]
BN_EPS = 1e-5

SCRATCH = 16384         # SWDGE ring: 16384/16 = 1024 descriptors per queue
MAXCH = 8               # chunks per dma_gather call (8*128 = 1024, HW limit)
NQ = 2                  # SWDGE queues (desc-gen pipelines against transfer)

F16 = mybir.dt.float16
BF16 = mybir.dt.bfloat16
F32 = mybir.dt.float32
I16 = mybir.dt.int16
BF16_NP = ml_dtypes.bfloat16

# module-level knobs / perf results (test.py pokes these)
TRACE = False
LAST_EXEC_NS = []       # per-launch exec_time_ns (when TRACE)

_PLAN_CACHE = {}
_PROG_CACHE = {}


# ---------------------------------------------------------------- host prep --

def _wrap_idx(flat):
    """dma_gather index layout: idx i -> [i%16, i//16], replicated to 128 parts."""
    n = len(flat)
    assert n % 16 == 0
    arr = np.asarray(flat, dtype=np.int16).reshape(n // 16, 16).T.copy()
    return np.tile(arr, (8, 1))


class _Plan:
    pass


def _distribute(total, bins):
    base, extra = divmod(int(total), bins)
    out = np.full(bins, base, dtype=np.int64)
    out[:extra] += 1
    return out


def _pack_core(lo, hi, kL, kH):
    """Pack one core's nodes into TILES tiles of <=128 slots, steering the
    per-tile lo/hi in-edge sums toward the shared chunk budgets kL/kH*128.

    Worst-fit decreasing on min remaining (lo, hi) headroom.
    """
    n = len(lo)
    loR = (kL * P).astype(np.float64)
    hiR = (kH * P).astype(np.float64)
    cap = np.full(TILES, P, dtype=np.int64)
    # all pad (empty) slots must be the LAST slots of the last tile: they are
    # the always-zero rows targeted by gather padding and the table-write memset
    cap[TILES - 1] = P - (TILES * P - n)
    filled = np.zeros(TILES, dtype=np.int64)
    slot = np.empty(n, dtype=np.int64)
    order = np.argsort(-(lo + hi), kind="stable")
    for i in order:
        score = np.minimum(loR - lo[i], hiR - hi[i])
        score[filled >= cap] = -np.inf
        t = int(np.argmax(score))
        loR[t] -= lo[i]
        hiR[t] -= hi[i]
        slot[i] = t * P + filled[t]
        filled[t] += 1
    return slot


def _make_plan(edge_index, batch, x):
    pl = _Plan()
    src = np.asarray(edge_index[0], dtype=np.int64)
    dst = np.asarray(edge_index[1], dtype=np.int64)
    batch = np.asarray(batch, dtype=np.int64)

    deg = np.bincount(dst, minlength=N).astype(np.int64) + 1
    dinv = (1.0 / np.sqrt(deg)).astype(np.float32)

    order = np.argsort(-deg, kind="stable")
    rank = np.empty(N, dtype=np.int64)
    rank[order] = np.arange(N)
    core_of = rank % NCORES

    # lo/hi membership of an edge depends only on its source CORE (the hi
    # gather base sits on the core-3/4 boundary), so per-node lo/hi in-edge
    # counts are fixed before slots are chosen -> bin-pack nodes into tiles
    # so per-(tile, half) counts land just under multiples of 128.
    islo_e = core_of[src] < NCORES // 2
    lo_n = np.bincount(dst[islo_e], minlength=N)
    hi_n = np.bincount(dst[~islo_e], minlength=N)
    totlo = np.zeros(NCORES, dtype=np.int64)
    tothi = np.zeros(NCORES, dtype=np.int64)
    for c in range(NCORES):
        m = core_of == c
        totlo[c] = lo_n[m].sum()
        tothi[c] = hi_n[m].sum()
    SLACK = 3
    kL = _distribute(-(-totlo.max() // P) + SLACK, TILES)
    kH = _distribute(-(-tothi.max() // P) + SLACK, TILES)
    slot_of = np.empty(N, dtype=np.int64)
    for c in range(NCORES):
        nodes = np.where(core_of == c)[0]
        slot_of[nodes] = _pack_core(lo_n[nodes], hi_n[nodes], kL, kH)
    raw_of = core_of * SLOTS + slot_of
    grow_of = raw_of                        # table row per node (pure concat)

    # real edges only: self-loops are handled by the per-tile identity matmul
    es, ed = src, dst
    ecore = core_of[ed]
    eslot = slot_of[ed]
    etile = eslot // P
    edstloc = eslot % P
    esg = grow_of[es]
    islo = islo_e

    # per-core sorted segment arrays
    NSEG = TILES * 2   # segment id: 2*tile + (0 lo / 1 hi)
    per_core = []
    seg_counts = np.zeros((NCORES, NSEG), dtype=np.int64)
    for c in range(NCORES):
        m = ecore == c
        seg = etile[m] * 2 + (~islo[m]).astype(np.int64)
        o2 = np.lexsort((esg[m], seg))
        d = {
            "seg": seg[o2],
            "dstloc": edstloc[m][o2],
            "esg": esg[m][o2],
        }
        seg_counts[c] = np.bincount(d["seg"], minlength=NSEG)
        per_core.append(d)

    # chunk plan: per tile, lo/hi chunk counts = max over cores
    CLO = np.ceil(seg_counts[:, 0::2].max(axis=0) / P).astype(int)
    CHI = np.ceil(seg_counts[:, 1::2].max(axis=0) / P).astype(int)
    # chunk order: group-major; within group: all lo chunks (tile order), then hi
    seg_chunk_start = np.zeros(NSEG, dtype=np.int64)   # global chunk idx per seg
    grp_clo = np.zeros(NGROUPS, dtype=np.int64)
    grp_chi = np.zeros(NGROUPS, dtype=np.int64)
    gcb = np.zeros(NGROUPS + 1, dtype=np.int64)
    for g in range(NGROUPS):
        ts = range(GROUP_T0[g], GROUP_T0[g] + GROUP_SIZES[g])
        grp_clo[g] = sum(CLO[t] for t in ts)
        grp_chi[g] = sum(CHI[t] for t in ts)
        ofs = gcb[g]
        for t in ts:
            seg_chunk_start[2 * t] = ofs
            ofs += CLO[t]
        for t in ts:
            seg_chunk_start[2 * t + 1] = ofs
            ofs += CHI[t]
        gcb[g + 1] = ofs
    CTOT = int(gcb[-1])

    # per-chunk default fill (pads): lo chunks -> ZLO, hi chunks -> absolute hi zero
    chunk_is_hi = np.zeros(CTOT, dtype=bool)
    for t in range(TILES):
        s = seg_chunk_start[2 * t + 1]
        chunk_is_hi[s:s + CHI[t]] = True

    pl.cores = []
    for c in range(NCORES):
        d = per_core[c]
        npad = CTOT * P
        dstloc_pad = np.zeros(npad, dtype=np.float16)
        row_pad = np.where(np.repeat(chunk_is_hi, P), RAW - 1,
                           ZLO_ROW).astype(np.int64)
        # position of each real edge
        cnt = seg_counts[c]
        seg_first = np.concatenate([[0], np.cumsum(cnt)[:-1]])
        within = np.arange(len(d["seg"])) - seg_first[d["seg"]]
        pos = seg_chunk_start[d["seg"]] * P + within
        dstloc_pad[pos] = d["dstloc"].astype(np.float16)
        row_pad[pos] = d["esg"]

        # gather index arrays (lo then hi, group-major)
        lo_parts, hi_parts = [], []
        for g in range(NGROUPS):
            a = gcb[g] * P
            b = a + grp_clo[g] * P
            e = gcb[g + 1] * P
            lo_parts.append(row_pad[a:b])
            hi_parts.append(row_pad[b:e] - HIB)
        lo_flat = np.concatenate(lo_parts)
        hi_flat = np.concatenate(hi_parts)
        assert lo_flat.min() >= 0 and lo_flat.max() < HIB <= 32768
        assert hi_flat.min() >= 0 and hi_flat.max() <= RAW - 1 - HIB <= 32767

        core = {
            "idxlo": _wrap_idx(lo_flat),
            "idxhi": _wrap_idx(hi_flat),
            "dstloc": dstloc_pad.reshape(CTOT, P).T.copy(),
        }
        pl.cores.append(core)

    # group gather call metadata (columns into wrapped idx tensors)
    pl.lo_cols = int(grp_clo.sum() * P // 16)
    pl.hi_cols = int(grp_chi.sum() * P // 16)
    lo_c0 = np.concatenate([[0], np.cumsum(grp_clo * 8)])
    hi_c0 = np.concatenate([[0], np.cumsum(grp_chi * 8)])
    pl.groups = []
    for g in range(NGROUPS):
        tiles = []
        for t in range(GROUP_T0[g], GROUP_T0[g] + GROUP_SIZES[g]):
            lo_local = int(seg_chunk_start[2 * t] - gcb[g])
            hi_local = int(seg_chunk_start[2 * t + 1] - gcb[g])
            tiles.append({
                "clo": int(CLO[t]), "chi": int(CHI[t]),
                "sp_lo": lo_local, "sp_hi": hi_local,
                "gc_lo": int(seg_chunk_start[2 * t]),
                "gc_hi": int(seg_chunk_start[2 * t + 1]),
            })
        pl.groups.append({
            "nclo": int(grp_clo[g]), "nchi": int(grp_chi[g]),
            "lo_col0": int(lo_c0[g]), "hi_col0": int(hi_c0[g]),
            "tiles": tiles,
        })
    pl.CTOT = CTOT
    pl.NCHMAX = int(max(CLO.max(), CHI.max()))

    # slot -> node map, batch values, dinv per slot, xT shards, table row map
    node_at = np.full((NCORES, SLOTS), -1, dtype=np.int64)
    node_at[core_of, slot_of] = np.arange(N)
    bv = np.full((NCORES, SLOTS), 99.0, dtype=np.float16)
    dv = np.zeros((NCORES, SLOTS), dtype=np.float32)   # pad slots: dinv = 0
    valid = node_at >= 0
    bv[valid] = batch[node_at[valid]].astype(np.float16)
    dv[valid] = dinv[node_at[valid]]
    for c in range(NCORES):
        pl.cores[c]["batchval"] = bv[c].reshape(TILES, P).T.copy()  # [128, 49]
        pl.cores[c]["dinv"] = dv[c].reshape(TILES, P).T.copy()      # [128, 49]
        xt = np.zeros((D_IN, SLOTS), dtype=np.float32)
        v = valid[c]
        xt[:, v] = np.asarray(x, dtype=np.float32)[node_at[c][v]].T
        pl.cores[c]["xT"] = xt.astype(BF16_NP)

    pl.rowmap = np.arange(RAW, dtype=np.int64).reshape(NCORES, SLOTS)
    pl.counts = np.bincount(batch, minlength=NGRAPH).astype(np.float32)
    pl.iota_rep = np.repeat(np.arange(P), pl.NCHMAX).astype(np.float16).reshape(1, -1)
    pl.giota = np.repeat(np.arange(NGRAPH), TILES).astype(np.float16).reshape(1, -1)
    pl.key = (tuple(CLO), tuple(CHI))
    return pl


# ---------------------------------------------------------- program builders --

def _make_gemm_emitter(nc, ctx, tc, k_tiles_fn, o_T, dinv_sb, bufs=2,
                       to_bufs=None):
    """Returns emit(t): table rows for slot tile t.

    out[slot, fout] = sum_k lhsT_k^T @ rhs_k with lhsT = feat-major input
    block (no output transpose needed); dinv-scale + fp16 cast -> o_T rows.
    """
    gps_pool = ctx.enter_context(
        tc.tile_pool(name="gemm_ps", bufs=bufs, space="PSUM"))
    to_pool = ctx.enter_context(
        tc.tile_pool(name="gemm_to", bufs=to_bufs or 2))
    WB = 4  # tiles per table-write DMA (amortizes the 625ns HWDGE slot)
    state = {}

    def emit(t):
        kt = k_tiles_fn(t)
        gps = gps_pool.tile([P, H], F32, space="PSUM")
        for ki, (lhsT, rhs) in enumerate(kt):
            nc.tensor.matmul(out=gps[:], lhsT=lhsT, rhs=rhs,
                             start=(ki == 0), stop=(ki == len(kt) - 1))
        j = t % WB
        if j == 0:
            to_new = to_pool.tile([P, WB, H], F16, tag="to")
            state["to"] = to_new
        to = state["to"]
        # pad slots have dinv == 0, so this scale also keeps their table rows
        # ZERO (they serve as the gather targets for chunk padding positions)
        nc.scalar.activation(out=to[:, j, :], in_=gps[:],
                             func=mybir.ActivationFunctionType.Copy,
                             scale=dinv_sb[:, t:t + 1])
        if j == WB - 1 or t == TILES - 1:
            t0, n = t - j, j + 1
            dst = o_T[t0 * P:(t0 + n) * P, :].rearrange(
                "(j p) h -> p j h", j=n, p=P)
            nc.sync.dma_start(out=dst, in_=to[:, 0:n, :])

    return emit


def _build_A(pl):
    nc = bacc.Bacc("TRN2", target_bir_lowering=False, debug=False, num_devices=NCORES)
    i_xT = nc.dram_tensor("xT", [D_IN, SLOTS], BF16, kind="ExternalInput").ap()
    i_W = nc.dram_tensor("W", [D_IN, H], BF16, kind="ExternalInput").ap()
    i_dinv = nc.dram_tensor("dinv", [P, TILES], F32, kind="ExternalInput").ap()
    o_T = nc.dram_tensor("Tout", [SLOTS, H], F16, kind="ExternalOutput").ap()
    with tile.TileContext(nc) as tc:
        with ExitStack() as ctx:
            const = ctx.enter_context(tc.tile_pool(name="const", bufs=1))
            dinv_sb = const.tile([P, TILES], F32)
            nc.sync.dma_start(out=dinv_sb[:], in_=i_dinv[:])
            w0 = const.tile([P, H], BF16)
            nc.sync.dma_start(out=w0[:], in_=i_W[0:P, :])
            w1 = const.tile([P, H], BF16)
            nc.sync.dma_start(out=w1[:], in_=i_W[P:2 * P, :])
            x0 = const.tile([P, SLOTS], BF16)
            x1 = const.tile([P, SLOTS], BF16)
            HLF = SLOTS // 2
            nc.sync.dma_start(out=x0[:, 0:HLF], in_=i_xT[0:P, 0:HLF])
            nc.sync.dma_start(out=x1[:, 0:HLF], in_=i_xT[P:2 * P, 0:HLF])
            nc.sync.dma_start(out=x0[:, HLF:], in_=i_xT[0:P, HLF:])
            nc.sync.dma_start(out=x1[:, HLF:], in_=i_xT[P:2 * P, HLF:])

            def k_tiles(t):
                sl = slice(t * P, (t + 1) * P)
                return [(x0[:, sl], w0[:]), (x1[:, sl], w1[:])]

            emit = _make_gemm_emitter(nc, ctx, tc, k_tiles, o_T, dinv_sb, bufs=6,
                                      to_bufs=13)
            for t in range(TILES):
                emit(t)
    nc.compile()
    return nc


def _scatter_body(nc, ctx, tc, pl, i_T, consume_tile, after_tile=None):
    """Shared gather + one-hot matmul scatter loop.

    consume_tile(t, ypsum) handles the per-tile PSUM result
    (ypsum = sum over in-edges of dinv[src]-scaled source rows, incl self-loop).
    """
    const = ctx.enter_context(tc.tile_pool(name="sc_const", bufs=1))
    stage = ctx.enter_context(tc.tile_pool(name="staging", bufs=2))
    st_pool = ctx.enter_context(tc.tile_pool(name="st", bufs=4))
    yp_pool = ctx.enter_context(tc.tile_pool(name="yps", bufs=3, space="PSUM"))

    i_idxlo = nc.dram_tensor("idxlo", [P, pl.lo_cols], I16, kind="ExternalInput").ap()
    i_idxhi = nc.dram_tensor("idxhi", [P, pl.hi_cols], I16, kind="ExternalInput").ap()
    i_dstloc = nc.dram_tensor("dstloc", [P, pl.CTOT], F16, kind="ExternalInput").ap()
    i_iota = nc.dram_tensor("iota_rep", [1, P * pl.NCHMAX], F16,
                            kind="ExternalInput").ap()
    i_ownT = nc.dram_tensor("ownT", [H, SLOTS], F16, kind="ExternalInput").ap()

    idxlo_sb = const.tile([P, pl.lo_cols], I16)
    nc.sync.dma_start(out=idxlo_sb[:], in_=i_idxlo[:])
    idxhi_sb = const.tile([P, pl.hi_cols], I16)
    nc.sync.dma_start(out=idxhi_sb[:], in_=i_idxhi[:])
    dstloc_sb = const.tile([P, pl.CTOT], F16)
    nc.sync.dma_start(out=dstloc_sb[:], in_=i_dstloc[:])
    iota_sb = const.tile([P, P * pl.NCHMAX], F16)
    nc.sync.dma_start(out=iota_sb[:], in_=i_iota.to_broadcast([P, P * pl.NCHMAX]))
    iota3 = iota_sb[:].rearrange("p (j c) -> p j c", j=P, c=pl.NCHMAX)
    identH = const.tile([P, P], F16)
    make_identity(nc, identH[:])
    ownT_sb = const.tile([P, SLOTS], F16)
    nc.sync.dma_start(out=ownT_sb[:], in_=i_ownT[:])

    qn = [0]

    def gather(staging, base, src_ap, idx_sb, col0, nch):
        for o in range(0, nch, MAXCH):
            n = min(MAXCH, nch - o)
            c0 = col0 + o * 8
            nc.gpsimd.dma_gather(
                out_ap=staging[:, base + o:base + o + n, :], in_ap=src_ap,
                idxs_ap=idx_sb[:, c0:c0 + n * 8],
                num_idxs=n * P, num_idxs_reg=n * P, elem_size=H,
                queue_num=qn[0])
            qn[0] = (qn[0] + 1) % NQ

    def onehot(gc0, nch):
        st = st_pool.tile([P, P, nch], F16, tag="st")
        nc.vector.tensor_tensor(
            out=st[:],
            in0=iota3[:, :, 0:nch],
            in1=dstloc_sb[:, gc0:gc0 + nch].unsqueeze(1).to_broadcast([P, P, nch]),
            op=mybir.AluOpType.is_equal)
        return st

    for g, grp in enumerate(pl.groups):
        nclo, nchi = grp["nclo"], grp["nchi"]
        staging = stage.tile([P, nclo + nchi, H], F16, tag="staging")
        gather(staging, 0, i_T[:], idxlo_sb, grp["lo_col0"], nclo)
        gather(staging, nclo, i_T[HIB:, :], idxhi_sb, grp["hi_col0"], nchi)
        for ti, td in enumerate(grp["tiles"]):
            t = GROUP_T0[g] + ti
            stlo = onehot(td["gc_lo"], td["clo"]) if td["clo"] else None
            sthi = onehot(td["gc_hi"], td["chi"]) if td["chi"] else None
            ypsum = yp_pool.tile([P, H], F32, space="PSUM")
            # self-loop rows: ypsum = ownT_tile^T @ I  (= own rows, [slot, feat])
            nc.tensor.matmul(out=ypsum[:], lhsT=ownT_sb[:, t * P:(t + 1) * P],
                             rhs=identH[:],
                             start=True, stop=(td["clo"] + td["chi"] == 0))
            for i in range(td["clo"]):
                nc.tensor.matmul(
                    out=ypsum[:], lhsT=stlo[:, :, i],
                    rhs=staging[:, td["sp_lo"] + i, :],
                    start=False,
                    stop=(i == td["clo"] - 1 and td["chi"] == 0))
            for i in range(td["chi"]):
                nc.tensor.matmul(
                    out=ypsum[:], lhsT=sthi[:, :, i],
                    rhs=staging[:, td["sp_hi"] + i, :],
                    start=False, stop=(i == td["chi"] - 1))
            consume_tile(t, ypsum)
            if after_tile is not None:
                after_tile(t)


def _vec_input(nc, const, name):
    ap = nc.dram_tensor(name, [H, 1], F32, kind="ExternalInput").ap()
    sb = const.tile([H, 1], F32, tag=f"vec_{name}")
    nc.sync.dma_start(out=sb[:], in_=ap[:])
    return sb


def _build_BC(pl):
    nc = bacc.Bacc("TRN2", target_bir_lowering=False, debug=False,
                   num_devices=NCORES, dynamic_dma_scratch_size=SCRATCH,
                   num_swdge_queues=NQ)
    i_T = nc.dram_tensor("T", [TAB, H], F16, kind="ExternalInput").ap()
    i_W = nc.dram_tensor("W", [H, H], BF16, kind="ExternalInput").ap()
    i_dinv = nc.dram_tensor("dinv", [P, TILES], F32, kind="ExternalInput").ap()
    o_T = nc.dram_tensor("Tout", [SLOTS, H], F16, kind="ExternalOutput").ap()
    with tile.TileContext(nc) as tc:
        with ExitStack() as ctx:
            const = ctx.enter_context(tc.tile_pool(name="bc_const", bufs=1))
            ycp_pool = ctx.enter_context(tc.tile_pool(name="ycp", bufs=3))
            h_pool = ctx.enter_context(tc.tile_pool(name="ht", bufs=3))
            tps_pool = ctx.enter_context(tc.tile_pool(name="tps", bufs=2, space="PSUM"))

            b_sb = _vec_input(nc, const, "bvec")
            g_sb = _vec_input(nc, const, "bn_g")
            bb_sb = _vec_input(nc, const, "bn_b")
            m_sb = _vec_input(nc, const, "bn_m")
            v_sb = _vec_input(nc, const, "bn_v")
            # scale = g / sqrt(v+eps); bias = (b - m)*scale + beta
            eps = const.tile([H, 1], F32)
            nc.vector.memset(eps[:], BN_EPS)
            sq = const.tile([H, 1], F32)
            nc.scalar.activation(out=sq[:], in_=v_sb[:],
                                 func=mybir.ActivationFunctionType.Sqrt,
                                 bias=eps[:], scale=1.0)
            rs = const.tile([H, 1], F32)
            nc.vector.reciprocal(out=rs[:], in_=sq[:])
            scale = const.tile([H, 1], F32)
            nc.vector.tensor_mul(out=scale[:], in0=rs[:], in1=g_sb[:])
            bias = const.tile([H, 1], F32)
            nc.vector.tensor_sub(out=bias[:], in0=b_sb[:], in1=m_sb[:])
            nc.vector.tensor_mul(out=bias[:], in0=bias[:], in1=scale[:])
            nc.vector.tensor_add(out=bias[:], in0=bias[:], in1=bb_sb[:])

            identB = const.tile([P, P], BF16)
            make_identity(nc, identB[:])
            dinv_sb = const.tile([P, TILES], F32)
            nc.sync.dma_start(out=dinv_sb[:], in_=i_dinv[:])
            w_sb = const.tile([H, H], BF16)
            nc.sync.dma_start(out=w_sb[:], in_=i_W[:])

            h_tiles = {}

            def consume(t, ypsum):
                ycp = ycp_pool.tile([P, H], BF16)
                nc.scalar.activation(out=ycp[:], in_=ypsum[:],
                                     func=mybir.ActivationFunctionType.Copy,
                                     scale=dinv_sb[:, t:t + 1])
                tp = tps_pool.tile([P, P], BF16, space="PSUM")
                nc.tensor.transpose(out=tp[:], in_=ycp[:], identity=identB[:])
                h_t = h_pool.tile([P, H], BF16)
                nc.scalar.activation(
                    out=h_t[:], in_=tp[:],
                    func=mybir.ActivationFunctionType.Relu,
                    bias=bias[:], scale=scale[:])
                h_tiles[t] = h_t

            emit = _make_gemm_emitter(nc, ctx, tc,
                                      lambda t: [(h_tiles.pop(t)[:], w_sb[:])],
                                      o_T, dinv_sb, to_bufs=6)

            # emit each tile's GEMM right after its scatter completes so the
            # table write overlaps the remaining scatter instead of tailing it
            _scatter_body(nc, ctx, tc, pl, i_T, consume, emit)
    nc.compile()
    return nc


def _build_D(pl):
    nc = bacc.Bacc("TRN2", target_bir_lowering=False, debug=False,
                   num_devices=NCORES, dynamic_dma_scratch_size=SCRATCH,
                   num_swdge_queues=NQ)
    i_T = nc.dram_tensor("T", [TAB, H], F16, kind="ExternalInput").ap()
    i_bv = nc.dram_tensor("batchval", [P, TILES], F16, kind="ExternalInput").ap()
    i_gi = nc.dram_tensor("giota", [1, NGRAPH * TILES], F16,
                          kind="ExternalInput").ap()
    i_dinv = nc.dram_tensor("dinv", [P, TILES], F32, kind="ExternalInput").ap()
    o_pool = nc.dram_tensor("pool", [NGRAPH, H], F32, kind="ExternalOutput").ap()
    with tile.TileContext(nc) as tc:
        with ExitStack() as ctx:
            const = ctx.enter_context(tc.tile_pool(name="d_const", bufs=1))
            h3_pool = ctx.enter_context(tc.tile_pool(name="h3", bufs=3))
            pp_pool = ctx.enter_context(tc.tile_pool(name="pp", bufs=1, space="PSUM"))

            bv_sb = const.tile([P, TILES], F16)
            nc.sync.dma_start(out=bv_sb[:], in_=i_bv[:])
            gi_sb = const.tile([P, NGRAPH * TILES], F16)
            nc.sync.dma_start(out=gi_sb[:], in_=i_gi.to_broadcast([P, NGRAPH * TILES]))
            dinv_sb = const.tile([P, TILES], F32)
            nc.sync.dma_start(out=dinv_sb[:], in_=i_dinv[:])
            # oh_all[p, g, t] = (batchval[p, t] == g)
            oh_all = const.tile([P, NGRAPH, TILES], F16)
            nc.vector.tensor_tensor(
                out=oh_all[:],
                in0=gi_sb[:].rearrange("p (g t) -> p g t", g=NGRAPH, t=TILES),
                in1=bv_sb[:].unsqueeze(1).to_broadcast([P, NGRAPH, TILES]),
                op=mybir.AluOpType.is_equal)
            pp = pp_pool.tile([NGRAPH, H], F32, space="PSUM")

            def consume(t, ypsum):
                h3 = h3_pool.tile([P, H], F16)
                nc.scalar.activation(out=h3[:], in_=ypsum[:],
                                     func=mybir.ActivationFunctionType.Copy,
                                     scale=dinv_sb[:, t:t + 1])
                nc.tensor.matmul(out=pp[:], lhsT=oh_all[:, :, t], rhs=h3[:],
                                 start=(t == 0), stop=(t == TILES - 1))

            _scatter_body(nc, ctx, tc, pl, i_T, consume)
            pcp = const.tile([NGRAPH, H], F32)
            nc.vector.tensor_copy(out=pcp[:], in_=pp[:])
            nc.sync.dma_start(out=o_pool[:], in_=pcp[:])
    nc.compile()
    return nc


# ------------------------------------------------------------------- driver --

def _run(nc, in_maps):
    res = run_bass_kernel_spmd(nc, in_maps, core_ids=list(range(NCORES)),
                               trace=TRACE)
    if TRACE:
        LAST_EXEC_NS.append(res.exec_time_ns)
    return res.results


def _assemble_table(pl, shards):
    T = np.zeros((TAB, H), dtype=np.float16)
    for c in range(NCORES):
        T[pl.rowmap[c]] = shards[c]
    return T


def kernel(**inputs):
    ins = {k: np.asarray(v) for k, v in inputs.items()}
    key = hashlib.sha1(
        ins["edge_index"].tobytes() + ins["batch"].tobytes()
    ).hexdigest()
    if key not in _PLAN_CACHE:
        _PLAN_CACHE[key] = _make_plan(ins["edge_index"], ins["batch"], ins["x"])
    pl = _PLAN_CACHE[key]

    pk = pl.key
    if pk not in _PROG_CACHE:
        _PROG_CACHE[pk] = {
            "A": _build_A(pl),
            "BC": _build_BC(pl),
            "D": _build_D(pl),
        }
    progs = _PROG_CACHE[pk]

    LAST_EXEC_NS.clear()
    W1 = ins["W1"].astype(BF16_NP)
    # Launch A: T1 = dinv * (x @ W1)
    resA = _run(progs["A"], [
        {"xT": pl.cores[c]["xT"], "W": W1, "dinv": pl.cores[c]["dinv"]}
        for c in range(NCORES)
    ])
    shardsA = [r["Tout"] for r in resA]
    T1 = _assemble_table(pl, shardsA)

    def meta(c):
        cc = pl.cores[c]
        return {"idxlo": cc["idxlo"], "idxhi": cc["idxhi"],
                "dstloc": cc["dstloc"], "iota_rep": pl.iota_rep,
                "dinv": cc["dinv"]}

    def ownT(shard):
        return np.ascontiguousarray(shard.T)

    def vec(name):
        return ins[name].astype(np.float32).reshape(H, 1)

    # Launch B: layer-1 scatter + BN1/ReLU + @W2
    resB = _run(progs["BC"], [
        {**meta(c), "T": T1, "ownT": ownT(shardsA[c]), "W": ins["W2"].astype(BF16_NP),
         "bvec": vec("b1"), "bn_g": vec("bn1_g"), "bn_b": vec("bn1_b"),
         "bn_m": vec("bn1_m"), "bn_v": vec("bn1_v")} for c in range(NCORES)
    ])
    shardsB = [r["Tout"] for r in resB]
    T2 = _assemble_table(pl, shardsB)

    # Launch C: layer-2 scatter + BN2/ReLU + @W3
    resC = _run(progs["BC"], [
        {**meta(c), "T": T2, "ownT": ownT(shardsB[c]), "W": ins["W3"].astype(BF16_NP),
         "bvec": vec("b2"), "bn_g": vec("bn2_g"), "bn_b": vec("bn2_b"),
         "bn_m": vec("bn2_m"), "bn_v": vec("bn2_v")} for c in range(NCORES)
    ])
    shardsC = [r["Tout"] for r in resC]
    T3 = _assemble_table(pl, shardsC)

    # Launch D: layer-3 scatter + pooling partials
    resD = _run(progs["D"], [
        {**meta(c), "T": T3, "ownT": ownT(shardsC[c]),
         "batchval": pl.cores[c]["batchval"], "giota": pl.giota}
        for c in range(NCORES)
    ])
    pooled_sum = np.sum([r["pool"] for r in resD], axis=0).astype(np.float64)

    counts = pl.counts.astype(np.float64)
    pooled_sum += counts[:, None] * ins["b3"].astype(np.float64)[None, :]
    pooled = pooled_sum / np.maximum(counts, 1.0)[:, None]

    z = np.maximum(pooled @ ins["Wc1"].astype(np.float64)
                   + ins["bc1"].astype(np.float64), 0.0)
    out = z @ ins["Wc2"].astype(np.float64) + ins["bc2"].astype(np.float64)
    return out.astype(np.float32)


# revision 49
# speedup vs baseline: 1.6474x; 1.0158x over previous
"""Trainium2 Bass kernel for DocumentClassificationGNN (3-layer GCN + BN/ReLU +
global mean pool + MLP head), distributed over 8 NeuronCores.

Strategy (node/graph parallel, per the sharding hint):
  - Nodes are assigned to (core, slot) sorted by in-degree so every core/tile
    carries a balanced edge load.  Edges are partitioned by DESTINATION core so
    the segment-sum scatter is device-local.
  - Per layer: a dense GEMM produces a node-major fp16 feature table that the
    host replicates to all cores ("all-gather" through the host between
    launches); each core gathers its in-edge source rows with dma_gather and
    scatter-adds them into PSUM with one-hot matmuls.
  - The symmetric norm deg^-1/2[src]*deg^-1/2[dst] is SEPARABLE: table rows
    are pre-scaled by dinv[src] at write time and the scatter output is
    post-scaled by dinv[dst], so the one-hot matrices are pure 0/1 and are
    generated in batched DVE is_equal ops (2-byte fast path) with the chunk
    dim innermost: s_t[p, j, c].
  - Self-loops never enter the edge stream: each tile's own table rows are
    bulk-loaded and added via one identity matmul (contribution dinv_d*T'[d]).
  - conv bias + BN + ReLU fuse into one scalar-engine activation; GEMMs run in
    bf16; launch D does per-tile onehot(batch) pooling accumulated in one PSUM
    bank.
  - Device output: per-core pooled partial sums [64, 128].  Host: sum, +n_g*b3,
    divide by counts, tiny classifier MLP.

Programs (3 compiles, 4 launches):
  A : T1 = dinv * (x @ W1)                          -> T1 table shard
  BC: Y = scatter(T); h' = relu(BN(dinv*Y + b)); T' = dinv * (h' @ W_next)
  D : Y3 = scatter(T3); pooled_partial = onehot(batch)^T @ (dinv*Y3)
"""

import hashlib
import numpy as np
from contextlib import ExitStack

import ml_dtypes

import concourse.bass as bass
import concourse.bacc as bacc
import concourse.tile as tile
from concourse import mybir
from concourse.bass_utils import run_bass_kernel_spmd
from concourse.masks import make_identity

P = 128
NCORES = 8
N = 50000
D_IN = 256
H = 128
NGRAPH = 64
SLOTS = 6272            # 49 tiles of 128 slots per core (6250 real nodes + pad)
TILES = SLOTS // P      # 49
RAW = NCORES * SLOTS    # 50176
TAB = RAW               # table = concatenated shards, no extra rows
HIB = 4 * SLOTS         # hi-region gather base: cores 0-3 lo, cores 4-7 hi
                        # (both index ranges fit the int16 dma_gather indices)
ZLO_ROW = SLOTS - 1     # core-0 pad slot: always-zero row used by lo pads
# dst tiles per gather group: small first groups so the first tiles'
# staging lands early and the PE/consume pipeline starts ~15us sooner
GROUP_SIZES = [2, 5, 7, 7, 7, 7, 7, 7]
assert sum(GROUP_SIZES) == TILES
NGROUPS = len(GROUP_SIZES)
GROUP_T0 = [sum(GROUP_SIZES[:g]) for g in range(NGROUPS)]
BN_EPS = 1e-5

SCRATCH = 16384         # SWDGE ring: 16384/16 = 1024 descriptors per queue
MAXCH = 8               # chunks per dma_gather call (8*128 = 1024, HW limit)
NQ = 2                  # SWDGE queues (desc-gen pipelines against transfer)

F16 = mybir.dt.float16
BF16 = mybir.dt.bfloat16
F32 = mybir.dt.float32
I16 = mybir.dt.int16
BF16_NP = ml_dtypes.bfloat16

# module-level knobs / perf results (test.py pokes these)
TRACE = False
LAST_EXEC_NS = []       # per-launch exec_time_ns (when TRACE)

_PLAN_CACHE = {}
_PROG_CACHE = {}


# ---------------------------------------------------------------- host prep --

def _wrap_idx(flat):
    """dma_gather index layout: idx i -> [i%16, i//16], replicated to 128 parts."""
    n = len(flat)
    assert n % 16 == 0
    arr = np.asarray(flat, dtype=np.int16).reshape(n // 16, 16).T.copy()
    return np.tile(arr, (8, 1))


class _Plan:
    pass


def _distribute(total, bins):
    base, extra = divmod(int(total), bins)
    out = np.full(bins, base, dtype=np.int64)
    out[:extra] += 1
    return out


def _pack_core(lo, hi, kL, kH):
    """Pack one core's nodes into TILES tiles of <=128 slots, steering the
    per-tile lo/hi in-edge sums toward the shared chunk budgets kL/kH*128.

    Worst-fit decreasing on min remaining (lo, hi) headroom.
    """
    n = len(lo)
    loR = (kL * P).astype(np.float64)
    hiR = (kH * P).astype(np.float64)
    cap = np.full(TILES, P, dtype=np.int64)
    # all pad (empty) slots must be the LAST slots of the last tile: they are
    # the always-zero rows targeted by gather padding and the table-write memset
    cap[TILES - 1] = P - (TILES * P - n)
    filled = np.zeros(TILES, dtype=np.int64)
    slot = np.empty(n, dtype=np.int64)
    order = np.argsort(-(lo + hi), kind="stable")
    for i in order:
        score = np.minimum(loR - lo[i], hiR - hi[i])
        score[filled >= cap] = -np.inf
        t = int(np.argmax(score))
        loR[t] -= lo[i]
        hiR[t] -= hi[i]
        slot[i] = t * P + filled[t]
        filled[t] += 1
    return slot


def _make_plan(edge_index, batch, x):
    pl = _Plan()
    src = np.asarray(edge_index[0], dtype=np.int64)
    dst = np.asarray(edge_index[1], dtype=np.int64)
    batch = np.asarray(batch, dtype=np.int64)

    deg = np.bincount(dst, minlength=N).astype(np.int64) + 1
    dinv = (1.0 / np.sqrt(deg)).astype(np.float32)

    order = np.argsort(-deg, kind="stable")
    rank = np.empty(N, dtype=np.int64)
    rank[order] = np.arange(N)
    core_of = rank % NCORES

    # lo/hi membership of an edge depends only on its source CORE (the hi
    # gather base sits on the core-3/4 boundary), so per-node lo/hi in-edge
    # counts are fixed before slots are chosen -> bin-pack nodes into tiles
    # so per-(tile, half) counts land just under multiples of 128.
    islo_e = core_of[src] < NCORES // 2
    lo_n = np.bincount(dst[islo_e], minlength=N)
    hi_n = np.bincount(dst[~islo_e], minlength=N)
    totlo = np.zeros(NCORES, dtype=np.int64)
    tothi = np.zeros(NCORES, dtype=np.int64)
    for c in range(NCORES):
        m = core_of == c
        totlo[c] = lo_n[m].sum()
        tothi[c] = hi_n[m].sum()
    SLACK = 3
    kL = _distribute(-(-totlo.max() // P) + SLACK, TILES)
    kH = _distribute(-(-tothi.max() // P) + SLACK, TILES)
    slot_of = np.empty(N, dtype=np.int64)
    for c in range(NCORES):
        nodes = np.where(core_of == c)[0]
        slot_of[nodes] = _pack_core(lo_n[nodes], hi_n[nodes], kL, kH)
    raw_of = core_of * SLOTS + slot_of
    grow_of = raw_of                        # table row per node (pure concat)

    # real edges only: self-loops are handled by the per-tile identity matmul
    es, ed = src, dst
    ecore = core_of[ed]
    eslot = slot_of[ed]
    etile = eslot // P
    edstloc = eslot % P
    esg = grow_of[es]
    islo = islo_e

    # per-core sorted segment arrays
    NSEG = TILES * 2   # segment id: 2*tile + (0 lo / 1 hi)
    per_core = []
    seg_counts = np.zeros((NCORES, NSEG), dtype=np.int64)
    for c in range(NCORES):
        m = ecore == c
        seg = etile[m] * 2 + (~islo[m]).astype(np.int64)
        o2 = np.lexsort((esg[m], seg))
        d = {
            "seg": seg[o2],
            "dstloc": edstloc[m][o2],
            "esg": esg[m][o2],
        }
        seg_counts[c] = np.bincount(d["seg"], minlength=NSEG)
        per_core.append(d)

    # chunk plan: per tile, lo/hi chunk counts = max over cores
    CLO = np.ceil(seg_counts[:, 0::2].max(axis=0) / P).astype(int)
    CHI = np.ceil(seg_counts[:, 1::2].max(axis=0) / P).astype(int)
    # chunk order: group-major; within group: all lo chunks (tile order), then hi
    seg_chunk_start = np.zeros(NSEG, dtype=np.int64)   # global chunk idx per seg
    grp_clo = np.zeros(NGROUPS, dtype=np.int64)
    grp_chi = np.zeros(NGROUPS, dtype=np.int64)
    gcb = np.zeros(NGROUPS + 1, dtype=np.int64)
    for g in range(NGROUPS):
        ts = range(GROUP_T0[g], GROUP_T0[g] + GROUP_SIZES[g])
        grp_clo[g] = sum(CLO[t] for t in ts)
        grp_chi[g] = sum(CHI[t] for t in ts)
        ofs = gcb[g]
        for t in ts:
            seg_chunk_start[2 * t] = ofs
            ofs += CLO[t]
        for t in ts:
            seg_chunk_start[2 * t + 1] = ofs
            ofs += CHI[t]
        gcb[g + 1] = ofs
    CTOT = int(gcb[-1])

    # per-chunk default fill (pads): lo chunks -> ZLO, hi chunks -> absolute hi zero
    chunk_is_hi = np.zeros(CTOT, dtype=bool)
    for t in range(TILES):
        s = seg_chunk_start[2 * t + 1]
        chunk_is_hi[s:s + CHI[t]] = True

    pl.cores = []
    for c in range(NCORES):
        d = per_core[c]
        npad = CTOT * P
        dstloc_pad = np.zeros(npad, dtype=np.float16)
        row_pad = np.where(np.repeat(chunk_is_hi, P), RAW - 1,
                           ZLO_ROW).astype(np.int64)
        # position of each real edge
        cnt = seg_counts[c]
        seg_first = np.concatenate([[0], np.cumsum(cnt)[:-1]])
        within = np.arange(len(d["seg"])) - seg_first[d["seg"]]
        pos = seg_chunk_start[d["seg"]] * P + within
        dstloc_pad[pos] = d["dstloc"].astype(np.float16)
        row_pad[pos] = d["esg"]

        # gather index arrays (lo then hi, group-major)
        lo_parts, hi_parts = [], []
        for g in range(NGROUPS):
            a = gcb[g] * P
            b = a + grp_clo[g] * P
            e = gcb[g + 1] * P
            lo_parts.append(row_pad[a:b])
            hi_parts.append(row_pad[b:e] - HIB)
        lo_flat = np.concatenate(lo_parts)
        hi_flat = np.concatenate(hi_parts)
        assert lo_flat.min() >= 0 and lo_flat.max() < HIB <= 32768
        assert hi_flat.min() >= 0 and hi_flat.max() <= RAW - 1 - HIB <= 32767

        core = {
            "idxlo": _wrap_idx(lo_flat),
            "idxhi": _wrap_idx(hi_flat),
            "dstloc": dstloc_pad.reshape(CTOT, P).T.copy(),
        }
        pl.cores.append(core)

    # group gather call metadata (columns into wrapped idx tensors)
    pl.lo_cols = int(grp_clo.sum() * P // 16)
    pl.hi_cols = int(grp_chi.sum() * P // 16)
    lo_c0 = np.concatenate([[0], np.cumsum(grp_clo * 8)])
    hi_c0 = np.concatenate([[0], np.cumsum(grp_chi * 8)])
    pl.groups = []
    for g in range(NGROUPS):
        tiles = []
        for t in range(GROUP_T0[g], GROUP_T0[g] + GROUP_SIZES[g]):
            lo_local = int(seg_chunk_start[2 * t] - gcb[g])
            hi_local = int(seg_chunk_start[2 * t + 1] - gcb[g])
            tiles.append({
                "clo": int(CLO[t]), "chi": int(CHI[t]),
                "sp_lo": lo_local, "sp_hi": hi_local,
                "gc_lo": int(seg_chunk_start[2 * t]),
                "gc_hi": int(seg_chunk_start[2 * t + 1]),
            })
        pl.groups.append({
            "nclo": int(grp_clo[g]), "nchi": int(grp_chi[g]),
            "lo_col0": int(lo_c0[g]), "hi_col0": int(hi_c0[g]),
            "tiles": tiles,
        })
    pl.CTOT = CTOT
    pl.NCHMAX = int(max(CLO.max(), CHI.max()))

    # slot -> node map, batch values, dinv per slot, xT shards, table row map
    node_at = np.full((NCORES, SLOTS), -1, dtype=np.int64)
    node_at[core_of, slot_of] = np.arange(N)
    bv = np.full((NCORES, SLOTS), 99.0, dtype=np.float16)
    dv = np.zeros((NCORES, SLOTS), dtype=np.float32)   # pad slots: dinv = 0
    valid = node_at >= 0
    bv[valid] = batch[node_at[valid]].astype(np.float16)
    dv[valid] = dinv[node_at[valid]]
    for c in range(NCORES):
        pl.cores[c]["batchval"] = bv[c].reshape(TILES, P).T.copy()  # [128, 49]
        pl.cores[c]["dinv"] = dv[c].reshape(TILES, P).T.copy()      # [128, 49]
        xt = np.zeros((D_IN, SLOTS), dtype=np.float32)
        v = valid[c]
        xt[:, v] = np.asarray(x, dtype=np.float32)[node_at[c][v]].T
        pl.cores[c]["xT"] = xt.astype(BF16_NP)

    pl.rowmap = np.arange(RAW, dtype=np.int64).reshape(NCORES, SLOTS)
    pl.counts = np.bincount(batch, minlength=NGRAPH).astype(np.float32)
    pl.iota_rep = np.repeat(np.arange(P), pl.NCHMAX).astype(np.float16).reshape(1, -1)
    pl.giota = np.repeat(np.arange(NGRAPH), TILES).astype(np.float16).reshape(1, -1)
    pl.key = (tuple(CLO), tuple(CHI))
    return pl


# ---------------------------------------------------------- program builders --

def _make_gemm_emitter(nc, ctx, tc, k_tiles_fn, o_T, dinv_sb, bufs=2,
                       to_bufs=None):
    """Returns emit(t): table rows for slot tile t.

    out[slot, fout] = sum_k lhsT_k^T @ rhs_k with lhsT = feat-major input
    block (no output transpose needed); dinv-scale + fp16 cast -> o_T rows.
    """
    gps_pool = ctx.enter_context(
        tc.tile_pool(name="gemm_ps", bufs=bufs, space="PSUM"))
    to_pool = ctx.enter_context(
        tc.tile_pool(name="gemm_to", bufs=to_bufs or 2))
    WB = 4  # tiles per table-write DMA (amortizes the 625ns HWDGE slot)
    state = {}

    def emit(t):
        kt = k_tiles_fn(t)
        gps = gps_pool.tile([P, H], F32, space="PSUM")
        for ki, (lhsT, rhs) in enumerate(kt):
            nc.tensor.matmul(out=gps[:], lhsT=lhsT, rhs=rhs,
                             start=(ki == 0), stop=(ki == len(kt) - 1))
        j = t % WB
        if j == 0:
            to_new = to_pool.tile([P, WB, H], F16, tag="to")
            state["to"] = to_new
        to = state["to"]
        # pad slots have dinv == 0, so this scale also keeps their table rows
        # ZERO (they serve as the gather targets for chunk padding positions)
        nc.scalar.activation(out=to[:, j, :], in_=gps[:],
                             func=mybir.ActivationFunctionType.Copy,
                             scale=dinv_sb[:, t:t + 1])
        if j == WB - 1 or t == TILES - 1:
            t0, n = t - j, j + 1
            dst = o_T[t0 * P:(t0 + n) * P, :].rearrange(
                "(j p) h -> p j h", j=n, p=P)
            nc.sync.dma_start(out=dst, in_=to[:, 0:n, :])

    return emit


def _build_A(pl):
    nc = bacc.Bacc("TRN2", target_bir_lowering=False, debug=False, num_devices=NCORES)
    i_xT = nc.dram_tensor("xT", [D_IN, SLOTS], BF16, kind="ExternalInput").ap()
    i_W = nc.dram_tensor("W", [D_IN, H], BF16, kind="ExternalInput").ap()
    i_dinv = nc.dram_tensor("dinv", [P, TILES], F32, kind="ExternalInput").ap()
    o_T = nc.dram_tensor("Tout", [SLOTS, H], F16, kind="ExternalOutput").ap()
    with tile.TileContext(nc) as tc:
        with ExitStack() as ctx:
            const = ctx.enter_context(tc.tile_pool(name="const", bufs=1))
            dinv_sb = const.tile([P, TILES], F32)
            nc.sync.dma_start(out=dinv_sb[:], in_=i_dinv[:])
            w0 = const.tile([P, H], BF16)
            nc.sync.dma_start(out=w0[:], in_=i_W[0:P, :])
            w1 = const.tile([P, H], BF16)
            nc.sync.dma_start(out=w1[:], in_=i_W[P:2 * P, :])
            x0 = const.tile([P, SLOTS], BF16)
            x1 = const.tile([P, SLOTS], BF16)
            HLF = SLOTS // 2
            nc.sync.dma_start(out=x0[:, 0:HLF], in_=i_xT[0:P, 0:HLF])
            nc.sync.dma_start(out=x1[:, 0:HLF], in_=i_xT[P:2 * P, 0:HLF])
            nc.sync.dma_start(out=x0[:, HLF:], in_=i_xT[0:P, HLF:])
            nc.sync.dma_start(out=x1[:, HLF:], in_=i_xT[P:2 * P, HLF:])

            def k_tiles(t):
                sl = slice(t * P, (t + 1) * P)
                return [(x0[:, sl], w0[:]), (x1[:, sl], w1[:])]

            emit = _make_gemm_emitter(nc, ctx, tc, k_tiles, o_T, dinv_sb, bufs=6,
                                      to_bufs=13)
            for t in range(TILES):
                emit(t)
    nc.compile()
    return nc


def _scatter_body(nc, ctx, tc, pl, i_T, consume_tile, after_tile=None,
                  mid_loads=None):
    """Shared gather + one-hot matmul scatter loop.

    consume_tile(t, ypsum) handles the per-tile PSUM result
    (ypsum = sum over in-edges of dinv[src]-scaled source rows, incl self-loop).
    """
    const = ctx.enter_context(tc.tile_pool(name="sc_const", bufs=1))
    stage = ctx.enter_context(tc.tile_pool(name="staging", bufs=2))
    st_pool = ctx.enter_context(tc.tile_pool(name="st", bufs=4))
    yp_pool = ctx.enter_context(tc.tile_pool(name="yps", bufs=3, space="PSUM"))

    i_idxlo = nc.dram_tensor("idxlo", [P, pl.lo_cols], I16, kind="ExternalInput").ap()
    i_idxhi = nc.dram_tensor("idxhi", [P, pl.hi_cols], I16, kind="ExternalInput").ap()
    i_dstloc = nc.dram_tensor("dstloc", [P, pl.CTOT], F16, kind="ExternalInput").ap()
    i_iota = nc.dram_tensor("iota_rep", [1, P * pl.NCHMAX], F16,
                            kind="ExternalInput").ap()
    i_ownT = nc.dram_tensor("ownT", [H, SLOTS], F16, kind="ExternalInput").ap()

    idxlo_sb = const.tile([P, pl.lo_cols], I16)
    nc.sync.dma_start(out=idxlo_sb[:], in_=i_idxlo[:])
    idxhi_sb = const.tile([P, pl.hi_cols], I16)
    nc.sync.dma_start(out=idxhi_sb[:], in_=i_idxhi[:])
    dstloc_sb = const.tile([P, pl.CTOT], F16)
    nc.sync.dma_start(out=dstloc_sb[:], in_=i_dstloc[:])
    iota_sb = const.tile([P, P * pl.NCHMAX], F16)
    nc.sync.dma_start(out=iota_sb[:], in_=i_iota.to_broadcast([P, P * pl.NCHMAX]))
    iota3 = iota_sb[:].rearrange("p (j c) -> p j c", j=P, c=pl.NCHMAX)
    identH = const.tile([P, P], F16)
    make_identity(nc, identH[:])
    ownT_sb = const.tile([P, SLOTS], F16)
    nc.sync.dma_start(out=ownT_sb[:], in_=i_ownT[:])
    if mid_loads is not None:
        # non-scatter-critical input loads go AFTER the idx/ownT loads so the
        # first gather is not stuck behind their fixed HWDGE slots
        mid_loads()

    qn = [0]

    def gather(staging, base, src_ap, idx_sb, col0, nch):
        for o in range(0, nch, MAXCH):
            n = min(MAXCH, nch - o)
            c0 = col0 + o * 8
            nc.gpsimd.dma_gather(
                out_ap=staging[:, base + o:base + o + n, :], in_ap=src_ap,
                idxs_ap=idx_sb[:, c0:c0 + n * 8],
                num_idxs=n * P, num_idxs_reg=n * P, elem_size=H,
                queue_num=qn[0])
            qn[0] = (qn[0] + 1) % NQ

    def onehot(gc0, nch):
        st = st_pool.tile([P, P, nch], F16, tag="st")
        nc.vector.tensor_tensor(
            out=st[:],
            in0=iota3[:, :, 0:nch],
            in1=dstloc_sb[:, gc0:gc0 + nch].unsqueeze(1).to_broadcast([P, P, nch]),
            op=mybir.AluOpType.is_equal)
        return st

    for g, grp in enumerate(pl.groups):
        nclo, nchi = grp["nclo"], grp["nchi"]
        staging = stage.tile([P, nclo + nchi, H], F16, tag="staging")
        gather(staging, 0, i_T[:], idxlo_sb, grp["lo_col0"], nclo)
        gather(staging, nclo, i_T[HIB:, :], idxhi_sb, grp["hi_col0"], nchi)
        for ti, td in enumerate(grp["tiles"]):
            t = GROUP_T0[g] + ti
            stlo = onehot(td["gc_lo"], td["clo"]) if td["clo"] else None
            sthi = onehot(td["gc_hi"], td["chi"]) if td["chi"] else None
            ypsum = yp_pool.tile([P, H], F32, space="PSUM")
            # self-loop rows: ypsum = ownT_tile^T @ I  (= own rows, [slot, feat])
            nc.tensor.matmul(out=ypsum[:], lhsT=ownT_sb[:, t * P:(t + 1) * P],
                             rhs=identH[:],
                             start=True, stop=(td["clo"] + td["chi"] == 0))
            for i in range(td["clo"]):
                nc.tensor.matmul(
                    out=ypsum[:], lhsT=stlo[:, :, i],
                    rhs=staging[:, td["sp_lo"] + i, :],
                    start=False,
                    stop=(i == td["clo"] - 1 and td["chi"] == 0))
            for i in range(td["chi"]):
                nc.tensor.matmul(
                    out=ypsum[:], lhsT=sthi[:, :, i],
                    rhs=staging[:, td["sp_hi"] + i, :],
                    start=False, stop=(i == td["chi"] - 1))
            consume_tile(t, ypsum)
            if after_tile is not None:
                after_tile(t)


def _vec_input(nc, const, name):
    ap = nc.dram_tensor(name, [H, 1], F32, kind="ExternalInput").ap()
    sb = const.tile([H, 1], F32, tag=f"vec_{name}")
    nc.sync.dma_start(out=sb[:], in_=ap[:])
    return sb


def _build_BC(pl):
    nc = bacc.Bacc("TRN2", target_bir_lowering=False, debug=False,
                   num_devices=NCORES, dynamic_dma_scratch_size=SCRATCH,
                   num_swdge_queues=NQ)
    i_T = nc.dram_tensor("T", [TAB, H], F16, kind="ExternalInput").ap()
    i_W = nc.dram_tensor("W", [H, H], BF16, kind="ExternalInput").ap()
    i_dinv = nc.dram_tensor("dinv", [P, TILES], F32, kind="ExternalInput").ap()
    o_T = nc.dram_tensor("Tout", [SLOTS, H], F16, kind="ExternalOutput").ap()
    with tile.TileContext(nc) as tc:
        with ExitStack() as ctx:
            const = ctx.enter_context(tc.tile_pool(name="bc_const", bufs=1))
            ycp_pool = ctx.enter_context(tc.tile_pool(name="ycp", bufs=3))
            h_pool = ctx.enter_context(tc.tile_pool(name="ht", bufs=3))
            tps_pool = ctx.enter_context(tc.tile_pool(name="tps", bufs=2, space="PSUM"))

            identB = const.tile([P, P], BF16)
            make_identity(nc, identB[:])
            dinv_sb = const.tile([P, TILES], F32)
            w_sb = const.tile([H, H], BF16)
            scale = const.tile([H, 1], F32)
            bias = const.tile([H, 1], F32)

            def mid_loads():
                b_sb = _vec_input(nc, const, "bvec")
                g_sb = _vec_input(nc, const, "bn_g")
                bb_sb = _vec_input(nc, const, "bn_b")
                m_sb = _vec_input(nc, const, "bn_m")
                v_sb = _vec_input(nc, const, "bn_v")
                nc.sync.dma_start(out=dinv_sb[:], in_=i_dinv[:])
                nc.sync.dma_start(out=w_sb[:], in_=i_W[:])
                # scale = g / sqrt(v+eps); bias = (b - m)*scale + beta
                eps = const.tile([H, 1], F32)
                nc.vector.memset(eps[:], BN_EPS)
                sq = const.tile([H, 1], F32)
                nc.scalar.activation(out=sq[:], in_=v_sb[:],
                                     func=mybir.ActivationFunctionType.Sqrt,
                                     bias=eps[:], scale=1.0)
                rs = const.tile([H, 1], F32)
                nc.vector.reciprocal(out=rs[:], in_=sq[:])
                nc.vector.tensor_mul(out=scale[:], in0=rs[:], in1=g_sb[:])
                nc.vector.tensor_sub(out=bias[:], in0=b_sb[:], in1=m_sb[:])
                nc.vector.tensor_mul(out=bias[:], in0=bias[:], in1=scale[:])
                nc.vector.tensor_add(out=bias[:], in0=bias[:], in1=bb_sb[:])

            h_tiles = {}

            def consume(t, ypsum):
                ycp = ycp_pool.tile([P, H], BF16)
                nc.scalar.activation(out=ycp[:], in_=ypsum[:],
                                     func=mybir.ActivationFunctionType.Copy,
                                     scale=dinv_sb[:, t:t + 1])
                tp = tps_pool.tile([P, P], BF16, space="PSUM")
                nc.tensor.transpose(out=tp[:], in_=ycp[:], identity=identB[:])
                h_t = h_pool.tile([P, H], BF16)
                nc.scalar.activation(
                    out=h_t[:], in_=tp[:],
                    func=mybir.ActivationFunctionType.Relu,
                    bias=bias[:], scale=scale[:])
                h_tiles[t] = h_t

            emit = _make_gemm_emitter(nc, ctx, tc,
                                      lambda t: [(h_tiles.pop(t)[:], w_sb[:])],
                                      o_T, dinv_sb, to_bufs=6)

            # emit each tile's GEMM right after its scatter completes so the
            # table write overlaps the remaining scatter instead of tailing it
            _scatter_body(nc, ctx, tc, pl, i_T, consume, emit,
                          mid_loads=mid_loads)
    nc.compile()
    return nc


def _build_D(pl):
    nc = bacc.Bacc("TRN2", target_bir_lowering=False, debug=False,
                   num_devices=NCORES, dynamic_dma_scratch_size=SCRATCH,
                   num_swdge_queues=NQ)
    i_T = nc.dram_tensor("T", [TAB, H], F16, kind="ExternalInput").ap()
    i_bv = nc.dram_tensor("batchval", [P, TILES], F16, kind="ExternalInput").ap()
    i_gi = nc.dram_tensor("giota", [1, NGRAPH * TILES], F16,
                          kind="ExternalInput").ap()
    i_dinv = nc.dram_tensor("dinv", [P, TILES], F32, kind="ExternalInput").ap()
    o_pool = nc.dram_tensor("pool", [NGRAPH, H], F32, kind="ExternalOutput").ap()
    with tile.TileContext(nc) as tc:
        with ExitStack() as ctx:
            const = ctx.enter_context(tc.tile_pool(name="d_const", bufs=1))
            h3_pool = ctx.enter_context(tc.tile_pool(name="h3", bufs=3))
            pp_pool = ctx.enter_context(tc.tile_pool(name="pp", bufs=1, space="PSUM"))

            bv_sb = const.tile([P, TILES], F16)
            gi_sb = const.tile([P, NGRAPH * TILES], F16)
            dinv_sb = const.tile([P, TILES], F32)
            oh_all = const.tile([P, NGRAPH, TILES], F16)
            pp = pp_pool.tile([NGRAPH, H], F32, space="PSUM")

            def mid_loads():
                nc.sync.dma_start(out=bv_sb[:], in_=i_bv[:])
                nc.sync.dma_start(out=gi_sb[:],
                                  in_=i_gi.to_broadcast([P, NGRAPH * TILES]))
                nc.sync.dma_start(out=dinv_sb[:], in_=i_dinv[:])
                # oh_all[p, g, t] = (batchval[p, t] == g)
                nc.vector.tensor_tensor(
                    out=oh_all[:],
                    in0=gi_sb[:].rearrange("p (g t) -> p g t", g=NGRAPH, t=TILES),
                    in1=bv_sb[:].unsqueeze(1).to_broadcast([P, NGRAPH, TILES]),
                    op=mybir.AluOpType.is_equal)

            def consume(t, ypsum):
                h3 = h3_pool.tile([P, H], F16)
                nc.scalar.activation(out=h3[:], in_=ypsum[:],
                                     func=mybir.ActivationFunctionType.Copy,
                                     scale=dinv_sb[:, t:t + 1])
                nc.tensor.matmul(out=pp[:], lhsT=oh_all[:, :, t], rhs=h3[:],
                                 start=(t == 0), stop=(t == TILES - 1))

            _scatter_body(nc, ctx, tc, pl, i_T, consume,
                          mid_loads=mid_loads)
            pcp = const.tile([NGRAPH, H], F32)
            nc.vector.tensor_copy(out=pcp[:], in_=pp[:])
            nc.sync.dma_start(out=o_pool[:], in_=pcp[:])
    nc.compile()
    return nc


# ------------------------------------------------------------------- driver --

def _run(nc, in_maps):
    res = run_bass_kernel_spmd(nc, in_maps, core_ids=list(range(NCORES)),
                               trace=TRACE)
    if TRACE:
        LAST_EXEC_NS.append(res.exec_time_ns)
    return res.results


def _assemble_table(pl, shards):
    T = np.zeros((TAB, H), dtype=np.float16)
    for c in range(NCORES):
        T[pl.rowmap[c]] = shards[c]
    return T


def kernel(**inputs):
    ins = {k: np.asarray(v) for k, v in inputs.items()}
    key = hashlib.sha1(
        ins["edge_index"].tobytes() + ins["batch"].tobytes()
    ).hexdigest()
    if key not in _PLAN_CACHE:
        _PLAN_CACHE[key] = _make_plan(ins["edge_index"], ins["batch"], ins["x"])
    pl = _PLAN_CACHE[key]

    pk = pl.key
    if pk not in _PROG_CACHE:
        _PROG_CACHE[pk] = {
            "A": _build_A(pl),
            "BC": _build_BC(pl),
            "D": _build_D(pl),
        }
    progs = _PROG_CACHE[pk]

    LAST_EXEC_NS.clear()
    W1 = ins["W1"].astype(BF16_NP)
    # Launch A: T1 = dinv * (x @ W1)
    resA = _run(progs["A"], [
        {"xT": pl.cores[c]["xT"], "W": W1, "dinv": pl.cores[c]["dinv"]}
        for c in range(NCORES)
    ])
    shardsA = [r["Tout"] for r in resA]
    T1 = _assemble_table(pl, shardsA)

    def meta(c):
        cc = pl.cores[c]
        return {"idxlo": cc["idxlo"], "idxhi": cc["idxhi"],
                "dstloc": cc["dstloc"], "iota_rep": pl.iota_rep,
                "dinv": cc["dinv"]}

    def ownT(shard):
        return np.ascontiguousarray(shard.T)

    def vec(name):
        return ins[name].astype(np.float32).reshape(H, 1)

    # Launch B: layer-1 scatter + BN1/ReLU + @W2
    resB = _run(progs["BC"], [
        {**meta(c), "T": T1, "ownT": ownT(shardsA[c]), "W": ins["W2"].astype(BF16_NP),
         "bvec": vec("b1"), "bn_g": vec("bn1_g"), "bn_b": vec("bn1_b"),
         "bn_m": vec("bn1_m"), "bn_v": vec("bn1_v")} for c in range(NCORES)
    ])
    shardsB = [r["Tout"] for r in resB]
    T2 = _assemble_table(pl, shardsB)

    # Launch C: layer-2 scatter + BN2/ReLU + @W3
    resC = _run(progs["BC"], [
        {**meta(c), "T": T2, "ownT": ownT(shardsB[c]), "W": ins["W3"].astype(BF16_NP),
         "bvec": vec("b2"), "bn_g": vec("bn2_g"), "bn_b": vec("bn2_b"),
         "bn_m": vec("bn2_m"), "bn_v": vec("bn2_v")} for c in range(NCORES)
    ])
    shardsC = [r["Tout"] for r in resC]
    T3 = _assemble_table(pl, shardsC)

    # Launch D: layer-3 scatter + pooling partials
    resD = _run(progs["D"], [
        {**meta(c), "T": T3, "ownT": ownT(shardsC[c]),
         "batchval": pl.cores[c]["batchval"], "giota": pl.giota}
        for c in range(NCORES)
    ])
    pooled_sum = np.sum([r["pool"] for r in resD], axis=0).astype(np.float64)

    counts = pl.counts.astype(np.float64)
    pooled_sum += counts[:, None] * ins["b3"].astype(np.float64)[None, :]
    pooled = pooled_sum / np.maximum(counts, 1.0)[:, None]

    z = np.maximum(pooled @ ins["Wc1"].astype(np.float64)
                   + ins["bc1"].astype(np.float64), 0.0)
    out = z @ ins["Wc2"].astype(np.float64) + ins["bc2"].astype(np.float64)
    return out.astype(np.float32)


# revision 51
# speedup vs baseline: 1.6562x; 1.0054x over previous
"""Trainium2 Bass kernel for DocumentClassificationGNN (3-layer GCN + BN/ReLU +
global mean pool + MLP head), distributed over 8 NeuronCores.

Strategy (node/graph parallel, per the sharding hint):
  - Nodes are assigned to (core, slot) sorted by in-degree so every core/tile
    carries a balanced edge load.  Edges are partitioned by DESTINATION core so
    the segment-sum scatter is device-local.
  - Per layer: a dense GEMM produces a node-major fp16 feature table that the
    host replicates to all cores ("all-gather" through the host between
    launches); each core gathers its in-edge source rows with dma_gather and
    scatter-adds them into PSUM with one-hot matmuls.
  - The symmetric norm deg^-1/2[src]*deg^-1/2[dst] is SEPARABLE: table rows
    are pre-scaled by dinv[src] at write time and the scatter output is
    post-scaled by dinv[dst], so the one-hot matrices are pure 0/1 and are
    generated in batched DVE is_equal ops (2-byte fast path) with the chunk
    dim innermost: s_t[p, j, c].
  - Self-loops never enter the edge stream: each tile's own table rows are
    bulk-loaded and added via one identity matmul (contribution dinv_d*T'[d]).
  - conv bias + BN + ReLU fuse into one scalar-engine activation; GEMMs run in
    bf16; launch D does per-tile onehot(batch) pooling accumulated in one PSUM
    bank.
  - Device output: per-core pooled partial sums [64, 128].  Host: sum, +n_g*b3,
    divide by counts, tiny classifier MLP.

Programs (3 compiles, 4 launches):
  A : T1 = dinv * (x @ W1)                          -> T1 table shard
  BC: Y = scatter(T); h' = relu(BN(dinv*Y + b)); T' = dinv * (h' @ W_next)
  D : Y3 = scatter(T3); pooled_partial = onehot(batch)^T @ (dinv*Y3)
"""

import hashlib
import numpy as np
from contextlib import ExitStack

import ml_dtypes

import concourse.bass as bass
import concourse.bacc as bacc
import concourse.tile as tile
from concourse import mybir
from concourse.bass_utils import run_bass_kernel_spmd
from concourse.masks import make_identity

P = 128
NCORES = 8
N = 50000
D_IN = 256
H = 128
NGRAPH = 64
SLOTS = 6272            # 49 tiles of 128 slots per core (6250 real nodes + pad)
TILES = SLOTS // P      # 49
RAW = NCORES * SLOTS    # 50176
TAB = RAW               # table = concatenated shards, no extra rows
HIB = 4 * SLOTS         # hi-region gather base: cores 0-3 lo, cores 4-7 hi
                        # (both index ranges fit the int16 dma_gather indices)
ZLO_ROW = SLOTS - 1     # core-0 pad slot: always-zero row used by lo pads
# dst tiles per gather group: small first groups so the first tiles'
# staging lands early and the PE/consume pipeline starts ~15us sooner
GROUP_SIZES = [2, 5, 7, 7, 7, 7, 7, 7]
assert sum(GROUP_SIZES) == TILES
NGROUPS = len(GROUP_SIZES)
GROUP_T0 = [sum(GROUP_SIZES[:g]) for g in range(NGROUPS)]
BN_EPS = 1e-5

SCRATCH = 16384         # SWDGE ring: 16384/16 = 1024 descriptors per queue
MAXCH = 8               # chunks per dma_gather call (8*128 = 1024, HW limit)
NQ = 2                  # SWDGE queues (desc-gen pipelines against transfer)

F16 = mybir.dt.float16
BF16 = mybir.dt.bfloat16
F32 = mybir.dt.float32
I16 = mybir.dt.int16
BF16_NP = ml_dtypes.bfloat16

# module-level knobs / perf results (test.py pokes these)
TRACE = False
LAST_EXEC_NS = []       # per-launch exec_time_ns (when TRACE)

_PLAN_CACHE = {}
_PROG_CACHE = {}


# ---------------------------------------------------------------- host prep --

def _wrap_idx(flat):
    """dma_gather index layout: idx i -> [i%16, i//16], replicated to 128 parts."""
    n = len(flat)
    assert n % 16 == 0
    arr = np.asarray(flat, dtype=np.int16).reshape(n // 16, 16).T.copy()
    return np.tile(arr, (8, 1))


class _Plan:
    pass


def _distribute(total, bins):
    base, extra = divmod(int(total), bins)
    out = np.full(bins, base, dtype=np.int64)
    out[:extra] += 1
    return out


def _pack_core(lo, hi, kL, kH):
    """Pack one core's nodes into TILES tiles of <=128 slots, steering the
    per-tile lo/hi in-edge sums toward the shared chunk budgets kL/kH*128.

    Worst-fit decreasing on min remaining (lo, hi) headroom.
    """
    n = len(lo)
    loR = (kL * P).astype(np.float64)
    hiR = (kH * P).astype(np.float64)
    cap = np.full(TILES, P, dtype=np.int64)
    # all pad (empty) slots must be the LAST slots of the last tile: they are
    # the always-zero rows targeted by gather padding and the table-write memset
    cap[TILES - 1] = P - (TILES * P - n)
    filled = np.zeros(TILES, dtype=np.int64)
    slot = np.empty(n, dtype=np.int64)
    order = np.argsort(-(lo + hi), kind="stable")
    for i in order:
        score = np.minimum(loR - lo[i], hiR - hi[i])
        score[filled >= cap] = -np.inf
        t = int(np.argmax(score))
        loR[t] -= lo[i]
        hiR[t] -= hi[i]
        slot[i] = t * P + filled[t]
        filled[t] += 1
    return slot


def _make_plan(edge_index, batch, x):
    pl = _Plan()
    src = np.asarray(edge_index[0], dtype=np.int64)
    dst = np.asarray(edge_index[1], dtype=np.int64)
    batch = np.asarray(batch, dtype=np.int64)

    deg = np.bincount(dst, minlength=N).astype(np.int64) + 1
    dinv = (1.0 / np.sqrt(deg)).astype(np.float32)

    order = np.argsort(-deg, kind="stable")
    rank = np.empty(N, dtype=np.int64)
    rank[order] = np.arange(N)
    core_of = rank % NCORES

    # lo/hi membership of an edge depends only on its source CORE (the hi
    # gather base sits on the core-3/4 boundary), so per-node lo/hi in-edge
    # counts are fixed before slots are chosen -> bin-pack nodes into tiles
    # so per-(tile, half) counts land just under multiples of 128.
    islo_e = core_of[src] < NCORES // 2
    lo_n = np.bincount(dst[islo_e], minlength=N)
    hi_n = np.bincount(dst[~islo_e], minlength=N)
    totlo = np.zeros(NCORES, dtype=np.int64)
    tothi = np.zeros(NCORES, dtype=np.int64)
    for c in range(NCORES):
        m = core_of == c
        totlo[c] = lo_n[m].sum()
        tothi[c] = hi_n[m].sum()
    SLACK = 3
    kL = _distribute(-(-totlo.max() // P) + SLACK, TILES)
    kH = _distribute(-(-tothi.max() // P) + SLACK, TILES)
    slot_of = np.empty(N, dtype=np.int64)
    for c in range(NCORES):
        nodes = np.where(core_of == c)[0]
        slot_of[nodes] = _pack_core(lo_n[nodes], hi_n[nodes], kL, kH)
    raw_of = core_of * SLOTS + slot_of
    grow_of = raw_of                        # table row per node (pure concat)

    # real edges only: self-loops are handled by the per-tile identity matmul
    es, ed = src, dst
    ecore = core_of[ed]
    eslot = slot_of[ed]
    etile = eslot // P
    edstloc = eslot % P
    esg = grow_of[es]
    islo = islo_e

    # per-core sorted segment arrays
    NSEG = TILES * 2   # segment id: 2*tile + (0 lo / 1 hi)
    per_core = []
    seg_counts = np.zeros((NCORES, NSEG), dtype=np.int64)
    for c in range(NCORES):
        m = ecore == c
        seg = etile[m] * 2 + (~islo[m]).astype(np.int64)
        o2 = np.lexsort((esg[m], seg))
        d = {
            "seg": seg[o2],
            "dstloc": edstloc[m][o2],
            "esg": esg[m][o2],
        }
        seg_counts[c] = np.bincount(d["seg"], minlength=NSEG)
        per_core.append(d)

    # chunk plan: per tile, lo/hi chunk counts = max over cores
    CLO = np.ceil(seg_counts[:, 0::2].max(axis=0) / P).astype(int)
    CHI = np.ceil(seg_counts[:, 1::2].max(axis=0) / P).astype(int)
    # chunk order: group-major; within group: all lo chunks (tile order), then hi
    seg_chunk_start = np.zeros(NSEG, dtype=np.int64)   # global chunk idx per seg
    grp_clo = np.zeros(NGROUPS, dtype=np.int64)
    grp_chi = np.zeros(NGROUPS, dtype=np.int64)
    gcb = np.zeros(NGROUPS + 1, dtype=np.int64)
    for g in range(NGROUPS):
        ts = range(GROUP_T0[g], GROUP_T0[g] + GROUP_SIZES[g])
        grp_clo[g] = sum(CLO[t] for t in ts)
        grp_chi[g] = sum(CHI[t] for t in ts)
        ofs = gcb[g]
        for t in ts:
            seg_chunk_start[2 * t] = ofs
            ofs += CLO[t]
        for t in ts:
            seg_chunk_start[2 * t + 1] = ofs
            ofs += CHI[t]
        gcb[g + 1] = ofs
    CTOT = int(gcb[-1])

    # per-chunk default fill (pads): lo chunks -> ZLO, hi chunks -> absolute hi zero
    chunk_is_hi = np.zeros(CTOT, dtype=bool)
    for t in range(TILES):
        s = seg_chunk_start[2 * t + 1]
        chunk_is_hi[s:s + CHI[t]] = True

    pl.cores = []
    for c in range(NCORES):
        d = per_core[c]
        npad = CTOT * P
        dstloc_pad = np.zeros(npad, dtype=np.float16)
        row_pad = np.where(np.repeat(chunk_is_hi, P), RAW - 1,
                           ZLO_ROW).astype(np.int64)
        # position of each real edge
        cnt = seg_counts[c]
        seg_first = np.concatenate([[0], np.cumsum(cnt)[:-1]])
        within = np.arange(len(d["seg"])) - seg_first[d["seg"]]
        pos = seg_chunk_start[d["seg"]] * P + within
        dstloc_pad[pos] = d["dstloc"].astype(np.float16)
        row_pad[pos] = d["esg"]

        # gather index arrays (lo then hi, group-major)
        lo_parts, hi_parts = [], []
        for g in range(NGROUPS):
            a = gcb[g] * P
            b = a + grp_clo[g] * P
            e = gcb[g + 1] * P
            lo_parts.append(row_pad[a:b])
            hi_parts.append(row_pad[b:e] - HIB)
        lo_flat = np.concatenate(lo_parts)
        hi_flat = np.concatenate(hi_parts)
        assert lo_flat.min() >= 0 and lo_flat.max() < HIB <= 32768
        assert hi_flat.min() >= 0 and hi_flat.max() <= RAW - 1 - HIB <= 32767

        core = {
            "idxlo": _wrap_idx(lo_flat),
            "idxhi": _wrap_idx(hi_flat),
            "dstloc": dstloc_pad.reshape(CTOT, P).T.copy(),
        }
        pl.cores.append(core)

    # group gather call metadata (columns into wrapped idx tensors)
    pl.lo_cols = int(grp_clo.sum() * P // 16)
    pl.hi_cols = int(grp_chi.sum() * P // 16)
    lo_c0 = np.concatenate([[0], np.cumsum(grp_clo * 8)])
    hi_c0 = np.concatenate([[0], np.cumsum(grp_chi * 8)])
    pl.groups = []
    for g in range(NGROUPS):
        tiles = []
        for t in range(GROUP_T0[g], GROUP_T0[g] + GROUP_SIZES[g]):
            lo_local = int(seg_chunk_start[2 * t] - gcb[g])
            hi_local = int(seg_chunk_start[2 * t + 1] - gcb[g])
            tiles.append({
                "clo": int(CLO[t]), "chi": int(CHI[t]),
                "sp_lo": lo_local, "sp_hi": hi_local,
                "gc_lo": int(seg_chunk_start[2 * t]),
                "gc_hi": int(seg_chunk_start[2 * t + 1]),
            })
        pl.groups.append({
            "nclo": int(grp_clo[g]), "nchi": int(grp_chi[g]),
            "lo_col0": int(lo_c0[g]), "hi_col0": int(hi_c0[g]),
            "tiles": tiles,
        })
    pl.CTOT = CTOT
    pl.NCHMAX = int(max(CLO.max(), CHI.max()))

    # slot -> node map, batch values, dinv per slot, xT shards, table row map
    node_at = np.full((NCORES, SLOTS), -1, dtype=np.int64)
    node_at[core_of, slot_of] = np.arange(N)
    bv = np.full((NCORES, SLOTS), 99.0, dtype=np.float16)
    dv = np.zeros((NCORES, SLOTS), dtype=np.float32)   # pad slots: dinv = 0
    valid = node_at >= 0
    bv[valid] = batch[node_at[valid]].astype(np.float16)
    dv[valid] = dinv[node_at[valid]]
    for c in range(NCORES):
        pl.cores[c]["batchval"] = bv[c].reshape(TILES, P).T.copy()  # [128, 49]
        pl.cores[c]["dinv"] = dv[c].reshape(TILES, P).T.copy()      # [128, 49]
        xt = np.zeros((D_IN, SLOTS), dtype=np.float32)
        v = valid[c]
        xt[:, v] = np.asarray(x, dtype=np.float32)[node_at[c][v]].T
        pl.cores[c]["xT"] = xt.astype(BF16_NP)

    pl.rowmap = np.arange(RAW, dtype=np.int64).reshape(NCORES, SLOTS)
    pl.counts = np.bincount(batch, minlength=NGRAPH).astype(np.float32)
    pl.iota_rep = np.repeat(np.arange(P), pl.NCHMAX).astype(np.float16).reshape(1, -1)
    pl.giota = np.repeat(np.arange(NGRAPH), TILES).astype(np.float16).reshape(1, -1)
    pl.key = (tuple(CLO), tuple(CHI))
    return pl


# ---------------------------------------------------------- program builders --

def _make_gemm_emitter(nc, ctx, tc, k_tiles_fn, o_T, dinv_sb, bufs=2,
                       to_bufs=None):
    """Returns emit(t): table rows for slot tile t.

    out[slot, fout] = sum_k lhsT_k^T @ rhs_k with lhsT = feat-major input
    block (no output transpose needed); dinv-scale + fp16 cast -> o_T rows.
    """
    gps_pool = ctx.enter_context(
        tc.tile_pool(name="gemm_ps", bufs=bufs, space="PSUM"))
    to_pool = ctx.enter_context(
        tc.tile_pool(name="gemm_to", bufs=to_bufs or 2))
    WB = 4  # tiles per table-write DMA (amortizes the 625ns HWDGE slot)
    state = {}

    def emit(t):
        kt = k_tiles_fn(t)
        gps = gps_pool.tile([P, H], F32, space="PSUM")
        for ki, (lhsT, rhs) in enumerate(kt):
            nc.tensor.matmul(out=gps[:], lhsT=lhsT, rhs=rhs,
                             start=(ki == 0), stop=(ki == len(kt) - 1))
        j = t % WB
        if j == 0:
            to_new = to_pool.tile([P, WB, H], F16, tag="to")
            state["to"] = to_new
        to = state["to"]
        # pad slots have dinv == 0, so this scale also keeps their table rows
        # ZERO (they serve as the gather targets for chunk padding positions)
        nc.scalar.activation(out=to[:, j, :], in_=gps[:],
                             func=mybir.ActivationFunctionType.Copy,
                             scale=dinv_sb[:, t:t + 1])
        if j == WB - 1 or t == TILES - 1:
            t0, n = t - j, j + 1
            dst = o_T[t0 * P:(t0 + n) * P, :].rearrange(
                "(j p) h -> p j h", j=n, p=P)
            nc.sync.dma_start(out=dst, in_=to[:, 0:n, :])

    return emit


def _build_A(pl):
    nc = bacc.Bacc("TRN2", target_bir_lowering=False, debug=False, num_devices=NCORES)
    i_xT = nc.dram_tensor("xT", [D_IN, SLOTS], BF16, kind="ExternalInput").ap()
    i_W = nc.dram_tensor("W", [D_IN, H], BF16, kind="ExternalInput").ap()
    i_dinv = nc.dram_tensor("dinv", [P, TILES], F32, kind="ExternalInput").ap()
    o_T = nc.dram_tensor("Tout", [SLOTS, H], F16, kind="ExternalOutput").ap()
    with tile.TileContext(nc) as tc:
        with ExitStack() as ctx:
            const = ctx.enter_context(tc.tile_pool(name="const", bufs=1))
            dinv_sb = const.tile([P, TILES], F32)
            nc.sync.dma_start(out=dinv_sb[:], in_=i_dinv[:])
            w0 = const.tile([P, H], BF16)
            nc.sync.dma_start(out=w0[:], in_=i_W[0:P, :])
            w1 = const.tile([P, H], BF16)
            nc.sync.dma_start(out=w1[:], in_=i_W[P:2 * P, :])
            x0 = const.tile([P, SLOTS], BF16)
            x1 = const.tile([P, SLOTS], BF16)
            XCH = 784   # SLOTS/8: early chunks unblock the first tiles' GEMMs
            for o in range(0, SLOTS, XCH):
                w = min(XCH, SLOTS - o)
                nc.sync.dma_start(out=x0[:, o:o + w], in_=i_xT[0:P, o:o + w])
                nc.sync.dma_start(out=x1[:, o:o + w], in_=i_xT[P:2 * P, o:o + w])

            def k_tiles(t):
                sl = slice(t * P, (t + 1) * P)
                return [(x0[:, sl], w0[:]), (x1[:, sl], w1[:])]

            emit = _make_gemm_emitter(nc, ctx, tc, k_tiles, o_T, dinv_sb, bufs=6,
                                      to_bufs=13)
            for t in range(TILES):
                emit(t)
    nc.compile()
    return nc


def _scatter_body(nc, ctx, tc, pl, i_T, consume_tile, after_tile=None,
                  mid_loads=None):
    """Shared gather + one-hot matmul scatter loop.

    consume_tile(t, ypsum) handles the per-tile PSUM result
    (ypsum = sum over in-edges of dinv[src]-scaled source rows, incl self-loop).
    """
    const = ctx.enter_context(tc.tile_pool(name="sc_const", bufs=1))
    stage = ctx.enter_context(tc.tile_pool(name="staging", bufs=2))
    st_pool = ctx.enter_context(tc.tile_pool(name="st", bufs=4))
    yp_pool = ctx.enter_context(tc.tile_pool(name="yps", bufs=3, space="PSUM"))

    i_idxlo = nc.dram_tensor("idxlo", [P, pl.lo_cols], I16, kind="ExternalInput").ap()
    i_idxhi = nc.dram_tensor("idxhi", [P, pl.hi_cols], I16, kind="ExternalInput").ap()
    i_dstloc = nc.dram_tensor("dstloc", [P, pl.CTOT], F16, kind="ExternalInput").ap()
    i_iota = nc.dram_tensor("iota_rep", [1, P * pl.NCHMAX], F16,
                            kind="ExternalInput").ap()
    i_ownT = nc.dram_tensor("ownT", [H, SLOTS], F16, kind="ExternalInput").ap()

    idxlo_sb = const.tile([P, pl.lo_cols], I16)
    nc.sync.dma_start(out=idxlo_sb[:], in_=i_idxlo[:])
    idxhi_sb = const.tile([P, pl.hi_cols], I16)
    nc.sync.dma_start(out=idxhi_sb[:], in_=i_idxhi[:])
    dstloc_sb = const.tile([P, pl.CTOT], F16)
    nc.sync.dma_start(out=dstloc_sb[:], in_=i_dstloc[:])
    iota_sb = const.tile([P, P * pl.NCHMAX], F16)
    nc.sync.dma_start(out=iota_sb[:], in_=i_iota.to_broadcast([P, P * pl.NCHMAX]))
    iota3 = iota_sb[:].rearrange("p (j c) -> p j c", j=P, c=pl.NCHMAX)
    identH = const.tile([P, P], F16)
    make_identity(nc, identH[:])
    ownT_sb = const.tile([P, SLOTS], F16)
    nc.sync.dma_start(out=ownT_sb[:], in_=i_ownT[:])
    if mid_loads is not None:
        # non-scatter-critical input loads go AFTER the idx/ownT loads so the
        # first gather is not stuck behind their fixed HWDGE slots
        mid_loads()

    qn = [0]

    def gather(staging, base, src_ap, idx_sb, col0, nch):
        for o in range(0, nch, MAXCH):
            n = min(MAXCH, nch - o)
            c0 = col0 + o * 8
            nc.gpsimd.dma_gather(
                out_ap=staging[:, base + o:base + o + n, :], in_ap=src_ap,
                idxs_ap=idx_sb[:, c0:c0 + n * 8],
                num_idxs=n * P, num_idxs_reg=n * P, elem_size=H,
                queue_num=qn[0])
            qn[0] = (qn[0] + 1) % NQ

    def onehot(gc0, nch):
        st = st_pool.tile([P, P, nch], F16, tag="st")
        nc.vector.tensor_tensor(
            out=st[:],
            in0=iota3[:, :, 0:nch],
            in1=dstloc_sb[:, gc0:gc0 + nch].unsqueeze(1).to_broadcast([P, P, nch]),
            op=mybir.AluOpType.is_equal)
        return st

    for g, grp in enumerate(pl.groups):
        nclo, nchi = grp["nclo"], grp["nchi"]
        staging = stage.tile([P, nclo + nchi, H], F16, tag="staging")
        gather(staging, 0, i_T[:], idxlo_sb, grp["lo_col0"], nclo)
        gather(staging, nclo, i_T[HIB:, :], idxhi_sb, grp["hi_col0"], nchi)
        for ti, td in enumerate(grp["tiles"]):
            t = GROUP_T0[g] + ti
            stlo = onehot(td["gc_lo"], td["clo"]) if td["clo"] else None
            sthi = onehot(td["gc_hi"], td["chi"]) if td["chi"] else None
            ypsum = yp_pool.tile([P, H], F32, space="PSUM")
            # self-loop rows: ypsum = ownT_tile^T @ I  (= own rows, [slot, feat])
            nc.tensor.matmul(out=ypsum[:], lhsT=ownT_sb[:, t * P:(t + 1) * P],
                             rhs=identH[:],
                             start=True, stop=(td["clo"] + td["chi"] == 0))
            for i in range(td["clo"]):
                nc.tensor.matmul(
                    out=ypsum[:], lhsT=stlo[:, :, i],
                    rhs=staging[:, td["sp_lo"] + i, :],
                    start=False,
                    stop=(i == td["clo"] - 1 and td["chi"] == 0))
            for i in range(td["chi"]):
                nc.tensor.matmul(
                    out=ypsum[:], lhsT=sthi[:, :, i],
                    rhs=staging[:, td["sp_hi"] + i, :],
                    start=False, stop=(i == td["chi"] - 1))
            consume_tile(t, ypsum)
            if after_tile is not None:
                after_tile(t)


def _vec_input(nc, const, name):
    ap = nc.dram_tensor(name, [H, 1], F32, kind="ExternalInput").ap()
    sb = const.tile([H, 1], F32, tag=f"vec_{name}")
    nc.sync.dma_start(out=sb[:], in_=ap[:])
    return sb


def _build_BC(pl):
    nc = bacc.Bacc("TRN2", target_bir_lowering=False, debug=False,
                   num_devices=NCORES, dynamic_dma_scratch_size=SCRATCH,
                   num_swdge_queues=NQ)
    i_T = nc.dram_tensor("T", [TAB, H], F16, kind="ExternalInput").ap()
    i_W = nc.dram_tensor("W", [H, H], BF16, kind="ExternalInput").ap()
    i_dinv = nc.dram_tensor("dinv", [P, TILES], F32, kind="ExternalInput").ap()
    o_T = nc.dram_tensor("Tout", [SLOTS, H], F16, kind="ExternalOutput").ap()
    with tile.TileContext(nc) as tc:
        with ExitStack() as ctx:
            const = ctx.enter_context(tc.tile_pool(name="bc_const", bufs=1))
            ycp_pool = ctx.enter_context(tc.tile_pool(name="ycp", bufs=3))
            h_pool = ctx.enter_context(tc.tile_pool(name="ht", bufs=3))
            tps_pool = ctx.enter_context(tc.tile_pool(name="tps", bufs=2, space="PSUM"))

            identB = const.tile([P, P], BF16)
            make_identity(nc, identB[:])
            dinv_sb = const.tile([P, TILES], F32)
            w_sb = const.tile([H, H], BF16)
            scale = const.tile([H, 1], F32)
            bias = const.tile([H, 1], F32)

            def mid_loads():
                b_sb = _vec_input(nc, const, "bvec")
                g_sb = _vec_input(nc, const, "bn_g")
                bb_sb = _vec_input(nc, const, "bn_b")
                m_sb = _vec_input(nc, const, "bn_m")
                v_sb = _vec_input(nc, const, "bn_v")
                nc.sync.dma_start(out=dinv_sb[:], in_=i_dinv[:])
                nc.sync.dma_start(out=w_sb[:], in_=i_W[:])
                # scale = g / sqrt(v+eps); bias = (b - m)*scale + beta
                eps = const.tile([H, 1], F32)
                nc.vector.memset(eps[:], BN_EPS)
                sq = const.tile([H, 1], F32)
                nc.scalar.activation(out=sq[:], in_=v_sb[:],
                                     func=mybir.ActivationFunctionType.Sqrt,
                                     bias=eps[:], scale=1.0)
                rs = const.tile([H, 1], F32)
                nc.vector.reciprocal(out=rs[:], in_=sq[:])
                nc.vector.tensor_mul(out=scale[:], in0=rs[:], in1=g_sb[:])
                nc.vector.tensor_sub(out=bias[:], in0=b_sb[:], in1=m_sb[:])
                nc.vector.tensor_mul(out=bias[:], in0=bias[:], in1=scale[:])
                nc.vector.tensor_add(out=bias[:], in0=bias[:], in1=bb_sb[:])

            h_tiles = {}

            def consume(t, ypsum):
                ycp = ycp_pool.tile([P, H], BF16)
                nc.scalar.activation(out=ycp[:], in_=ypsum[:],
                                     func=mybir.ActivationFunctionType.Copy,
                                     scale=dinv_sb[:, t:t + 1])
                tp = tps_pool.tile([P, P], BF16, space="PSUM")
                nc.tensor.transpose(out=tp[:], in_=ycp[:], identity=identB[:])
                h_t = h_pool.tile([P, H], BF16)
                nc.scalar.activation(
                    out=h_t[:], in_=tp[:],
                    func=mybir.ActivationFunctionType.Relu,
                    bias=bias[:], scale=scale[:])
                h_tiles[t] = h_t

            emit = _make_gemm_emitter(nc, ctx, tc,
                                      lambda t: [(h_tiles.pop(t)[:], w_sb[:])],
                                      o_T, dinv_sb, to_bufs=6)

            # emit each tile's GEMM right after its scatter completes so the
            # table write overlaps the remaining scatter instead of tailing it
            _scatter_body(nc, ctx, tc, pl, i_T, consume, emit,
                          mid_loads=mid_loads)
    nc.compile()
    return nc


def _build_D(pl):
    nc = bacc.Bacc("TRN2", target_bir_lowering=False, debug=False,
                   num_devices=NCORES, dynamic_dma_scratch_size=SCRATCH,
                   num_swdge_queues=NQ)
    i_T = nc.dram_tensor("T", [TAB, H], F16, kind="ExternalInput").ap()
    i_bv = nc.dram_tensor("batchval", [P, TILES], F16, kind="ExternalInput").ap()
    i_gi = nc.dram_tensor("giota", [1, NGRAPH * TILES], F16,
                          kind="ExternalInput").ap()
    i_dinv = nc.dram_tensor("dinv", [P, TILES], F32, kind="ExternalInput").ap()
    o_pool = nc.dram_tensor("pool", [NGRAPH, H], F32, kind="ExternalOutput").ap()
    with tile.TileContext(nc) as tc:
        with ExitStack() as ctx:
            const = ctx.enter_context(tc.tile_pool(name="d_const", bufs=1))
            h3_pool = ctx.enter_context(tc.tile_pool(name="h3", bufs=3))
            pp_pool = ctx.enter_context(tc.tile_pool(name="pp", bufs=1, space="PSUM"))

            bv_sb = const.tile([P, TILES], F16)
            gi_sb = const.tile([P, NGRAPH * TILES], F16)
            dinv_sb = const.tile([P, TILES], F32)
            oh_all = const.tile([P, NGRAPH, TILES], F16)
            pp = pp_pool.tile([NGRAPH, H], F32, space="PSUM")

            def mid_loads():
                nc.sync.dma_start(out=bv_sb[:], in_=i_bv[:])
                nc.sync.dma_start(out=gi_sb[:],
                                  in_=i_gi.to_broadcast([P, NGRAPH * TILES]))
                nc.sync.dma_start(out=dinv_sb[:], in_=i_dinv[:])
                # oh_all[p, g, t] = (batchval[p, t] == g)
                nc.vector.tensor_tensor(
                    out=oh_all[:],
                    in0=gi_sb[:].rearrange("p (g t) -> p g t", g=NGRAPH, t=TILES),
                    in1=bv_sb[:].unsqueeze(1).to_broadcast([P, NGRAPH, TILES]),
                    op=mybir.AluOpType.is_equal)

            def consume(t, ypsum):
                h3 = h3_pool.tile([P, H], F16)
                nc.scalar.activation(out=h3[:], in_=ypsum[:],
                                     func=mybir.ActivationFunctionType.Copy,
                                     scale=dinv_sb[:, t:t + 1])
                nc.tensor.matmul(out=pp[:], lhsT=oh_all[:, :, t], rhs=h3[:],
                                 start=(t == 0), stop=(t == TILES - 1))

            _scatter_body(nc, ctx, tc, pl, i_T, consume,
                          mid_loads=mid_loads)
            pcp = const.tile([NGRAPH, H], F32)
            nc.vector.tensor_copy(out=pcp[:], in_=pp[:])
            nc.sync.dma_start(out=o_pool[:], in_=pcp[:])
    nc.compile()
    return nc


# ------------------------------------------------------------------- driver --

def _run(nc, in_maps):
    res = run_bass_kernel_spmd(nc, in_maps, core_ids=list(range(NCORES)),
                               trace=TRACE)
    if TRACE:
        LAST_EXEC_NS.append(res.exec_time_ns)
    return res.results


def _assemble_table(pl, shards):
    T = np.zeros((TAB, H), dtype=np.float16)
    for c in range(NCORES):
        T[pl.rowmap[c]] = shards[c]
    return T


def kernel(**inputs):
    ins = {k: np.asarray(v) for k, v in inputs.items()}
    key = hashlib.sha1(
        ins["edge_index"].tobytes() + ins["batch"].tobytes()
    ).hexdigest()
    if key not in _PLAN_CACHE:
        _PLAN_CACHE[key] = _make_plan(ins["edge_index"], ins["batch"], ins["x"])
    pl = _PLAN_CACHE[key]

    pk = pl.key
    if pk not in _PROG_CACHE:
        _PROG_CACHE[pk] = {
            "A": _build_A(pl),
            "BC": _build_BC(pl),
            "D": _build_D(pl),
        }
    progs = _PROG_CACHE[pk]

    LAST_EXEC_NS.clear()
    W1 = ins["W1"].astype(BF16_NP)
    # Launch A: T1 = dinv * (x @ W1)
    resA = _run(progs["A"], [
        {"xT": pl.cores[c]["xT"], "W": W1, "dinv": pl.cores[c]["dinv"]}
        for c in range(NCORES)
    ])
    shardsA = [r["Tout"] for r in resA]
    T1 = _assemble_table(pl, shardsA)

    def meta(c):
        cc = pl.cores[c]
        return {"idxlo": cc["idxlo"], "idxhi": cc["idxhi"],
                "dstloc": cc["dstloc"], "iota_rep": pl.iota_rep,
                "dinv": cc["dinv"]}

    def ownT(shard):
        return np.ascontiguousarray(shard.T)

    def vec(name):
        return ins[name].astype(np.float32).reshape(H, 1)

    # Launch B: layer-1 scatter + BN1/ReLU + @W2
    resB = _run(progs["BC"], [
        {**meta(c), "T": T1, "ownT": ownT(shardsA[c]), "W": ins["W2"].astype(BF16_NP),
         "bvec": vec("b1"), "bn_g": vec("bn1_g"), "bn_b": vec("bn1_b"),
         "bn_m": vec("bn1_m"), "bn_v": vec("bn1_v")} for c in range(NCORES)
    ])
    shardsB = [r["Tout"] for r in resB]
    T2 = _assemble_table(pl, shardsB)

    # Launch C: layer-2 scatter + BN2/ReLU + @W3
    resC = _run(progs["BC"], [
        {**meta(c), "T": T2, "ownT": ownT(shardsB[c]), "W": ins["W3"].astype(BF16_NP),
         "bvec": vec("b2"), "bn_g": vec("bn2_g"), "bn_b": vec("bn2_b"),
         "bn_m": vec("bn2_m"), "bn_v": vec("bn2_v")} for c in range(NCORES)
    ])
    shardsC = [r["Tout"] for r in resC]
    T3 = _assemble_table(pl, shardsC)

    # Launch D: layer-3 scatter + pooling partials
    resD = _run(progs["D"], [
        {**meta(c), "T": T3, "ownT": ownT(shardsC[c]),
         "batchval": pl.cores[c]["batchval"], "giota": pl.giota}
        for c in range(NCORES)
    ])
    pooled_sum = np.sum([r["pool"] for r in resD], axis=0).astype(np.float64)

    counts = pl.counts.astype(np.float64)
    pooled_sum += counts[:, None] * ins["b3"].astype(np.float64)[None, :]
    pooled = pooled_sum / np.maximum(counts, 1.0)[:, None]

    z = np.maximum(pooled @ ins["Wc1"].astype(np.float64)
                   + ins["bc1"].astype(np.float64), 0.0)
    out = z @ ins["Wc2"].astype(np.float64) + ins["bc2"].astype(np.float64)
    return out.astype(np.float32)


# revision 52
# speedup vs baseline: 1.6625x; 1.0038x over previous
"""Trainium2 Bass kernel for DocumentClassificationGNN (3-layer GCN + BN/ReLU +
global mean pool + MLP head), distributed over 8 NeuronCores.

Strategy (node/graph parallel, per the sharding hint):
  - Nodes are assigned to (core, slot) sorted by in-degree so every core/tile
    carries a balanced edge load.  Edges are partitioned by DESTINATION core so
    the segment-sum scatter is device-local.
  - Per layer: a dense GEMM produces a node-major fp16 feature table that the
    host replicates to all cores ("all-gather" through the host between
    launches); each core gathers its in-edge source rows with dma_gather and
    scatter-adds them into PSUM with one-hot matmuls.
  - The symmetric norm deg^-1/2[src]*deg^-1/2[dst] is SEPARABLE: table rows
    are pre-scaled by dinv[src] at write time and the scatter output is
    post-scaled by dinv[dst], so the one-hot matrices are pure 0/1 and are
    generated in batched DVE is_equal ops (2-byte fast path) with the chunk
    dim innermost: s_t[p, j, c].
  - Self-loops never enter the edge stream: each tile's own table rows are
    bulk-loaded and added via one identity matmul (contribution dinv_d*T'[d]).
  - conv bias + BN + ReLU fuse into one scalar-engine activation; GEMMs run in
    bf16; launch D does per-tile onehot(batch) pooling accumulated in one PSUM
    bank.
  - Device output: per-core pooled partial sums [64, 128].  Host: sum, +n_g*b3,
    divide by counts, tiny classifier MLP.

Programs (3 compiles, 4 launches):
  A : T1 = dinv * (x @ W1)                          -> T1 table shard
  BC: Y = scatter(T); h' = relu(BN(dinv*Y + b)); T' = dinv * (h' @ W_next)
  D : Y3 = scatter(T3); pooled_partial = onehot(batch)^T @ (dinv*Y3)
"""

import hashlib
import numpy as np
from contextlib import ExitStack

import ml_dtypes

import concourse.bass as bass
import concourse.bacc as bacc
import concourse.tile as tile
from concourse import mybir
from concourse.bass_utils import run_bass_kernel_spmd
from concourse.masks import make_identity

P = 128
NCORES = 8
N = 50000
D_IN = 256
H = 128
NGRAPH = 64
SLOTS = 6272            # 49 tiles of 128 slots per core (6250 real nodes + pad)
TILES = SLOTS // P      # 49
RAW = NCORES * SLOTS    # 50176
TAB = RAW               # table = concatenated shards, no extra rows
HIB = 4 * SLOTS         # hi-region gather base: cores 0-3 lo, cores 4-7 hi
                        # (both index ranges fit the int16 dma_gather indices)
ZLO_ROW = SLOTS - 1     # core-0 pad slot: always-zero row used by lo pads
# dst tiles per gather group: small first groups so the first tiles'
# staging lands early and the PE/consume pipeline starts ~15us sooner
GROUP_SIZES = [2, 5, 7, 7, 7, 7, 7, 7]
assert sum(GROUP_SIZES) == TILES
NGROUPS = len(GROUP_SIZES)
GROUP_T0 = [sum(GROUP_SIZES[:g]) for g in range(NGROUPS)]
BN_EPS = 1e-5

SCRATCH = 16384         # SWDGE ring: 16384/16 = 1024 descriptors per queue
MAXCH = 8               # chunks per dma_gather call (8*128 = 1024, HW limit)
NQ = 2                  # SWDGE queues (desc-gen pipelines against transfer)

F16 = mybir.dt.float16
BF16 = mybir.dt.bfloat16
F32 = mybir.dt.float32
I16 = mybir.dt.int16
BF16_NP = ml_dtypes.bfloat16

# module-level knobs / perf results (test.py pokes these)
TRACE = False
LAST_EXEC_NS = []       # per-launch exec_time_ns (when TRACE)

_PLAN_CACHE = {}
_PROG_CACHE = {}


# ---------------------------------------------------------------- host prep --

def _wrap_idx(flat):
    """dma_gather index layout: idx i -> [i%16, i//16], replicated to 128 parts."""
    n = len(flat)
    assert n % 16 == 0
    arr = np.asarray(flat, dtype=np.int16).reshape(n // 16, 16).T.copy()
    return np.tile(arr, (8, 1))


class _Plan:
    pass


def _distribute(total, bins):
    base, extra = divmod(int(total), bins)
    out = np.full(bins, base, dtype=np.int64)
    out[:extra] += 1
    return out


def _pack_core(lo, hi, kL, kH):
    """Pack one core's nodes into TILES tiles of <=128 slots, steering the
    per-tile lo/hi in-edge sums toward the shared chunk budgets kL/kH*128.

    Worst-fit decreasing on min remaining (lo, hi) headroom.
    """
    n = len(lo)
    loR = (kL * P).astype(np.float64)
    hiR = (kH * P).astype(np.float64)
    cap = np.full(TILES, P, dtype=np.int64)
    # all pad (empty) slots must be the LAST slots of the last tile: they are
    # the always-zero rows targeted by gather padding and the table-write memset
    cap[TILES - 1] = P - (TILES * P - n)
    filled = np.zeros(TILES, dtype=np.int64)
    slot = np.empty(n, dtype=np.int64)
    order = np.argsort(-(lo + hi), kind="stable")
    for i in order:
        score = np.minimum(loR - lo[i], hiR - hi[i])
        score[filled >= cap] = -np.inf
        t = int(np.argmax(score))
        loR[t] -= lo[i]
        hiR[t] -= hi[i]
        slot[i] = t * P + filled[t]
        filled[t] += 1
    return slot


def _make_plan(edge_index, batch, x):
    pl = _Plan()
    src = np.asarray(edge_index[0], dtype=np.int64)
    dst = np.asarray(edge_index[1], dtype=np.int64)
    batch = np.asarray(batch, dtype=np.int64)

    deg = np.bincount(dst, minlength=N).astype(np.int64) + 1
    dinv = (1.0 / np.sqrt(deg)).astype(np.float32)

    order = np.argsort(-deg, kind="stable")
    rank = np.empty(N, dtype=np.int64)
    rank[order] = np.arange(N)
    core_of = rank % NCORES

    # lo/hi membership of an edge depends only on its source CORE (the hi
    # gather base sits on the core-3/4 boundary), so per-node lo/hi in-edge
    # counts are fixed before slots are chosen -> bin-pack nodes into tiles
    # so per-(tile, half) counts land just under multiples of 128.
    islo_e = core_of[src] < NCORES // 2
    lo_n = np.bincount(dst[islo_e], minlength=N)
    hi_n = np.bincount(dst[~islo_e], minlength=N)
    totlo = np.zeros(NCORES, dtype=np.int64)
    tothi = np.zeros(NCORES, dtype=np.int64)
    for c in range(NCORES):
        m = core_of == c
        totlo[c] = lo_n[m].sum()
        tothi[c] = hi_n[m].sum()
    SLACK = 3
    kL = _distribute(-(-totlo.max() // P) + SLACK, TILES)
    kH = _distribute(-(-tothi.max() // P) + SLACK, TILES)
    slot_of = np.empty(N, dtype=np.int64)
    for c in range(NCORES):
        nodes = np.where(core_of == c)[0]
        slot_of[nodes] = _pack_core(lo_n[nodes], hi_n[nodes], kL, kH)
    raw_of = core_of * SLOTS + slot_of
    grow_of = raw_of                        # table row per node (pure concat)

    # real edges only: self-loops are handled by the per-tile identity matmul
    es, ed = src, dst
    ecore = core_of[ed]
    eslot = slot_of[ed]
    etile = eslot // P
    edstloc = eslot % P
    esg = grow_of[es]
    islo = islo_e

    # per-core sorted segment arrays
    NSEG = TILES * 2   # segment id: 2*tile + (0 lo / 1 hi)
    per_core = []
    seg_counts = np.zeros((NCORES, NSEG), dtype=np.int64)
    for c in range(NCORES):
        m = ecore == c
        seg = etile[m] * 2 + (~islo[m]).astype(np.int64)
        o2 = np.lexsort((esg[m], seg))
        d = {
            "seg": seg[o2],
            "dstloc": edstloc[m][o2],
            "esg": esg[m][o2],
        }
        seg_counts[c] = np.bincount(d["seg"], minlength=NSEG)
        per_core.append(d)

    # chunk plan: per tile, lo/hi chunk counts = max over cores
    CLO = np.ceil(seg_counts[:, 0::2].max(axis=0) / P).astype(int)
    CHI = np.ceil(seg_counts[:, 1::2].max(axis=0) / P).astype(int)
    # chunk order: group-major; within group: all lo chunks (tile order), then hi
    seg_chunk_start = np.zeros(NSEG, dtype=np.int64)   # global chunk idx per seg
    grp_clo = np.zeros(NGROUPS, dtype=np.int64)
    grp_chi = np.zeros(NGROUPS, dtype=np.int64)
    gcb = np.zeros(NGROUPS + 1, dtype=np.int64)
    for g in range(NGROUPS):
        ts = range(GROUP_T0[g], GROUP_T0[g] + GROUP_SIZES[g])
        grp_clo[g] = sum(CLO[t] for t in ts)
        grp_chi[g] = sum(CHI[t] for t in ts)
        ofs = gcb[g]
        for t in ts:
            seg_chunk_start[2 * t] = ofs
            ofs += CLO[t]
        for t in ts:
            seg_chunk_start[2 * t + 1] = ofs
            ofs += CHI[t]
        gcb[g + 1] = ofs
    CTOT = int(gcb[-1])

    # per-chunk default fill (pads): lo chunks -> ZLO, hi chunks -> absolute hi zero
    chunk_is_hi = np.zeros(CTOT, dtype=bool)
    for t in range(TILES):
        s = seg_chunk_start[2 * t + 1]
        chunk_is_hi[s:s + CHI[t]] = True

    pl.cores = []
    for c in range(NCORES):
        d = per_core[c]
        npad = CTOT * P
        dstloc_pad = np.zeros(npad, dtype=np.float16)
        row_pad = np.where(np.repeat(chunk_is_hi, P), RAW - 1,
                           ZLO_ROW).astype(np.int64)
        # position of each real edge
        cnt = seg_counts[c]
        seg_first = np.concatenate([[0], np.cumsum(cnt)[:-1]])
        within = np.arange(len(d["seg"])) - seg_first[d["seg"]]
        pos = seg_chunk_start[d["seg"]] * P + within
        dstloc_pad[pos] = d["dstloc"].astype(np.float16)
        row_pad[pos] = d["esg"]

        # gather index arrays (lo then hi, group-major)
        lo_parts, hi_parts = [], []
        for g in range(NGROUPS):
            a = gcb[g] * P
            b = a + grp_clo[g] * P
            e = gcb[g + 1] * P
            lo_parts.append(row_pad[a:b])
            hi_parts.append(row_pad[b:e] - HIB)
        lo_flat = np.concatenate(lo_parts)
        hi_flat = np.concatenate(hi_parts)
        assert lo_flat.min() >= 0 and lo_flat.max() < HIB <= 32768
        assert hi_flat.min() >= 0 and hi_flat.max() <= RAW - 1 - HIB <= 32767

        core = {
            "idxlo": _wrap_idx(lo_flat),
            "idxhi": _wrap_idx(hi_flat),
            "dstloc": dstloc_pad.reshape(CTOT, P).T.copy(),
        }
        pl.cores.append(core)

    # group gather call metadata (columns into wrapped idx tensors)
    pl.lo_cols = int(grp_clo.sum() * P // 16)
    pl.hi_cols = int(grp_chi.sum() * P // 16)
    lo_c0 = np.concatenate([[0], np.cumsum(grp_clo * 8)])
    hi_c0 = np.concatenate([[0], np.cumsum(grp_chi * 8)])
    pl.groups = []
    for g in range(NGROUPS):
        tiles = []
        for t in range(GROUP_T0[g], GROUP_T0[g] + GROUP_SIZES[g]):
            lo_local = int(seg_chunk_start[2 * t] - gcb[g])
            hi_local = int(seg_chunk_start[2 * t + 1] - gcb[g])
            tiles.append({
                "clo": int(CLO[t]), "chi": int(CHI[t]),
                "sp_lo": lo_local, "sp_hi": hi_local,
                "gc_lo": int(seg_chunk_start[2 * t]),
                "gc_hi": int(seg_chunk_start[2 * t + 1]),
            })
        pl.groups.append({
            "nclo": int(grp_clo[g]), "nchi": int(grp_chi[g]),
            "lo_col0": int(lo_c0[g]), "hi_col0": int(hi_c0[g]),
            "tiles": tiles,
        })
    pl.CTOT = CTOT
    pl.NCHMAX = int(max(CLO.max(), CHI.max()))

    # slot -> node map, batch values, dinv per slot, xT shards, table row map
    node_at = np.full((NCORES, SLOTS), -1, dtype=np.int64)
    node_at[core_of, slot_of] = np.arange(N)
    bv = np.full((NCORES, SLOTS), 99.0, dtype=np.float16)
    dv = np.zeros((NCORES, SLOTS), dtype=np.float32)   # pad slots: dinv = 0
    valid = node_at >= 0
    bv[valid] = batch[node_at[valid]].astype(np.float16)
    dv[valid] = dinv[node_at[valid]]
    for c in range(NCORES):
        pl.cores[c]["batchval"] = bv[c].reshape(TILES, P).T.copy()  # [128, 49]
        pl.cores[c]["dinv"] = dv[c].reshape(TILES, P).T.copy()      # [128, 49]
        xt = np.zeros((D_IN, SLOTS), dtype=np.float32)
        v = valid[c]
        xt[:, v] = np.asarray(x, dtype=np.float32)[node_at[c][v]].T
        pl.cores[c]["xT"] = xt.astype(BF16_NP)

    pl.rowmap = np.arange(RAW, dtype=np.int64).reshape(NCORES, SLOTS)
    pl.counts = np.bincount(batch, minlength=NGRAPH).astype(np.float32)
    pl.iota_rep = np.repeat(np.arange(P), pl.NCHMAX).astype(np.float16).reshape(1, -1)
    pl.giota = np.repeat(np.arange(NGRAPH), TILES).astype(np.float16).reshape(1, -1)
    pl.key = (tuple(CLO), tuple(CHI))
    return pl


# ---------------------------------------------------------- program builders --

def _make_gemm_emitter(nc, ctx, tc, k_tiles_fn, o_T, dinv_sb, bufs=2,
                       to_bufs=None, identB=None):
    """Returns emit(t): table rows for slot tile t.

    out[slot, fout] = sum_k lhsT_k^T @ rhs_k with lhsT = feat-major input
    block (no output transpose needed); dinv-scale + fp16 cast -> o_T rows.
    """
    gps_pool = ctx.enter_context(
        tc.tile_pool(name="gemm_ps", bufs=bufs, space="PSUM"))
    to_pool = ctx.enter_context(
        tc.tile_pool(name="gemm_to", bufs=to_bufs or 2))
    WB = 4  # tiles per table-write DMA (amortizes the 625ns HWDGE slot)
    state = {}
    if identB is not None:
        pre_pool = ctx.enter_context(tc.tile_pool(name="gemm_pre", bufs=2))
        tp2_pool = ctx.enter_context(
            tc.tile_pool(name="gemm_tp2", bufs=1, space="PSUM"))

    def emit(t):
        kt = k_tiles_fn(t)
        gps = gps_pool.tile([P, H], F32, space="PSUM")
        for ki, (lhsT, rhs) in enumerate(kt):
            nc.tensor.matmul(out=gps[:], lhsT=lhsT, rhs=rhs,
                             start=(ki == 0), stop=(ki == len(kt) - 1))
        j = t % WB
        if j == 0:
            to_new = to_pool.tile([P, WB, H], F16, tag="to")
            state["to"] = to_new
        to = state["to"]
        # pad slots have dinv == 0, so this scale also keeps their table rows
        # ZERO (they serve as the gather targets for chunk padding positions)
        if identB is None:
            # row-major table rows: [slot, feat] -> o_T[SLOTS, H]
            nc.scalar.activation(out=to[:, j, :], in_=gps[:],
                                 func=mybir.ActivationFunctionType.Copy,
                                 scale=dinv_sb[:, t:t + 1])
            if j == WB - 1 or t == TILES - 1:
                t0, n = t - j, j + 1
                dst = o_T[t0 * P:(t0 + n) * P, :].rearrange(
                    "(j p) h -> p j h", j=n, p=P)
                nc.sync.dma_start(out=dst, in_=to[:, 0:n, :])
        else:
            # transposed table out [H, SLOTS]: the write is then contiguous
            # per partition (1KB runs, no sub-512B DMA penalty); host
            # transposes back during table assembly (free)
            pre = pre_pool.tile([P, H], BF16)
            nc.scalar.activation(out=pre[:], in_=gps[:],
                                 func=mybir.ActivationFunctionType.Copy,
                                 scale=dinv_sb[:, t:t + 1])
            tp2 = tp2_pool.tile([P, P], BF16, space="PSUM")
            nc.tensor.transpose(out=tp2[:], in_=pre[:], identity=identB[:])
            nc.scalar.activation(out=to[:, j, :], in_=tp2[:],
                                 func=mybir.ActivationFunctionType.Copy)
            if j == WB - 1 or t == TILES - 1:
                t0, n = t - j, j + 1
                dst = o_T[:, t0 * P:(t0 + n) * P].rearrange(
                    "f (j p) -> f j p", j=n, p=P)
                nc.sync.dma_start(out=dst, in_=to[:, 0:n, :])

    return emit


def _build_A(pl):
    nc = bacc.Bacc("TRN2", target_bir_lowering=False, debug=False, num_devices=NCORES)
    i_xT = nc.dram_tensor("xT", [D_IN, SLOTS], BF16, kind="ExternalInput").ap()
    i_W = nc.dram_tensor("W", [D_IN, H], BF16, kind="ExternalInput").ap()
    i_dinv = nc.dram_tensor("dinv", [P, TILES], F32, kind="ExternalInput").ap()
    o_T = nc.dram_tensor("Tout", [SLOTS, H], F16, kind="ExternalOutput").ap()
    with tile.TileContext(nc) as tc:
        with ExitStack() as ctx:
            const = ctx.enter_context(tc.tile_pool(name="const", bufs=1))
            dinv_sb = const.tile([P, TILES], F32)
            nc.sync.dma_start(out=dinv_sb[:], in_=i_dinv[:])
            w0 = const.tile([P, H], BF16)
            nc.sync.dma_start(out=w0[:], in_=i_W[0:P, :])
            w1 = const.tile([P, H], BF16)
            nc.sync.dma_start(out=w1[:], in_=i_W[P:2 * P, :])
            x0 = const.tile([P, SLOTS], BF16)
            x1 = const.tile([P, SLOTS], BF16)
            XCH = 784   # SLOTS/8: early chunks unblock the first tiles' GEMMs
            for o in range(0, SLOTS, XCH):
                w = min(XCH, SLOTS - o)
                nc.sync.dma_start(out=x0[:, o:o + w], in_=i_xT[0:P, o:o + w])
                nc.sync.dma_start(out=x1[:, o:o + w], in_=i_xT[P:2 * P, o:o + w])

            def k_tiles(t):
                sl = slice(t * P, (t + 1) * P)
                return [(x0[:, sl], w0[:]), (x1[:, sl], w1[:])]

            emit = _make_gemm_emitter(nc, ctx, tc, k_tiles, o_T, dinv_sb, bufs=6,
                                      to_bufs=13)
            for t in range(TILES):
                emit(t)
    nc.compile()
    return nc


def _scatter_body(nc, ctx, tc, pl, i_T, consume_tile, after_tile=None,
                  mid_loads=None):
    """Shared gather + one-hot matmul scatter loop.

    consume_tile(t, ypsum) handles the per-tile PSUM result
    (ypsum = sum over in-edges of dinv[src]-scaled source rows, incl self-loop).
    """
    const = ctx.enter_context(tc.tile_pool(name="sc_const", bufs=1))
    stage = ctx.enter_context(tc.tile_pool(name="staging", bufs=2))
    st_pool = ctx.enter_context(tc.tile_pool(name="st", bufs=4))
    yp_pool = ctx.enter_context(tc.tile_pool(name="yps", bufs=3, space="PSUM"))

    i_idxlo = nc.dram_tensor("idxlo", [P, pl.lo_cols], I16, kind="ExternalInput").ap()
    i_idxhi = nc.dram_tensor("idxhi", [P, pl.hi_cols], I16, kind="ExternalInput").ap()
    i_dstloc = nc.dram_tensor("dstloc", [P, pl.CTOT], F16, kind="ExternalInput").ap()
    i_iota = nc.dram_tensor("iota_rep", [1, P * pl.NCHMAX], F16,
                            kind="ExternalInput").ap()
    i_ownT = nc.dram_tensor("ownT", [H, SLOTS], F16, kind="ExternalInput").ap()

    idxlo_sb = const.tile([P, pl.lo_cols], I16)
    nc.sync.dma_start(out=idxlo_sb[:], in_=i_idxlo[:])
    idxhi_sb = const.tile([P, pl.hi_cols], I16)
    nc.sync.dma_start(out=idxhi_sb[:], in_=i_idxhi[:])
    dstloc_sb = const.tile([P, pl.CTOT], F16)
    nc.sync.dma_start(out=dstloc_sb[:], in_=i_dstloc[:])
    iota_sb = const.tile([P, P * pl.NCHMAX], F16)
    nc.sync.dma_start(out=iota_sb[:], in_=i_iota.to_broadcast([P, P * pl.NCHMAX]))
    iota3 = iota_sb[:].rearrange("p (j c) -> p j c", j=P, c=pl.NCHMAX)
    identH = const.tile([P, P], F16)
    make_identity(nc, identH[:])
    ownT_sb = const.tile([P, SLOTS], F16)
    nc.sync.dma_start(out=ownT_sb[:], in_=i_ownT[:])
    if mid_loads is not None:
        # non-scatter-critical input loads go AFTER the idx/ownT loads so the
        # first gather is not stuck behind their fixed HWDGE slots
        mid_loads()

    qn = [0]

    def gather(staging, base, src_ap, idx_sb, col0, nch):
        for o in range(0, nch, MAXCH):
            n = min(MAXCH, nch - o)
            c0 = col0 + o * 8
            nc.gpsimd.dma_gather(
                out_ap=staging[:, base + o:base + o + n, :], in_ap=src_ap,
                idxs_ap=idx_sb[:, c0:c0 + n * 8],
                num_idxs=n * P, num_idxs_reg=n * P, elem_size=H,
                queue_num=qn[0])
            qn[0] = (qn[0] + 1) % NQ

    def onehot(gc0, nch):
        st = st_pool.tile([P, P, nch], F16, tag="st")
        nc.vector.tensor_tensor(
            out=st[:],
            in0=iota3[:, :, 0:nch],
            in1=dstloc_sb[:, gc0:gc0 + nch].unsqueeze(1).to_broadcast([P, P, nch]),
            op=mybir.AluOpType.is_equal)
        return st

    for g, grp in enumerate(pl.groups):
        nclo, nchi = grp["nclo"], grp["nchi"]
        staging = stage.tile([P, nclo + nchi, H], F16, tag="staging")
        gather(staging, 0, i_T[:], idxlo_sb, grp["lo_col0"], nclo)
        gather(staging, nclo, i_T[HIB:, :], idxhi_sb, grp["hi_col0"], nchi)
        for ti, td in enumerate(grp["tiles"]):
            t = GROUP_T0[g] + ti
            stlo = onehot(td["gc_lo"], td["clo"]) if td["clo"] else None
            sthi = onehot(td["gc_hi"], td["chi"]) if td["chi"] else None
            ypsum = yp_pool.tile([P, H], F32, space="PSUM")
            # self-loop rows: ypsum = ownT_tile^T @ I  (= own rows, [slot, feat])
            nc.tensor.matmul(out=ypsum[:], lhsT=ownT_sb[:, t * P:(t + 1) * P],
                             rhs=identH[:],
                             start=True, stop=(td["clo"] + td["chi"] == 0))
            for i in range(td["clo"]):
                nc.tensor.matmul(
                    out=ypsum[:], lhsT=stlo[:, :, i],
                    rhs=staging[:, td["sp_lo"] + i, :],
                    start=False,
                    stop=(i == td["clo"] - 1 and td["chi"] == 0))
            for i in range(td["chi"]):
                nc.tensor.matmul(
                    out=ypsum[:], lhsT=sthi[:, :, i],
                    rhs=staging[:, td["sp_hi"] + i, :],
                    start=False, stop=(i == td["chi"] - 1))
            consume_tile(t, ypsum)
            if after_tile is not None:
                after_tile(t)


def _vec_input(nc, const, name):
    ap = nc.dram_tensor(name, [H, 1], F32, kind="ExternalInput").ap()
    sb = const.tile([H, 1], F32, tag=f"vec_{name}")
    nc.sync.dma_start(out=sb[:], in_=ap[:])
    return sb


def _build_BC(pl):
    nc = bacc.Bacc("TRN2", target_bir_lowering=False, debug=False,
                   num_devices=NCORES, dynamic_dma_scratch_size=SCRATCH,
                   num_swdge_queues=NQ)
    i_T = nc.dram_tensor("T", [TAB, H], F16, kind="ExternalInput").ap()
    i_W = nc.dram_tensor("W", [H, H], BF16, kind="ExternalInput").ap()
    i_dinv = nc.dram_tensor("dinv", [P, TILES], F32, kind="ExternalInput").ap()
    o_T = nc.dram_tensor("Tout", [H, SLOTS], F16, kind="ExternalOutput").ap()
    with tile.TileContext(nc) as tc:
        with ExitStack() as ctx:
            const = ctx.enter_context(tc.tile_pool(name="bc_const", bufs=1))
            ycp_pool = ctx.enter_context(tc.tile_pool(name="ycp", bufs=3))
            h_pool = ctx.enter_context(tc.tile_pool(name="ht", bufs=3))
            tps_pool = ctx.enter_context(tc.tile_pool(name="tps", bufs=2, space="PSUM"))

            identB = const.tile([P, P], BF16)
            make_identity(nc, identB[:])
            dinv_sb = const.tile([P, TILES], F32)
            w_sb = const.tile([H, H], BF16)
            scale = const.tile([H, 1], F32)
            bias = const.tile([H, 1], F32)

            def mid_loads():
                b_sb = _vec_input(nc, const, "bvec")
                g_sb = _vec_input(nc, const, "bn_g")
                bb_sb = _vec_input(nc, const, "bn_b")
                m_sb = _vec_input(nc, const, "bn_m")
                v_sb = _vec_input(nc, const, "bn_v")
                nc.sync.dma_start(out=dinv_sb[:], in_=i_dinv[:])
                nc.sync.dma_start(out=w_sb[:], in_=i_W[:])
                # scale = g / sqrt(v+eps); bias = (b - m)*scale + beta
                eps = const.tile([H, 1], F32)
                nc.vector.memset(eps[:], BN_EPS)
                sq = const.tile([H, 1], F32)
                nc.scalar.activation(out=sq[:], in_=v_sb[:],
                                     func=mybir.ActivationFunctionType.Sqrt,
                                     bias=eps[:], scale=1.0)
                rs = const.tile([H, 1], F32)
                nc.vector.reciprocal(out=rs[:], in_=sq[:])
                nc.vector.tensor_mul(out=scale[:], in0=rs[:], in1=g_sb[:])
                nc.vector.tensor_sub(out=bias[:], in0=b_sb[:], in1=m_sb[:])
                nc.vector.tensor_mul(out=bias[:], in0=bias[:], in1=scale[:])
                nc.vector.tensor_add(out=bias[:], in0=bias[:], in1=bb_sb[:])

            h_tiles = {}

            def consume(t, ypsum):
                ycp = ycp_pool.tile([P, H], BF16)
                nc.scalar.activation(out=ycp[:], in_=ypsum[:],
                                     func=mybir.ActivationFunctionType.Copy,
                                     scale=dinv_sb[:, t:t + 1])
                tp = tps_pool.tile([P, P], BF16, space="PSUM")
                nc.tensor.transpose(out=tp[:], in_=ycp[:], identity=identB[:])
                h_t = h_pool.tile([P, H], BF16)
                nc.scalar.activation(
                    out=h_t[:], in_=tp[:],
                    func=mybir.ActivationFunctionType.Relu,
                    bias=bias[:], scale=scale[:])
                h_tiles[t] = h_t

            emit = _make_gemm_emitter(nc, ctx, tc,
                                      lambda t: [(h_tiles.pop(t)[:], w_sb[:])],
                                      o_T, dinv_sb, to_bufs=6, identB=identB)

            # emit each tile's GEMM right after its scatter completes so the
            # table write overlaps the remaining scatter instead of tailing it
            _scatter_body(nc, ctx, tc, pl, i_T, consume, emit,
                          mid_loads=mid_loads)
    nc.compile()
    return nc


def _build_D(pl):
    nc = bacc.Bacc("TRN2", target_bir_lowering=False, debug=False,
                   num_devices=NCORES, dynamic_dma_scratch_size=SCRATCH,
                   num_swdge_queues=NQ)
    i_T = nc.dram_tensor("T", [TAB, H], F16, kind="ExternalInput").ap()
    i_bv = nc.dram_tensor("batchval", [P, TILES], F16, kind="ExternalInput").ap()
    i_gi = nc.dram_tensor("giota", [1, NGRAPH * TILES], F16,
                          kind="ExternalInput").ap()
    i_dinv = nc.dram_tensor("dinv", [P, TILES], F32, kind="ExternalInput").ap()
    o_pool = nc.dram_tensor("pool", [NGRAPH, H], F32, kind="ExternalOutput").ap()
    with tile.TileContext(nc) as tc:
        with ExitStack() as ctx:
            const = ctx.enter_context(tc.tile_pool(name="d_const", bufs=1))
            h3_pool = ctx.enter_context(tc.tile_pool(name="h3", bufs=3))
            pp_pool = ctx.enter_context(tc.tile_pool(name="pp", bufs=1, space="PSUM"))

            bv_sb = const.tile([P, TILES], F16)
            gi_sb = const.tile([P, NGRAPH * TILES], F16)
            dinv_sb = const.tile([P, TILES], F32)
            oh_all = const.tile([P, NGRAPH, TILES], F16)
            pp = pp_pool.tile([NGRAPH, H], F32, space="PSUM")

            def mid_loads():
                nc.sync.dma_start(out=bv_sb[:], in_=i_bv[:])
                nc.sync.dma_start(out=gi_sb[:],
                                  in_=i_gi.to_broadcast([P, NGRAPH * TILES]))
                nc.sync.dma_start(out=dinv_sb[:], in_=i_dinv[:])
                # oh_all[p, g, t] = (batchval[p, t] == g)
                nc.vector.tensor_tensor(
                    out=oh_all[:],
                    in0=gi_sb[:].rearrange("p (g t) -> p g t", g=NGRAPH, t=TILES),
                    in1=bv_sb[:].unsqueeze(1).to_broadcast([P, NGRAPH, TILES]),
                    op=mybir.AluOpType.is_equal)

            def consume(t, ypsum):
                h3 = h3_pool.tile([P, H], F16)
                nc.scalar.activation(out=h3[:], in_=ypsum[:],
                                     func=mybir.ActivationFunctionType.Copy,
                                     scale=dinv_sb[:, t:t + 1])
                nc.tensor.matmul(out=pp[:], lhsT=oh_all[:, :, t], rhs=h3[:],
                                 start=(t == 0), stop=(t == TILES - 1))

            _scatter_body(nc, ctx, tc, pl, i_T, consume,
                          mid_loads=mid_loads)
            pcp = const.tile([NGRAPH, H], F32)
            nc.vector.tensor_copy(out=pcp[:], in_=pp[:])
            nc.sync.dma_start(out=o_pool[:], in_=pcp[:])
    nc.compile()
    return nc


# ------------------------------------------------------------------- driver --

def _run(nc, in_maps):
    res = run_bass_kernel_spmd(nc, in_maps, core_ids=list(range(NCORES)),
                               trace=TRACE)
    if TRACE:
        LAST_EXEC_NS.append(res.exec_time_ns)
    return res.results


def _assemble_table(pl, shards):
    T = np.zeros((TAB, H), dtype=np.float16)
    for c in range(NCORES):
        T[pl.rowmap[c]] = shards[c]
    return T


def kernel(**inputs):
    ins = {k: np.asarray(v) for k, v in inputs.items()}
    key = hashlib.sha1(
        ins["edge_index"].tobytes() + ins["batch"].tobytes()
    ).hexdigest()
    if key not in _PLAN_CACHE:
        _PLAN_CACHE[key] = _make_plan(ins["edge_index"], ins["batch"], ins["x"])
    pl = _PLAN_CACHE[key]

    pk = pl.key
    if pk not in _PROG_CACHE:
        _PROG_CACHE[pk] = {
            "A": _build_A(pl),
            "BC": _build_BC(pl),
            "D": _build_D(pl),
        }
    progs = _PROG_CACHE[pk]

    LAST_EXEC_NS.clear()
    W1 = ins["W1"].astype(BF16_NP)
    # Launch A: T1 = dinv * (x @ W1)
    resA = _run(progs["A"], [
        {"xT": pl.cores[c]["xT"], "W": W1, "dinv": pl.cores[c]["dinv"]}
        for c in range(NCORES)
    ])
    shardsA = [r["Tout"] for r in resA]
    T1 = _assemble_table(pl, shardsA)

    def meta(c):
        cc = pl.cores[c]
        return {"idxlo": cc["idxlo"], "idxhi": cc["idxhi"],
                "dstloc": cc["dstloc"], "iota_rep": pl.iota_rep,
                "dinv": cc["dinv"]}

    def ownT(shard):
        return np.ascontiguousarray(shard.T)

    def fromT(shardT):
        return np.ascontiguousarray(shardT.T)

    def vec(name):
        return ins[name].astype(np.float32).reshape(H, 1)

    # Launch B: layer-1 scatter + BN1/ReLU + @W2
    resB = _run(progs["BC"], [
        {**meta(c), "T": T1, "ownT": ownT(shardsA[c]), "W": ins["W2"].astype(BF16_NP),
         "bvec": vec("b1"), "bn_g": vec("bn1_g"), "bn_b": vec("bn1_b"),
         "bn_m": vec("bn1_m"), "bn_v": vec("bn1_v")} for c in range(NCORES)
    ])
    shardsB = [fromT(r["Tout"]) for r in resB]
    T2 = _assemble_table(pl, shardsB)

    # Launch C: layer-2 scatter + BN2/ReLU + @W3
    resC = _run(progs["BC"], [
        {**meta(c), "T": T2, "ownT": ownT(shardsB[c]), "W": ins["W3"].astype(BF16_NP),
         "bvec": vec("b2"), "bn_g": vec("bn2_g"), "bn_b": vec("bn2_b"),
         "bn_m": vec("bn2_m"), "bn_v": vec("bn2_v")} for c in range(NCORES)
    ])
    shardsC = [fromT(r["Tout"]) for r in resC]
    T3 = _assemble_table(pl, shardsC)

    # Launch D: layer-3 scatter + pooling partials
    resD = _run(progs["D"], [
        {**meta(c), "T": T3, "ownT": ownT(shardsC[c]),
         "batchval": pl.cores[c]["batchval"], "giota": pl.giota}
        for c in range(NCORES)
    ])
    pooled_sum = np.sum([r["pool"] for r in resD], axis=0).astype(np.float64)

    counts = pl.counts.astype(np.float64)
    pooled_sum += counts[:, None] * ins["b3"].astype(np.float64)[None, :]
    pooled = pooled_sum / np.maximum(counts, 1.0)[:, None]

    z = np.maximum(pooled @ ins["Wc1"].astype(np.float64)
                   + ins["bc1"].astype(np.float64), 0.0)
    out = z @ ins["Wc2"].astype(np.float64) + ins["bc2"].astype(np.float64)
    return out.astype(np.float32)
